# revision 1
# baseline (speedup 1.0000x reference)
"""Multi-head self-attention TRN2 kernel (B=2, T=2048, E=1024, H=16, D=64).

Sharding: tensor-parallel over heads — each of the 8 cores owns 2 heads.
Because the reference reshapes (B,H,T,D)->(B,T,E) with NO transpose, each
head's attention output maps to 128 complete contiguous rows of the
out_proj input, so the whole computation is embarrassingly parallel
across heads (no collectives).

Per-core pipeline (all matmuls bf16, accumulation fp32):
  1. qT/kT = (W_qk)^T-style projection producing q^T,k^T in [d, T] layout
     (heads stacked on partitions 0-63 / 64-127 -> row-tiled score matmuls).
  2. v in natural [T, d] layout, augmented with a ones column (gives the
     softmax denominator for free as row 64 of the attn@v output).
  3. scores^T tiles [kj=128, qi] -> exp on ScalarE (no max subtraction:
     scores ~ N(0,1), exp is safe in fp32) -> P^T bf16.
  4. attn@v: o^T[d(+denom), qi] accumulated over kj tiles in PSUM.
  5. normalize by reciprocal of denominator row (partition-broadcast).
  6. out_proj decomposed over j (the reshape mixing index): 16 accumulating
     matmuls with strided lhsT slices of o^T -- performs the "faithful
     reshape" for free.
"""

import numpy as np
import ml_dtypes

B, T, E, H, D = 2, 2048, 1024, 16, 64
N_CORES = 8
HL = H // N_CORES          # heads per core = 2
KP = E // 128              # 8 contraction partition-tiles
KT = T // 128              # 16 kj tiles
QC = T // 512              # 4 qi chunks of 512

_RUNNER = None


def _build_nc():
    import concourse.bacc as bacc
    import concourse.tile as tile
    import concourse.bass as bass
    import concourse.mybir as mybir

    fp32 = mybir.dt.float32
    bf16 = mybir.dt.bfloat16
    ADD = mybir.AluOpType.add
    MULT = mybir.AluOpType.mult
    EXP = mybir.ActivationFunctionType.Exp

    nc = bacc.Bacc("TRN2", target_bir_lowering=False, debug=False,
                   enable_asserts=True, num_devices=N_CORES)

    xt_d = nc.dram_tensor("xt", [E, B * T], bf16, kind="ExternalInput").ap()
    wqk_d = nc.dram_tensor("wqk", [E, 4 * D], bf16, kind="ExternalInput").ap()
    wv_d = nc.dram_tensor("wv", [E, 2 * (D + 1)], bf16, kind="ExternalInput").ap()
    wout_d = nc.dram_tensor("wout", [128, 8 * E], bf16, kind="ExternalInput").ap()
    bqk_d = nc.dram_tensor("bqk", [128, 2], fp32, kind="ExternalInput").ap()
    bv_d = nc.dram_tensor("bv", [128, 2 * (D + 1)], fp32, kind="ExternalInput").ap()
    ones_d = nc.dram_tensor("ones", [D + 1, D], fp32, kind="ExternalInput").ap()
    bout_d = nc.dram_tensor("bout", [128, E], fp32, kind="ExternalInput").ap()
    y_d = nc.dram_tensor("y", [B, HL, 128, E], fp32, kind="ExternalOutput").ap()

    with tile.TileContext(nc) as tc:
        with (
            tc.tile_pool(name="const", bufs=1) as cpool,
            tc.tile_pool(name="ppool", bufs=16) as ppool,
            tc.tile_pool(name="npool", bufs=3) as npool,
            tc.tile_pool(name="ypool", bufs=3) as ypool,
            tc.tile_pool(name="ps_s", bufs=2, space=bass.MemorySpace.PSUM) as ps_s,
            tc.tile_pool(name="ps_o", bufs=2, space=bass.MemorySpace.PSUM) as ps_o,
            tc.tile_pool(name="ps_sm", bufs=2, space=bass.MemorySpace.PSUM) as ps_sm,
        ):
            # ---- constants / persistent tiles ----
            xt_sb = cpool.tile([128, KP, B * T], bf16, tag="xt")
            wqk_sb = cpool.tile([128, KP, 4 * D], bf16, tag="wqk")
            wv_sb = cpool.tile([128, KP, 2 * (D + 1)], bf16, tag="wv")
            wout_sb = cpool.tile([128, 8, E], bf16, tag="wout")
            bqk_sb = cpool.tile([128, 2], fp32, tag="bqk")
            bv_sb = cpool.tile([128, 2 * (D + 1)], fp32, tag="bv")
            bout_sb = cpool.tile([128, E], fp32, tag="bout")
            qkT = cpool.tile([128, B, 2, T], bf16, tag="qkT")
            vaug = cpool.tile([128, B, KT, 2 * (D + 1)], bf16, tag="vaug")
            ofull = cpool.tile([128, B, HL, T], bf16, tag="ofull")
            ones_sb = cpool.tile([D + 1, D], fp32, tag="ones")

            # small weights first on the SP ring so the first matmuls
            # aren't queued behind the 8 MiB xt load (on the ACT ring)
            nc.sync.dma_start(wqk_sb[:], wqk_d.rearrange("(a p) n -> p a n", p=128))
            nc.sync.dma_start(wv_sb[:], wv_d.rearrange("(a p) n -> p a n", p=128))
            nc.sync.dma_start(bqk_sb[:], bqk_d[:])
            nc.sync.dma_start(bv_sb[:], bv_d[:])
            nc.sync.dma_start(ones_sb[:], ones_d[:])
            # xt split by T-columns: the first qk-proj chunk only needs the
            # first 512 columns (1 MiB) instead of the whole 8 MiB
            xt_r = xt_d.rearrange("(a p) n -> p a n", p=128)
            for cc in range(B * T // 512):
                nc.scalar.dma_start(xt_sb[:, :, cc * 512:(cc + 1) * 512],
                                    xt_r[:, :, cc * 512:(cc + 1) * 512])
            def proj_qk_m(b, n, m):
                ps = ps_sm.tile([128, 512], fp32, tag="sm", name="ps")
                for k in range(KP):
                    nc.tensor.matmul(
                        ps[:],
                        wqk_sb[:, k, m * 128:(m + 1) * 128],
                        xt_sb[:, k, b * T + n * 512: b * T + (n + 1) * 512],
                        start=(k == 0), stop=(k == KP - 1),
                    )
                nc.vector.tensor_scalar(
                    qkT[:, b, m, n * 512:(n + 1) * 512], ps[:],
                    bqk_sb[:, m:m + 1], None, op0=ADD,
                )

            def proj_qk(b, n):
                # q^T / k^T chunk n: [128=(h0|h1)*d, 512]
                for m in range(2):
                    proj_qk_m(b, n, m)

            def proj_v(b, r):
                # v natural [T, 2*(D+1)] row tile r; W_v has zero columns at
                # the two "ones" slots and bv carries 1.0 there
                vp = ps_sm.tile([128, 512], fp32, tag="sm", name="vp")
                for k in range(KP):
                    nc.tensor.matmul(
                        vp[:, 0:2 * (D + 1)],
                        xt_sb[:, k, b * T + r * 128: b * T + (r + 1) * 128],
                        wv_sb[:, k, :],
                        start=(k == 0), stop=(k == KP - 1),
                    )
                nc.vector.tensor_tensor(
                    vaug[:, b, r, :], vp[:, 0:2 * (D + 1)], bv_sb[:], op=ADD,
                )

            def proj(b):
                for n in range(QC):
                    proj_qk(b, n)
                for r in range(KT):
                    proj_v(b, r)

            def sc(b, qc, kt):
                # both heads row-tiled (partitions 0-63 / 64-127) so the
                # two K=64 matmuls run concurrently in the PE array
                S = ps_s.tile([128, 2 * 512], fp32, tag="S", name="S")
                for h in range(HL):
                    nc.tensor.matmul(
                        S[:, h * 512:(h + 1) * 512],
                        qkT[h * D:(h + 1) * D, b, 1, kt * 128:(kt + 1) * 128],
                        qkT[h * D:(h + 1) * D, b, 0, qc * 512:(qc + 1) * 512],
                        start=True, stop=True,
                    )
                return S

            def norm_h(b, qc, os_, h):
                rc = npool.tile([D + 1, 512], fp32, tag="rc", name="rc")
                nc.vector.reciprocal(rc[D:D + 1, :], os_[h][D:D + 1, :])
                # broadcast recip row (partition 64) to partitions
                # 0-63 via a K=1 PE matmul
                rbp = ps_sm.tile([D, 512], fp32, tag="sm", name=f"rbp{h}")
                nc.tensor.matmul(rbp[:], ones_sb[D:D + 1, :],
                                 rc[D:D + 1, :], start=True, stop=True)
                rb = npool.tile([D, 512], fp32, tag="rb", name="rb")
                nc.vector.tensor_copy(rb[:], rbp[:])
                nc.vector.tensor_tensor(
                    ofull[0:D, b, h, qc * 512:(qc + 1) * 512],
                    os_[h][0:D, :], rb[:], op=MULT,
                )

            def norm(b, qc, os_):
                for h in range(HL):
                    norm_h(b, qc, os_, h)

            def dup_h(b, h):
                # partitions 64-127 := partitions 0-63 shifted left one qi
                # element, so a single rectangular lhsT AP serves both
                # j-parities in the paired out_proj matmuls
                nc.sync.dma_start(ofull[D:128, b, h, 0:T - 1],
                                  ofull[0:D, b, h, 1:T])

            def outproj_n5(b, h, n5):
                of2 = ofull[:, b, h, :].rearrange("p (t j) -> p j t", j=16)
                yp = ps_sm.tile([128, 512], fp32, tag="sm", name="yp")
                for jj in range(8):
                    nc.tensor.matmul(
                        yp[:],
                        of2[:, 2 * jj, :],
                        wout_sb[:, jj, n5 * 512:(n5 + 1) * 512],
                        start=(jj == 0), stop=(jj == 7),
                    )
                ys = ypool.tile([128, 512], fp32, tag="ys", name="ys")
                nc.vector.tensor_tensor(
                    ys[:], yp[:], bout_sb[:, n5 * 512:(n5 + 1) * 512], op=ADD,
                )
                nc.sync.dma_start(y_d[b, h, :, n5 * 512:(n5 + 1) * 512], ys[:])

            def outproj(b, h):
                for n5 in range(2):
                    outproj_n5(b, h, n5)

            import os as _os
            _reps = int(_os.environ.get("KERNEL_EMIT_REPS", "1"))
            # ---- unified emission: one flat loop over (b, qc, kt) with a
            # slot-scheduled filler map.  Minimal prologue: first qk chunk
            # + first v rows of b0; everything else (rest of proj(b0),
            # proj(b1), weight DMAs, out_proj(b0)) is emitted as PE-filler
            # at specific (b,qc,kt) slots inside the ACT-bound phase.
            # Emission order IS dependency order: each filler piece must be
            # emitted before the consumer that reads its output.
            def _emit_all():
                proj_qk(0, 0)
                _emit_rest()

            def _q(b, n):
                return lambda: proj_qk_m(b, n, 0)

            def _k(b, n):
                return lambda: proj_qk_m(b, n, 1)

            def _qk(b, n):
                return lambda: proj_qk(b, n)

            def _v(b, r0):
                return lambda: [proj_v(b, r) for r in range(r0, r0 + 4)]

            # fine-grained filler pieces (~1.7us each) so the scores
            # lookahead is never stalled behind a long piece.  k-chunks
            # first (they gate the exp stream: sc(kt) needs k cols kt//4);
            # q-chunks only gate qc boundaries; v-proj is deferred -- vmm
            # emission waits on the watermark, PSUM accumulation order
            # doesn't matter
            SCHED = {
                (0, 0, 1): _qk(0, 1),
                (0, 0, 2): _qk(0, 2),
                (0, 0, 3): _qk(0, 3),
                (0, 0, 5): _v(0, 0),
                (0, 0, 7): _v(0, 4),
                (0, 0, 9): _v(0, 8),
                (0, 0, 11): _v(0, 12),
                (0, 0, 13): _qk(1, 0),
                (0, 0, 15): _qk(1, 1),
                (0, 1, 1): _qk(1, 2),
                (0, 1, 3): _qk(1, 3),
                (0, 1, 5): _v(1, 0),
                (0, 1, 9): _v(1, 4),
                (0, 2, 5): _v(1, 8),
                (0, 2, 9): _v(1, 12),
                # out_proj weights loaded once startup DMA traffic is done
                (0, 2, 1): lambda: nc.sync.dma_start(wout_sb[:], wout_d[:]),
                (0, 2, 3): lambda: nc.sync.dma_start(bout_sb[:], bout_d[:]),
                # out_proj(b0) pieces inside attn(b1)'s ACT-bound phase
                (1, 0, 1): lambda: outproj_n5(0, 0, 0),
                (1, 0, 9): lambda: outproj_n5(0, 0, 1),
                (1, 1, 1): lambda: outproj_n5(0, 1, 0),
                (1, 1, 9): lambda: outproj_n5(0, 1, 1),
            }
            # slots that advance the "vaug rows emitted" watermark
            V_SLOTS = {
                (0, 0, 5): (0, 0), (0, 0, 7): (0, 4),
                (0, 0, 9): (0, 8), (0, 0, 11): (0, 12),
                (0, 1, 5): (1, 0), (0, 1, 9): (1, 4),
                (0, 2, 5): (1, 8), (0, 2, 9): (1, 12),
            }

            def _emit_rest():
                seq = [(b, qc, kt) for b in range(B) for qc in range(QC)
                       for kt in range(KT)]
                S = sc(*seq[0])
                os_all = {}
                vaug_rows = {0: 0, 1: 0}   # vaug row tiles emitted so far
                pend_vmm = []              # [(b, qc, kt, P-tile), ...]
                nvmm = {}                  # (b,qc) -> vmms emitted

                def flush_vmm():
                    rest = []
                    for (vb, vqc, vkt, vP) in pend_vmm:
                        if vkt < vaug_rows[vb]:
                            n = nvmm.get((vb, vqc), 0)
                            for h in range(HL):
                                nc.tensor.matmul(
                                    os_all[(vb, vqc)][h][:],
                                    vaug[:, vb, vkt,
                                         h * (D + 1):(h + 1) * (D + 1)],
                                    vP[:, h * 512:(h + 1) * 512],
                                    start=(n == 0), stop=(n == KT - 1),
                                )
                            nvmm[(vb, vqc)] = n + 1
                        else:
                            rest.append((vb, vqc, vkt, vP))
                    pend_vmm[:] = rest

                for i, (b, qc, kt) in enumerate(seq):
                    P = ppool.tile([128, 2 * 512], bf16, tag="P")
                    nc.scalar.activation(P[:], S[:], EXP, scale=0.125)
                    # emit next scores first (also across qc/b boundaries)
                    # so ACT stays fed back-to-back
                    if i + 1 < len(seq):
                        S = sc(*seq[i + 1])
                    if kt == 0 and i > 0:
                        pb, pqc, _ = seq[i - 1]
                        assert not any(x[0] == pb and x[1] == pqc
                                       for x in pend_vmm)
                        dup = (pb, pqc) == (0, QC - 1)
                        for h in range(HL):
                            norm_h(pb, pqc, os_all[(pb, pqc)], h)
                            if dup:
                                dup_h(0, h)
                        os_all.pop((pb, pqc))
                    piece = SCHED.get((b, qc, kt))
                    if piece is not None:
                        piece()
                        if (b, qc, kt) in V_SLOTS:
                            vb, r0 = V_SLOTS[(b, qc, kt)]
                            vaug_rows[vb] = r0 + 4
                    if kt == 0:
                        os_all[(b, qc)] = [
                            ps_o.tile([D + 1, 512], fp32, tag="o",
                                      name=f"o{h}")
                            for h in range(HL)]
                    pend_vmm.append((b, qc, kt, P))
                    flush_vmm()
                flush_vmm()
                assert not pend_vmm
                # tail: each head's dup DMA starts right after its own
                # normalize (h0's transfer overlaps h1's normalize chain)
                os_last = os_all.pop((B - 1, QC - 1))
                for h in range(HL):
                    norm_h(B - 1, QC - 1, os_last, h)
                    dup_h(1, h)
                # keep the PE HAM window busy across the norm/dup wait so
                # the final out_proj runs at full clock instead of cold
                wps = ps_sm.tile([D, D], fp32, tag="sm", name="wps")
                for i in range(24):
                    nc.tensor.matmul(wps[:], ones_sb[D:D + 1, :],
                                     ones_sb[D:D + 1, :],
                                     start=(i == 0), stop=(i == 23))
                for h in range(HL):
                    outproj(1, h)

            for _rep in range(_reps):
                _emit_all()

    nc.compile()
    return nc


def _get_runner():
    """Build + compile once; return a callable(in_maps) -> list of out dicts."""
    global _RUNNER
    if _RUNNER is not None:
        return _RUNNER

    import jax
    import concourse.mybir as mybir
    from concourse import bass2jax
    from jax.experimental.shard_map import shard_map
    from jax.sharding import Mesh, PartitionSpec

    nc = _build_nc()
    bass2jax.install_neuronx_cc_hook()

    partition_name = (nc.partition_id_tensor.name
                      if nc.partition_id_tensor else None)
    in_names, out_names, out_avals = [], [], []
    for alloc in nc.m.functions[0].allocations:
        if not isinstance(alloc, mybir.MemoryLocationSet):
            continue
        name = alloc.memorylocations[0].name
        if alloc.kind == "ExternalInput":
            if name != partition_name:
                in_names.append(name)
        elif alloc.kind == "ExternalOutput":
            out_names.append(name)
            out_avals.append(jax.core.ShapedArray(
                tuple(alloc.tensor_shape), mybir.dt.np(alloc.dtype)))

    n_params, n_outs = len(in_names), len(out_avals)
    all_names = in_names + out_names
    if partition_name is not None:
        all_names = all_names + [partition_name]

    def _body(*args):
        operands = list(args)
        if partition_name is not None:
            operands.append(bass2jax.partition_id_tensor())
        outs = bass2jax._bass_exec_p.bind(
            *operands,
            out_avals=tuple(out_avals),
            in_names=tuple(all_names),
            out_names=tuple(out_names),
            lowering_input_output_aliases=(),
            sim_require_finite=True,
            sim_require_nnan=True,
            nc=nc,
        )
        return tuple(outs)

    devices = jax.devices()[:N_CORES]
    mesh = Mesh(np.asarray(devices), ("core",))
    in_specs = (PartitionSpec("core"),) * (n_params + n_outs)
    out_specs = (PartitionSpec("core"),) * n_outs
    donate = tuple(range(n_params, n_params + n_outs))
    sharded = jax.jit(
        shard_map(_body, mesh=mesh, in_specs=in_specs, out_specs=out_specs,
                  check_rep=False),
        donate_argnums=donate, keep_unused=True,
    )

    def run(in_maps):
        concat_in = [
            np.concatenate([np.asarray(in_maps[c][nm]) for c in range(N_CORES)],
                           axis=0)
            for nm in in_names
        ]
        concat_zeros = [
            np.zeros((N_CORES * a.shape[0], *a.shape[1:]), a.dtype)
            for a in out_avals
        ]
        out_arrs = sharded(*concat_in, *concat_zeros)
        return [
            {nm: np.asarray(out_arrs[i]).reshape(N_CORES, *out_avals[i].shape)[c]
             for i, nm in enumerate(out_names)}
            for c in range(N_CORES)
        ]

    _RUNNER = run
    run._bench_parts = (sharded, mesh, in_names, out_names, out_avals,
                        n_params, _body)
    return run


def _make_bench(in_maps):
    """Device-resident benchmark closure: returns fn() that runs one
    execution with all inputs already on device (no donation)."""
    import jax
    from jax.experimental.shard_map import shard_map
    from jax.sharding import NamedSharding, PartitionSpec

    run = _get_runner()
    sharded, mesh, in_names, out_names, out_avals, n_params, _body = \
        run._bench_parts
    sh = NamedSharding(mesh, PartitionSpec("core"))

    nodonate = jax.jit(
        shard_map(_body, mesh=mesh,
                  in_specs=(PartitionSpec("core"),) * (n_params + len(out_avals)),
                  out_specs=(PartitionSpec("core"),) * len(out_avals),
                  check_rep=False),
        keep_unused=True,
    )
    concat_in = [
        np.concatenate([np.asarray(in_maps[c][nm]) for c in range(N_CORES)], axis=0)
        for nm in in_names
    ]
    concat_zeros = [
        np.zeros((N_CORES * a.shape[0], *a.shape[1:]), a.dtype) for a in out_avals
    ]
    dev_args = [jax.device_put(a, sh) for a in concat_in + concat_zeros]
    for a in dev_args:
        a.block_until_ready()

    def bench_once():
        outs = nodonate(*dev_args)
        for o in outs:
            o.block_until_ready()
        return outs

    def make_bench_k(k):
        n_in = len(in_names)

        def _body_k(*args):
            ins = list(args[:n_in])
            zs = list(args[n_in:])
            for _ in range(k):
                zs = list(_body(*ins, *zs))
            return tuple(zs)

        jk = jax.jit(
            shard_map(_body_k, mesh=mesh,
                      in_specs=(PartitionSpec("core"),) * len(dev_args),
                      out_specs=(PartitionSpec("core"),) * len(out_avals),
                      check_rep=False),
            keep_unused=True,
        )

        def run_k():
            outs = jk(*dev_args)
            for o in outs:
                o.block_until_ready()
            return outs

        return run_k

    bench_once.make_bench_k = make_bench_k
    bench_once.nodonate = nodonate
    bench_once.dev_args = dev_args
    return bench_once


def _prep_in_maps(x, W_qkv, b_qkv, W_out, b_out):
    bf = ml_dtypes.bfloat16
    xt = np.ascontiguousarray(
        x.reshape(B * T, E).T).astype(bf)                      # [E, B*T]
    wout = np.ascontiguousarray(
        W_out.reshape(8, 128, E).transpose(1, 0, 2).reshape(128, 8 * E)).astype(bf)
    bout = np.ascontiguousarray(
        np.broadcast_to(b_out.astype(np.float32)[None, :], (128, E)))

    in_maps = []
    for c in range(N_CORES):
        hs = [HL * c + i for i in range(HL)]
        qcols = np.concatenate(
            [W_qkv[:, 0 * E + h * D:0 * E + (h + 1) * D] for h in hs], axis=1)
        kcols = np.concatenate(
            [W_qkv[:, 1 * E + h * D:1 * E + (h + 1) * D] for h in hs], axis=1)
        wqk = np.ascontiguousarray(
            np.concatenate([qcols, kcols], axis=1)).astype(bf)  # [E, 256]
        zcol = np.zeros((E, 1), np.float32)
        wv = np.ascontiguousarray(np.concatenate(
            [arr for h in hs
             for arr in (W_qkv[:, 2 * E + h * D:2 * E + (h + 1) * D], zcol)],
            axis=1)).astype(bf)                                 # [E, 130]
        bq = np.concatenate([b_qkv[0 * E + h * D:0 * E + (h + 1) * D] for h in hs])
        bk = np.concatenate([b_qkv[1 * E + h * D:1 * E + (h + 1) * D] for h in hs])
        bqk = np.ascontiguousarray(
            np.stack([bq, bk], axis=1)).astype(np.float32)      # [128, 2]
        one = np.ones(1, np.float32)
        bvv = np.concatenate(
            [a for h in hs
             for a in (b_qkv[2 * E + h * D:2 * E + (h + 1) * D], one)])
        bv = np.ascontiguousarray(
            np.broadcast_to(bvv.astype(np.float32)[None, :], (128, 2 * (D + 1))))
        in_maps.append({
            "xt": xt, "wqk": wqk, "wv": wv, "wout": wout,
            "bqk": bqk, "bv": bv, "bout": bout,
            "ones": np.ones((D + 1, D), np.float32),
        })
    return in_maps


def kernel(x, W_qkv, b_qkv, W_out, b_out):
    x = np.asarray(x, dtype=np.float32)
    W_qkv = np.asarray(W_qkv, dtype=np.float32)
    b_qkv = np.asarray(b_qkv, dtype=np.float32)
    W_out = np.asarray(W_out, dtype=np.float32)
    b_out = np.asarray(b_out, dtype=np.float32)

    run = _get_runner()
    in_maps = _prep_in_maps(x, W_qkv, b_qkv, W_out, b_out)
    results = run(in_maps)

    out = np.empty((B, T, E), np.float32)
    for c in range(N_CORES):
        y = results[c]["y"]          # [B, HL, 128, E]
        for hl in range(HL):
            hg = HL * c + hl
            out[:, hg * 128:(hg + 1) * 128, :] = y[:, hl]
    return out



# revision 3
# speedup vs baseline: 1.1761x; 1.1761x over previous
"""Multi-head self-attention TRN2 kernel (B=2, T=2048, E=1024, H=16, D=64).

Sharding: tensor-parallel over heads - each of the 8 cores owns 2 heads.
Because the reference reshapes (B,H,T,D)->(B,T,E) with NO transpose, each
head's attention output maps to 128 complete contiguous rows of the
out_proj input, so the whole computation is embarrassingly parallel
across heads (no collectives).

Per-core pipeline (all matmuls bf16, accumulation fp32):
  1. qT/kT projections in [d, T] layout (heads stacked on partitions
     0-63 / 64-127); v in natural [T, d] layout augmented with a ones
     column (softmax denominator for free).
  2. scores^T tiles [kj=128, qi=512x2h] -> exp on ScalarE -> P^T bf16.
  3. attn@v in the cheap orientation: out o[q=128 partitions, d+1=65
     free] accumulated over kj tiles - PE cost 65 free/step instead of
     512 (the cost model charges output free size per accumulation
     step, so small-free x many-partitions wins 8x per instruction).
  4. normalize by per-partition (per-query) reciprocal on VectorE,
     downcast to bf16.
  5. o -> o^T via DMA xbar transposes (off the PE critical path), then
     shifted partition-duplicate copies build the [128=(d,j-parity), T]
     lhsT layout that performs the reference's "faithful reshape" for
     free in out_proj.
  6. out_proj: 8 accumulating matmuls per 512-column chunk.
"""

import numpy as np
import ml_dtypes

B, T, E, H, D = 2, 2048, 1024, 16, 64
N_CORES = 8
HL = H // N_CORES          # heads per core = 2
KP = E // 128              # 8 contraction partition-tiles
KT = T // 128              # 16 kj tiles
QC = T // 512              # 4 qi chunks of 512
QS = 4                     # q subtiles of 128 per chunk
DV = D + 1                 # v width incl. denominator ones column

_RUNNER = None


def _build_nc():
    import concourse.bacc as bacc
    import concourse.tile as tile
    import concourse.bass as bass
    import concourse.mybir as mybir

    from concourse import masks

    fp32 = mybir.dt.float32
    bf16 = mybir.dt.bfloat16
    ADD = mybir.AluOpType.add
    MULT = mybir.AluOpType.mult
    EXP = mybir.ActivationFunctionType.Exp
    COPY = mybir.ActivationFunctionType.Copy

    nc = bacc.Bacc("TRN2", target_bir_lowering=False, debug=False,
                   enable_asserts=True, num_devices=N_CORES)

    xt_d = nc.dram_tensor("xt", [E, B * T], bf16, kind="ExternalInput").ap()
    wqk_d = nc.dram_tensor("wqk", [E, 4 * D], bf16, kind="ExternalInput").ap()
    wv_d = nc.dram_tensor("wv", [E, 2 * DV], bf16, kind="ExternalInput").ap()
    wout_d = nc.dram_tensor("wout", [128, 8 * E], bf16, kind="ExternalInput").ap()
    bqk_d = nc.dram_tensor("bqk", [128, 2], fp32, kind="ExternalInput").ap()
    bv_d = nc.dram_tensor("bv", [128, 2 * DV], fp32, kind="ExternalInput").ap()
    bout_d = nc.dram_tensor("bout", [128, E], fp32, kind="ExternalInput").ap()
    y_d = nc.dram_tensor("y", [B, HL, 128, E], fp32, kind="ExternalOutput").ap()

    with tile.TileContext(nc) as tc:
        with (
            tc.tile_pool(name="const", bufs=1) as cpool,
            tc.tile_pool(name="ppool", bufs=22) as ppool,
            tc.tile_pool(name="npool", bufs=4) as npool,
            tc.tile_pool(name="ypool", bufs=6) as ypool,
            tc.tile_pool(name="ps_s", bufs=2, space=bass.MemorySpace.PSUM) as ps_s,
            tc.tile_pool(name="ps_o", bufs=1, space=bass.MemorySpace.PSUM) as ps_o,
            tc.tile_pool(name="ps_sm", bufs=2, space=bass.MemorySpace.PSUM) as ps_sm,
        ):
            # ---- constants / persistent tiles ----
            xt_sb = cpool.tile([128, KP, B * T], bf16, tag="xt")
            wqk_sb = cpool.tile([128, KP, 4 * D], bf16, tag="wqk")
            wv_sb = cpool.tile([128, KP, 2 * DV], bf16, tag="wv")
            wout_sb = cpool.tile([128, 8, E], bf16, tag="wout")
            bqk_sb = cpool.tile([128, 2], fp32, tag="bqk")
            bv_sb = cpool.tile([128, 2 * DV], fp32, tag="bv")
            bout_sb = cpool.tile([128, E], fp32, tag="bout")
            qkT = cpool.tile([128, B, 2, T], bf16, tag="qkT")
            vaug = cpool.tile([128, B, KT, 2 * DV], bf16, tag="vaug")
            # o natural layout, normalized bf16, 64 pad cols per head slot
            # (the pad transposes into oTd rows 64-127, which the shifted
            # dup copy then overwrites)
            # per-batch tensors (separate tags) so cross-batch false
            # dependencies cannot arise from coarse subtile tracking
            o_sb = [cpool.tile([128, KT, 192], bf16, tag=f"o_sb{bb}",
                               name=f"o_sb{bb}") for bb in range(B)]
            # o^T dup layout per head: rows 0-63 straight (written directly
            # by the xbar transpose), 64-127 shifted by one q (dup DMA)
            oTd = [cpool.tile([128, HL, T], bf16, tag=f"oTd{bb}",
                              name=f"oTd{bb}") for bb in range(B)]
            wub = cpool.tile([1, 128], bf16, tag="wub")
            ones1 = cpool.tile([1, 128], bf16, tag="ones1")
            bout_bf = cpool.tile([1, E], bf16, tag="bout_bf")
            ident = cpool.tile([128, 128], bf16, tag="ident")

            # warmup source (no DMA dependency); zero o_sb's pad lanes once
            nc.gpsimd.memset(wub[:], 0.0)
            nc.gpsimd.memset(ones1[:], 1.0)
            for bb in range(B):
                nc.vector.memset(o_sb[bb][:], 0.0)
            masks.make_identity(nc, ident[:])

            # ---- input DMAs: spread across the three HWDGE-ish rings so
            # queue overheads pipeline.  Transfers serialize on the shared
            # DMA engines; order = priority.  The first score tile needs
            # k cols + q[0:512], so those pieces go first, smallest first.
            wqk_r = wqk_d.rearrange("(a p) n -> p a n", p=128)
            xt_r = xt_d.rearrange("(a p) n -> p a n", p=128)
            nc.sync.dma_start(wqk_sb[:], wqk_r[:])
            # first 512 T-columns on the ACT ring (idle until exps start);
            # bigger slices keep the 1KB contiguous runs (no small-elem
            # DMA penalty)
            nc.scalar.dma_start(xt_sb[:, :, 0:512], xt_r[:, :, 0:512])
            nc.sync.dma_start(bqk_sb[:], bqk_d[:])
            nc.sync.dma_start(bv_sb[:], bv_d[:])
            nc.gpsimd.dma_start(wv_sb[:], wv_d.rearrange("(a p) n -> p a n", p=128))
            # rest of x on the gpsimd SWDGE ring (engine otherwise idle)
            for cc in range(1, B * T // 512):
                nc.gpsimd.dma_start(xt_sb[:, :, cc * 512:(cc + 1) * 512],
                                    xt_r[:, :, cc * 512:(cc + 1) * 512])

            # ---- PE warmup: keep the clock ramping from t~0 so the first
            # real matmuls run at full p-state.  src defaults to wub (no
            # deps -> the scheduler hoists them to kernel start); pass a
            # late-written AP to pin warmups into a late window instead.
            def warmup(n, src=None):
                s = wub if src is None else src
                for _ in range(n):
                    wt = ps_sm.tile([128, 128], fp32, tag="sm", name="wt")
                    nc.tensor.matmul(wt[:], s[0:1, 0:128], s[0:1, 0:128],
                                     start=True, stop=True)

            # ---- projection pieces ----
            def proj_q(b, n):
                # q^T chunk n: [128=(h0|h1)*d, 512]
                ps = ps_sm.tile([128, 512], fp32, tag="sm", name="psq")
                for k in range(KP):
                    nc.tensor.matmul(
                        ps[:],
                        wqk_sb[:, k, 0:128],
                        xt_sb[:, k, b * T + n * 512: b * T + (n + 1) * 512],
                        start=(k == 0), stop=(k == KP - 1),
                    )
                nc.vector.tensor_scalar(
                    qkT[:, b, 0, n * 512:(n + 1) * 512], ps[:],
                    bqk_sb[:, 0:1], None, op0=ADD,
                )

            def proj_q_half(b, n, hf):
                ps = ps_sm.tile([128, 512], fp32, tag="sm", name="psq")
                c0 = n * 512 + hf * 256
                for k in range(KP):
                    nc.tensor.matmul(
                        ps[:, 0:256],
                        wqk_sb[:, k, 0:128],
                        xt_sb[:, k, b * T + c0: b * T + c0 + 256],
                        start=(k == 0), stop=(k == KP - 1),
                    )
                nc.vector.tensor_scalar(
                    qkT[:, b, 0, c0:c0 + 256], ps[:, 0:256],
                    bqk_sb[:, 0:1], None, op0=ADD,
                )

            def proj_q_qs(b, n, qs):
                ps = ps_sm.tile([128, 512], fp32, tag="sm", name="psq")
                c0 = n * 512 + qs * 128
                for k in range(KP):
                    nc.tensor.matmul(
                        ps[:, 0:128],
                        wqk_sb[:, k, 0:128],
                        xt_sb[:, k, b * T + c0: b * T + c0 + 128],
                        start=(k == 0), stop=(k == KP - 1),
                    )
                nc.vector.tensor_scalar(
                    qkT[:, b, 0, c0:c0 + 128], ps[:, 0:128],
                    bqk_sb[:, 0:1], None, op0=ADD,
                )

            def proj_k(b, kt):
                # k^T tile kt: [128, 128]
                ps = ps_sm.tile([128, 512], fp32, tag="sm", name="psk")
                for k in range(KP):
                    nc.tensor.matmul(
                        ps[:, 0:128],
                        wqk_sb[:, k, 128:256],
                        xt_sb[:, k, b * T + kt * 128: b * T + (kt + 1) * 128],
                        start=(k == 0), stop=(k == KP - 1),
                    )
                nc.vector.tensor_scalar(
                    qkT[:, b, 1, kt * 128:(kt + 1) * 128], ps[:, 0:128],
                    bqk_sb[:, 1:2], None, op0=ADD,
                )

            def proj_v(b, r):
                # v natural [128, 2*DV] row tile r; W_v has zero columns at
                # the two "ones" slots and bv carries 1.0 there
                vp = ps_sm.tile([128, 512], fp32, tag="sm", name="vp")
                for k in range(KP):
                    nc.tensor.matmul(
                        vp[:, 0:2 * DV],
                        xt_sb[:, k, b * T + r * 128: b * T + (r + 1) * 128],
                        wv_sb[:, k, :],
                        start=(k == 0), stop=(k == KP - 1),
                    )
                nc.vector.tensor_tensor(
                    vaug[:, b, r, :], vp[:, 0:2 * DV], bv_sb[:], op=ADD,
                )

            def sc(b, qc, kt):
                # scores^T [kj=128, qi=512] per head, heads side by side
                S = ps_s.tile([128, 2 * 512], fp32, tag="S", name="S")
                for h in range(HL):
                    nc.tensor.matmul(
                        S[:, h * 512:(h + 1) * 512],
                        qkT[h * D:(h + 1) * D, b, 1, kt * 128:(kt + 1) * 128],
                        qkT[h * D:(h + 1) * D, b, 0, qc * 512:(qc + 1) * 512],
                        start=True, stop=True,
                    )
                return S

            def vmm_one(ot, P, b, h, qs, kt):
                # o[q-subtile, d+1] += P^T[kj, q-sub]^T @ vaug[kj, d+1]
                nc.tensor.matmul(
                    ot[h][:, qs * DV:(qs + 1) * DV],
                    P[:, h * 512 + qs * 128: h * 512 + (qs + 1) * 128],
                    vaug[:, b, kt, h * DV:(h + 1) * DV],
                    start=(kt == 0), stop=(kt == KT - 1),
                )

            def finalize(b, qc, ot, via_pe=False):
                # All norms (both heads) must precede the transposes: each
                # transpose reads a 128-col window spanning both head
                # slots.  via_pe: for the very last group the DMA
                # transpose+dup chain (~650ns queue overhead per hop) sits
                # exposed on the critical tail, so route it through PE
                # identity-matmuls + DVE/ACT copies instead (idle there).
                rd = npool.tile([128, 8], fp32, tag="rd", name="rd")
                for h in range(HL):
                    nc.vector.reciprocal(
                        rd[:, h * 4:(h + 1) * 4].rearrange(
                            "p (a c) -> p a c", c=1),
                        ot[h].rearrange(
                            "p (q c) -> p q c", c=DV)[:, :, D:D + 1],
                    )
                for qs in range(QS):
                    qt = qc * QS + qs
                    for h in range(HL):
                        if via_pe and h == 1:
                            nc.scalar.activation(
                                o_sb[b][:, qt, h * 64:(h + 1) * 64],
                                ot[h][:, qs * DV: qs * DV + D], COPY,
                                scale=rd[:, h * 4 + qs: h * 4 + qs + 1])
                        else:
                            nc.vector.tensor_scalar(
                                o_sb[b][:, qt, h * 64:(h + 1) * 64],
                                ot[h][:, qs * DV: qs * DV + D],
                                rd[:, h * 4 + qs: h * 4 + qs + 1],
                                None, op0=MULT,
                            )
                for qs in range(QS):
                    qt = qc * QS + qs
                    for h in range(HL):
                        # o^T; out rows 64-127 get the neighbour head slot /
                        # pad lanes, overwritten by the shifted dup below
                        if via_pe:
                            # bf16 [128,512] = same slot bytes as the sm tag
                            tp = ps_sm.tile([128, 512], bf16, tag="sm",
                                            name="tp")
                            nc.tensor.transpose(
                                tp[:, 0:128],
                                o_sb[b][:, qt, h * 64:h * 64 + 128],
                                ident[:])
                            nc.vector.tensor_copy(
                                oTd[b][0:128, h, qt * 128:(qt + 1) * 128],
                                tp[:, 0:128])
                        else:
                            nc.sync.dma_start_transpose(
                                oTd[b][0:128, h, qt * 128:(qt + 1) * 128],
                                o_sb[b][:, qt, h * 64:h * 64 + 128],
                            )
                for h in range(HL):
                    # shifted dup: via_pe does qc-1 + qc chunks; the DMA
                    # route defers to one whole-row dup after the last qc
                    # (fewer DMA producers -> fewer merged waits downstream)
                    chunks = []
                    if via_pe:
                        # via_pe only runs on the final group, so it owns
                        # the whole-row dup (in 512-col pieces)
                        chunks = [(512 * j, 512) for j in range(QC - 1)]
                        chunks.append(((QC - 1) * 512, 511))
                    elif qc == QC - 1:
                        chunks.append((0, T - 1))
                    for c0, cw in chunks:
                        if via_pe:
                            dp = ps_sm.tile([128, 512], fp32, tag="sm",
                                            name="dp")
                            nc.tensor.matmul(
                                dp[0:64, 0:cw], ident[0:64, 0:64],
                                oTd[b][0:64, h, c0 + 1:c0 + cw + 1],
                                start=True, stop=True)
                            nc.scalar.activation(
                                oTd[b][64:128, h, c0:c0 + cw],
                                dp[0:64, 0:cw], COPY)
                        else:
                            nc.sync.dma_start(
                                oTd[b][64:128, h, c0:c0 + cw],
                                oTd[b][0:64, h, c0 + 1:c0 + cw + 1])

            def outproj_piece(b, h, n5, s, split, tag="sm"):
                # one 1/split column chunk of out_proj for (b, h, n5); bias
                # folded in as a ones-row matmul so y DMAs straight out of
                # PSUM (no DVE hop on the critical tail)
                of2 = oTd[b][:, h, :].rearrange("p (t j) -> p j t", j=16)
                w = 512 // split
                c0 = n5 * 512 + s * w
                yp = ps_sm.tile([128, 512], fp32, tag=tag, name="yp") \
                    if tag == "sm" else \
                    ps_s.tile([128, 512], fp32, tag=tag, name="yp")
                for jj in range(8):
                    nc.tensor.matmul(
                        yp[:, 0:w],
                        of2[:, 2 * jj, :],
                        wout_sb[:, jj, c0:c0 + w],
                        start=(jj == 0), stop=False,
                    )
                nc.tensor.matmul(
                    yp[:, 0:w], ones1[0:1, :], bout_bf[0:1, c0:c0 + w],
                    start=False, stop=True,
                )
                # PSUM cannot feed a DMA directly; stage through SBUF on
                # DVE mid-stream / ACT at the tail (idle there)
                ys = ypool.tile([128, 512], fp32, tag="ys", name="ys")
                if b == 0 or (h + n5 + s) % 2:
                    nc.vector.tensor_copy(ys[:, 0:w], yp[:, 0:w])
                else:
                    nc.scalar.activation(ys[:, 0:w], yp[:, 0:w], COPY)
                # mid-stream y-writes ride the gpsimd SWDGE ring: keeps
                # their deep dependency chains off the SP ring's semaphore
                # window (an SP sem recycle once stalled the exp stream
                # 14us).  Tail y-writes (b=1) alternate SP/gpsimd.
                eng = nc.gpsimd if (h + n5 + s) % 2 else nc.sync
                eng.dma_start(y_d[b, h, :, c0:c0 + w], ys[:, 0:w])

            # ---- filler schedule: all pieces ~0.43us (8 accumulating
            # matmuls of F<=130) so the score stream is never delayed by a
            # long chain sitting ahead of it in the PE queue.  Greedy
            # deadline placement, one piece per slot unless a deadline
            # forces more.
            SCHED = {}
            VROWS = {}

            def at_slot(slot, fn, vrow=None):
                key = (slot // 64, (slot % 64) // 16, slot % 16)
                SCHED.setdefault(key, []).append(fn)
                if vrow is not None:
                    VROWS.setdefault(key, []).append(vrow)

            def chunk_slot(c):
                # xt chunk c lands on the gpsimd ring ~2.9us apart; convert
                # to the exp-slot index from which a piece may read it
                if c == 0:
                    return -100
                return int((7814 + 2913 * c + 900 - 11090) / 1038) + 1

            pieces = []  # [deadline, earliest, fn, vrow]
            for b in range(B):
                for kt in range(KT):
                    if b == 0 and kt < 2:
                        continue  # prologue
                    c = (b * T + kt * 128) // 512
                    pieces.append(
                        [64 * b + kt - 2, chunk_slot(c),
                         (lambda bb, kk: lambda: proj_k(bb, kk))(b, kt),
                         None])
                for r in range(KT):
                    c = (b * T + r * 128) // 512
                    pieces.append(
                        [(13 if b == 0 else 52) + r // 2, chunk_slot(c),
                         (lambda bb, rr: lambda: proj_v(bb, rr))(b, r),
                         (b, r)])
                for n in range(QC):
                    if b == 0 and n == 0:
                        continue  # prologue
                    for qs in range(QS):
                        c = (b * T + n * 512 + qs * 128) // 512
                        pieces.append(
                            [64 * b + 16 * n - 3, chunk_slot(c),
                             (lambda bb, nn, qq:
                              lambda: proj_q_qs(bb, nn, qq))(b, n, qs),
                             None])
            for h in range(HL):
                for n5 in range(2):
                    for s in range(2):
                        pieces.append(
                            [73 + 2 * (4 * h + 2 * n5 + s), 73,
                             (lambda hh, nn, ss:
                              lambda: outproj_piece(0, hh, nn, ss, 2))(
                                  h, n5, s),
                             None])
            todo = sorted(pieces, key=lambda p: (p[0], p[1]))
            for slot in range(B * QC * KT):
                # keep group-boundary slots free: the finalize/norm chain
                # must reach the DVE queue head unimpeded.  Deadline-due
                # pieces are placed unconditionally (a qkT piece placed past
                # its deadline would be read stale by the score lookahead).
                boundary = slot % KT in (15, 0)
                n = 1 if boundary else 0
                for p in todo[:]:
                    if p[1] > slot:
                        continue
                    if p[0] <= slot:
                        at_slot(slot, p[2], p[3])
                        todo.remove(p)
                        n += 1
                    elif n == 0:
                        at_slot(slot, p[2], p[3])
                        todo.remove(p)
                        n += 1
                    else:
                        break
            assert not todo, [p[:2] for p in todo]

            # wout/bout on the gpsimd ring once startup DMA traffic is done
            at_slot(33, lambda: nc.gpsimd.dma_start(wout_sb[:], wout_d[:]))
            at_slot(35, lambda: nc.gpsimd.dma_start(bout_sb[:], bout_d[:]))
            at_slot(37, lambda: nc.vector.tensor_copy(
                bout_bf[0:1, :], bout_sb[0:1, :]))

            # ---- unified emission ----
            def _emit_all():
                seq = [(b, qc, kt) for b in range(B) for qc in range(QC)
                       for kt in range(KT)]
                vrows = {0: 0, 1: 0}
                # PSUM accumulation: one pending group per 2KB zero region
                # (bank).  Each o bank (og0=h0, og1=h1) streams its qs=0
                # region kt-incrementally through the group; regions qs=1-3
                # drain region-major after the group's last kt, two regions
                # per slot.
                pmap = {}            # (b,qc) -> {kt: P}
                stream_kts = {}      # (b,qc) -> kts streamed (qs=0 regions)
                drain_q = []         # groups past kt=15, awaiting qs 1-3
                drained = {}         # (b,qc) -> drained region count (0..6)
                otiles = {}          # (b,qc) -> [o_h0, o_h1]
                finalized = []       # groups finalized, in order
                stream_q = []        # groups awaiting/undergoing streaming

                def alloc_group(g):
                    otiles[g] = [
                        ps_o.tile([128, QS * DV], fp32, tag=f"og{h}",
                                  name=f"og{h}")
                        for h in range(HL)
                    ]

                def group_done(g):
                    finalize(g[0], g[1], otiles[g],
                             via_pe=(g == (B - 1, QC - 1)))
                    finalized.append(g)
                    otiles.pop(g)

                def flush(now_i, all_=False):
                    # 1) drain the head of drain_q: regions (qs>=1), two per
                    # call (region = 16 matmuls of F=65 ~ 0.43us each)
                    nreg = 1000 if all_ else 2
                    while drain_q and nreg > 0:
                        g = drain_q[0]
                        d = drained.get(g, 0)
                        take = min(2 * (QS - 1) - d, nreg)
                        for idx in range(d, d + take):
                            qs = 1 + idx // HL
                            h = idx % HL
                            for kt in range(KT):
                                vmm_one(otiles[g], pmap[g][kt], g[0],
                                        h, qs, kt)
                        drained[g] = d + take
                        nreg -= take
                        if drained[g] == 2 * (QS - 1):
                            drain_q.pop(0)
                            pmap.pop(g)
                            group_done(g)
                    # 2) stream qs=0 regions (head of stream_q) as kts and
                    # vaug rows become available; a group may only take the
                    # o banks once the previous group has fully vacated them
                    while stream_q:
                        g = stream_q[0]
                        if g not in otiles:
                            if otiles:
                                break    # banks still owned by prior group
                            alloc_group(g)
                        hi = min(max(pmap[g].keys()) + 1 if pmap[g] else 0,
                                 vrows[g[0]])
                        sk = stream_kts.get(g, 0)
                        for kt in range(sk, hi):
                            for h in range(HL):
                                vmm_one(otiles[g], pmap[g][kt], g[0],
                                        h, 0, kt)
                        stream_kts[g] = max(sk, hi)
                        if stream_kts[g] == KT:
                            drain_q.append(g)
                            stream_q.pop(0)
                            continue
                        break

                # ---- prologue ----
                warmup(46)
                proj_q(0, 0)
                proj_k(0, 0)
                proj_k(0, 1)
                S = sc(0, 0, 0)

                P_last = None
                for i, (b, qc, kt) in enumerate(seq):
                    P = ppool.tile([128, 2 * 512], bf16, tag="P")
                    nc.scalar.activation(P[:], S[:], EXP, scale=0.125)
                    P_last = P
                    if i + 1 < len(seq):
                        S = sc(*seq[i + 1])
                    for fn in SCHED.get((b, qc, kt), []):
                        fn()
                    for (vb, r) in VROWS.get((b, qc, kt), []):
                        vrows[vb] = max(vrows[vb], r + 1)
                    if kt == 0:
                        stream_q.append((b, qc))
                        pmap[(b, qc)] = {}
                    pmap[(b, qc)][kt] = P
                    flush(i)

                # ---- tail ----
                while drain_q or stream_q:
                    flush(len(seq), all_=True)
                assert len(finalized) == B * QC, finalized
                # keep the PE p-state hot across the norm/transpose/dup
                # wait; chained on the last P tile so the scheduler cannot
                # hoist these out of the tail window
                warmup(8, src=P_last)
                for h in range(HL):
                    for n5 in range(2):
                        for s in range(2):
                            outproj_piece(1, h, n5, s, 2,
                                          tag=("S" if (n5 + s) % 2 else "sm"))

            _emit_all()

    nc.compile()
    return nc


def _get_runner():
    """Build + compile once; return a callable(in_maps) -> list of out dicts."""
    global _RUNNER
    if _RUNNER is not None:
        return _RUNNER

    import jax
    import concourse.mybir as mybir
    from concourse import bass2jax
    from jax.experimental.shard_map import shard_map
    from jax.sharding import Mesh, PartitionSpec

    nc = _build_nc()
    bass2jax.install_neuronx_cc_hook()

    partition_name = (nc.partition_id_tensor.name
                      if nc.partition_id_tensor else None)
    in_names, out_names, out_avals = [], [], []
    for alloc in nc.m.functions[0].allocations:
        if not isinstance(alloc, mybir.MemoryLocationSet):
            continue
        name = alloc.memorylocations[0].name
        if alloc.kind == "ExternalInput":
            if name != partition_name:
                in_names.append(name)
        elif alloc.kind == "ExternalOutput":
            out_names.append(name)
            out_avals.append(jax.core.ShapedArray(
                tuple(alloc.tensor_shape), mybir.dt.np(alloc.dtype)))

    n_params, n_outs = len(in_names), len(out_avals)
    all_names = in_names + out_names
    if partition_name is not None:
        all_names = all_names + [partition_name]

    def _body(*args):
        operands = list(args)
        if partition_name is not None:
            operands.append(bass2jax.partition_id_tensor())
        outs = bass2jax._bass_exec_p.bind(
            *operands,
            out_avals=tuple(out_avals),
            in_names=tuple(all_names),
            out_names=tuple(out_names),
            lowering_input_output_aliases=(),
            sim_require_finite=True,
            sim_require_nnan=True,
            nc=nc,
        )
        return tuple(outs)

    devices = jax.devices()[:N_CORES]
    mesh = Mesh(np.asarray(devices), ("core",))
    in_specs = (PartitionSpec("core"),) * (n_params + n_outs)
    out_specs = (PartitionSpec("core"),) * n_outs
    donate = tuple(range(n_params, n_params + n_outs))
    sharded = jax.jit(
        shard_map(_body, mesh=mesh, in_specs=in_specs, out_specs=out_specs,
                  check_rep=False),
        donate_argnums=donate, keep_unused=True,
    )

    def run(in_maps):
        concat_in = [
            np.concatenate([np.asarray(in_maps[c][nm]) for c in range(N_CORES)],
                           axis=0)
            for nm in in_names
        ]
        concat_zeros = [
            np.zeros((N_CORES * a.shape[0], *a.shape[1:]), a.dtype)
            for a in out_avals
        ]
        out_arrs = sharded(*concat_in, *concat_zeros)
        return [
            {nm: np.asarray(out_arrs[i]).reshape(N_CORES, *out_avals[i].shape)[c]
             for i, nm in enumerate(out_names)}
            for c in range(N_CORES)
        ]

    _RUNNER = run
    run._bench_parts = (sharded, mesh, in_names, out_names, out_avals,
                        n_params, _body)
    return run


def _make_bench(in_maps):
    """Device-resident benchmark closure: returns fn() that runs one
    execution with all inputs already on device (no donation)."""
    import jax
    from jax.experimental.shard_map import shard_map
    from jax.sharding import NamedSharding, PartitionSpec

    run = _get_runner()
    sharded, mesh, in_names, out_names, out_avals, n_params, _body = \
        run._bench_parts
    sh = NamedSharding(mesh, PartitionSpec("core"))

    nodonate = jax.jit(
        shard_map(_body, mesh=mesh,
                  in_specs=(PartitionSpec("core"),) * (n_params + len(out_avals)),
                  out_specs=(PartitionSpec("core"),) * len(out_avals),
                  check_rep=False),
        keep_unused=True,
    )
    concat_in = [
        np.concatenate([np.asarray(in_maps[c][nm]) for c in range(N_CORES)], axis=0)
        for nm in in_names
    ]
    concat_zeros = [
        np.zeros((N_CORES * a.shape[0], *a.shape[1:]), a.dtype) for a in out_avals
    ]
    dev_args = [jax.device_put(a, sh) for a in concat_in + concat_zeros]
    for a in dev_args:
        a.block_until_ready()

    def bench_once():
        outs = nodonate(*dev_args)
        for o in outs:
            o.block_until_ready()
        return outs

    bench_once.nodonate = nodonate
    bench_once.dev_args = dev_args
    return bench_once


def _prep_in_maps(x, W_qkv, b_qkv, W_out, b_out):
    bf = ml_dtypes.bfloat16
    xt = np.ascontiguousarray(
        x.reshape(B * T, E).T).astype(bf)                      # [E, B*T]
    wout = np.ascontiguousarray(
        W_out.reshape(8, 128, E).transpose(1, 0, 2).reshape(128, 8 * E)).astype(bf)
    bout = np.ascontiguousarray(
        np.broadcast_to(b_out.astype(np.float32)[None, :], (128, E)))

    in_maps = []
    for c in range(N_CORES):
        hs = [HL * c + i for i in range(HL)]
        qcols = np.concatenate(
            [W_qkv[:, 0 * E + h * D:0 * E + (h + 1) * D] for h in hs], axis=1)
        kcols = np.concatenate(
            [W_qkv[:, 1 * E + h * D:1 * E + (h + 1) * D] for h in hs], axis=1)
        wqk = np.ascontiguousarray(
            np.concatenate([qcols, kcols], axis=1)).astype(bf)  # [E, 256]
        zcol = np.zeros((E, 1), np.float32)
        wv = np.ascontiguousarray(np.concatenate(
            [arr for h in hs
             for arr in (W_qkv[:, 2 * E + h * D:2 * E + (h + 1) * D], zcol)],
            axis=1)).astype(bf)                                 # [E, 130]
        bq = np.concatenate([b_qkv[0 * E + h * D:0 * E + (h + 1) * D] for h in hs])
        bk = np.concatenate([b_qkv[1 * E + h * D:1 * E + (h + 1) * D] for h in hs])
        bqk = np.ascontiguousarray(
            np.stack([bq, bk], axis=1)).astype(np.float32)      # [128, 2]
        one = np.ones(1, np.float32)
        bvv = np.concatenate(
            [a for h in hs
             for a in (b_qkv[2 * E + h * D:2 * E + (h + 1) * D], one)])
        bv = np.ascontiguousarray(
            np.broadcast_to(bvv.astype(np.float32)[None, :], (128, 2 * DV)))
        in_maps.append({
            "xt": xt, "wqk": wqk, "wv": wv, "wout": wout,
            "bqk": bqk, "bv": bv, "bout": bout,
        })
    return in_maps


def kernel(x, W_qkv, b_qkv, W_out, b_out):
    x = np.asarray(x, dtype=np.float32)
    W_qkv = np.asarray(W_qkv, dtype=np.float32)
    b_qkv = np.asarray(b_qkv, dtype=np.float32)
    W_out = np.asarray(W_out, dtype=np.float32)
    b_out = np.asarray(b_out, dtype=np.float32)

    run = _get_runner()
    in_maps = _prep_in_maps(x, W_qkv, b_qkv, W_out, b_out)
    results = run(in_maps)

    out = np.empty((B, T, E), np.float32)
    for c in range(N_CORES):
        y = results[c]["y"]          # [B, HL, 128, E]
        for hl in range(HL):
            hg = HL * c + hl
            out[:, hg * 128:(hg + 1) * 128, :] = y[:, hl]
    return out


# revision 4
# speedup vs baseline: 1.2666x; 1.0770x over previous
"""Multi-head self-attention TRN2 kernel (B=2, T=2048, E=1024, H=16, D=64).

Sharding: tensor-parallel over heads - each of the 8 cores owns 2 heads.
Because the reference reshapes (B,H,T,D)->(B,T,E) with NO transpose, each
head's attention output maps to 128 complete contiguous rows of the
out_proj input, so the whole computation is embarrassingly parallel
across heads (no collectives).

Per-core pipeline (all matmuls bf16, accumulation fp32):
  1. qT/kT projections in [d, T] layout (heads stacked on partitions
     0-63 / 64-127); v in natural [T, d] layout augmented with a ones
     column (softmax denominator for free).
  2. scores^T tiles [kj=128, qi=512x2h] -> exp on ScalarE -> P^T bf16.
  3. attn@v in the cheap orientation: out o[q=128 partitions, d+1=65
     free] accumulated over kj tiles - PE cost 65 free/step instead of
     512 (the cost model charges output free size per accumulation
     step, so small-free x many-partitions wins 8x per instruction).
  4. normalize by per-partition (per-query) reciprocal on VectorE,
     downcast to bf16.
  5. o -> o^T via DMA xbar transposes (off the PE critical path), then
     shifted partition-duplicate copies build the [128=(d,j-parity), T]
     lhsT layout that performs the reference's "faithful reshape" for
     free in out_proj.
  6. out_proj: 8 accumulating matmuls per 512-column chunk.
"""

import numpy as np
import ml_dtypes

B, T, E, H, D = 2, 2048, 1024, 16, 64
N_CORES = 8
HL = H // N_CORES          # heads per core = 2
KP = E // 128              # 8 contraction partition-tiles
KT = T // 128              # 16 kj tiles
QC = T // 512              # 4 qi chunks of 512
QS = 4                     # q subtiles of 128 per chunk
DV = D + 1                 # v width incl. denominator ones column

_RUNNER = None


def _build_nc():
    import concourse.bacc as bacc
    import concourse.tile as tile
    import concourse.bass as bass
    import concourse.mybir as mybir

    from concourse import masks

    fp32 = mybir.dt.float32
    bf16 = mybir.dt.bfloat16
    ADD = mybir.AluOpType.add
    MULT = mybir.AluOpType.mult
    EXP = mybir.ActivationFunctionType.Exp
    COPY = mybir.ActivationFunctionType.Copy

    nc = bacc.Bacc("TRN2", target_bir_lowering=False, debug=False,
                   enable_asserts=True, num_devices=N_CORES)

    xt_d = nc.dram_tensor("xt", [E, B * T], bf16, kind="ExternalInput").ap()
    wqk_d = nc.dram_tensor("wqk", [E, 4 * D], bf16, kind="ExternalInput").ap()
    wv_d = nc.dram_tensor("wv", [E, 2 * DV], bf16, kind="ExternalInput").ap()
    wout_d = nc.dram_tensor("wout", [128, 8 * E], bf16, kind="ExternalInput").ap()
    bqk_d = nc.dram_tensor("bqk", [128, 2], fp32, kind="ExternalInput").ap()
    bv_d = nc.dram_tensor("bv", [128, 2 * DV], fp32, kind="ExternalInput").ap()
    bout_d = nc.dram_tensor("bout", [128, E], fp32, kind="ExternalInput").ap()
    y_d = nc.dram_tensor("y", [B, HL, 128, E], fp32, kind="ExternalOutput").ap()

    with tile.TileContext(nc) as tc:
        with (
            tc.tile_pool(name="const", bufs=1) as cpool,
            tc.tile_pool(name="ppool", bufs=26) as ppool,
            tc.tile_pool(name="npool", bufs=4) as npool,
            tc.tile_pool(name="ypool", bufs=4) as ypool,
            tc.tile_pool(name="ps_s", bufs=2, space=bass.MemorySpace.PSUM) as ps_s,
            tc.tile_pool(name="ps_o", bufs=1, space=bass.MemorySpace.PSUM) as ps_o,
            tc.tile_pool(name="ps_sm", bufs=2, space=bass.MemorySpace.PSUM) as ps_sm,
        ):
            # ---- constants / persistent tiles ----
            xt_sb = cpool.tile([128, KP, B * T], bf16, tag="xt")
            wqk_sb = cpool.tile([128, KP, 4 * D], bf16, tag="wqk")
            wv_sb = cpool.tile([128, KP, 2 * DV], bf16, tag="wv")
            wout_sb = cpool.tile([128, 8, E], bf16, tag="wout")
            bqk_sb = cpool.tile([128, 2], fp32, tag="bqk")
            bv_sb = cpool.tile([128, 2 * DV], fp32, tag="bv")
            bout_sb = cpool.tile([128, E], fp32, tag="bout")
            qkT = cpool.tile([128, B, 2, T], bf16, tag="qkT")
            vaug = cpool.tile([128, B, KT, 2 * DV], bf16, tag="vaug")
            # o natural layout, normalized bf16, 64 pad cols per head slot
            # (the pad transposes into oTd rows 64-127, which the shifted
            # dup copy then overwrites)
            # per-batch tensors (separate tags) so cross-batch false
            # dependencies cannot arise from coarse subtile tracking
            o_sb = [cpool.tile([128, KT, 192], bf16, tag=f"o_sb{bb}",
                               name=f"o_sb{bb}") for bb in range(B)]
            # o^T dup layout per head: rows 0-63 straight (written directly
            # by the xbar transpose), 64-127 shifted by one q (dup DMA)
            oTd = [cpool.tile([128, HL, T], bf16, tag=f"oTd{bb}",
                              name=f"oTd{bb}") for bb in range(B)]
            wub = cpool.tile([1, 128], bf16, tag="wub")
            ones1 = cpool.tile([1, 128], bf16, tag="ones1")
            bout_bf = cpool.tile([1, E], bf16, tag="bout_bf")
            ident = cpool.tile([128, 128], bf16, tag="ident")

            # warmup source (no DMA dependency); zero o_sb's pad lanes once
            nc.gpsimd.memset(wub[:], 0.0)
            nc.gpsimd.memset(ones1[:], 1.0)
            masks.make_identity(nc, ident[:])

            # ---- input DMAs: spread across the three HWDGE-ish rings so
            # queue overheads pipeline.  Transfers serialize on the shared
            # DMA engines; order = priority.  The first score tile needs
            # k cols + q[0:512], so those pieces go first, smallest first.
            wqk_r = wqk_d.rearrange("(a p) n -> p a n", p=128)
            xt_r = xt_d.rearrange("(a p) n -> p a n", p=128)
            nc.sync.dma_start(wqk_sb[:], wqk_r[:])
            # first 512 T-columns on the ACT ring (idle until exps start);
            # bigger slices keep the 1KB contiguous runs (no small-elem
            # DMA penalty)
            nc.scalar.dma_start(xt_sb[:, :, 0:256], xt_r[:, :, 0:256])
            nc.scalar.dma_start(xt_sb[:, :, 256:512], xt_r[:, :, 256:512])
            nc.scalar.dma_start(xt_sb[:, :, 512:1024], xt_r[:, :, 512:1024])
            nc.sync.dma_start(bqk_sb[:], bqk_d[:])
            nc.sync.dma_start(bv_sb[:], bv_d[:])
            # o_sb pad lanes zeroed on the gpsimd engine: doubles as a
            # delay so the Pool ring's chunk DMAs queue at the shared DMA
            # engines AFTER the ACT ring's critical first pieces
            for bb in range(B):
                nc.gpsimd.memset(o_sb[bb][:, :, 128:192], 0.0)
            # rest of x on the gpsimd SWDGE ring (engine otherwise idle);
            # wv mid-way (not needed until ~slot 11)
            for cc in range(2, B * T // 512):
                if cc == 4:
                    nc.gpsimd.dma_start(
                        wv_sb[:], wv_d.rearrange("(a p) n -> p a n", p=128))
                nc.gpsimd.dma_start(xt_sb[:, :, cc * 512:(cc + 1) * 512],
                                    xt_r[:, :, cc * 512:(cc + 1) * 512])

            # ---- PE warmup: keep the clock ramping from t~0 so the first
            # real matmuls run at full p-state.  src defaults to wub (no
            # deps -> the scheduler hoists them to kernel start); pass a
            # late-written AP to pin warmups into a late window instead.
            def warmup(n, src=None):
                s = wub if src is None else src
                for _ in range(n):
                    wt = ps_sm.tile([128, 128], fp32, tag="sm", name="wt")
                    nc.tensor.matmul(wt[:], s[0:1, 0:128], s[0:1, 0:128],
                                     start=True, stop=True)

            # ---- projection pieces ----
            def proj_q(b, n):
                # q^T chunk n: [128=(h0|h1)*d, 512]
                ps = ps_sm.tile([128, 512], fp32, tag="sm", name="psq")
                for k in range(KP):
                    nc.tensor.matmul(
                        ps[:],
                        wqk_sb[:, k, 0:128],
                        xt_sb[:, k, b * T + n * 512: b * T + (n + 1) * 512],
                        start=(k == 0), stop=(k == KP - 1),
                    )
                nc.vector.tensor_scalar(
                    qkT[:, b, 0, n * 512:(n + 1) * 512], ps[:],
                    bqk_sb[:, 0:1], None, op0=ADD,
                )

            def proj_q_half(b, n, hf):
                ps = ps_sm.tile([128, 512], fp32, tag="sm", name="psq")
                c0 = n * 512 + hf * 256
                for k in range(KP):
                    nc.tensor.matmul(
                        ps[:, 0:256],
                        wqk_sb[:, k, 0:128],
                        xt_sb[:, k, b * T + c0: b * T + c0 + 256],
                        start=(k == 0), stop=(k == KP - 1),
                    )
                nc.vector.tensor_scalar(
                    qkT[:, b, 0, c0:c0 + 256], ps[:, 0:256],
                    bqk_sb[:, 0:1], None, op0=ADD,
                )

            def proj_q_qs(b, n, qs):
                ps = ps_sm.tile([128, 512], fp32, tag="sm", name="psq")
                c0 = n * 512 + qs * 128
                for k in range(KP):
                    nc.tensor.matmul(
                        ps[:, 0:128],
                        wqk_sb[:, k, 0:128],
                        xt_sb[:, k, b * T + c0: b * T + c0 + 128],
                        start=(k == 0), stop=(k == KP - 1),
                    )
                nc.vector.tensor_scalar(
                    qkT[:, b, 0, c0:c0 + 128], ps[:, 0:128],
                    bqk_sb[:, 0:1], None, op0=ADD,
                )

            def proj_k(b, kt):
                # k^T tile kt: [128, 128]
                ps = ps_sm.tile([128, 512], fp32, tag="sm", name="psk")
                for k in range(KP):
                    nc.tensor.matmul(
                        ps[:, 0:128],
                        wqk_sb[:, k, 128:256],
                        xt_sb[:, k, b * T + kt * 128: b * T + (kt + 1) * 128],
                        start=(k == 0), stop=(k == KP - 1),
                    )
                nc.vector.tensor_scalar(
                    qkT[:, b, 1, kt * 128:(kt + 1) * 128], ps[:, 0:128],
                    bqk_sb[:, 1:2], None, op0=ADD,
                )

            def proj_v(b, r):
                # v natural [128, 2*DV] row tile r; W_v has zero columns at
                # the two "ones" slots and bv carries 1.0 there
                vp = ps_sm.tile([128, 512], fp32, tag="sm", name="vp")
                for k in range(KP):
                    nc.tensor.matmul(
                        vp[:, 0:2 * DV],
                        xt_sb[:, k, b * T + r * 128: b * T + (r + 1) * 128],
                        wv_sb[:, k, :],
                        start=(k == 0), stop=(k == KP - 1),
                    )
                nc.vector.tensor_tensor(
                    vaug[:, b, r, :], vp[:, 0:2 * DV], bv_sb[:], op=ADD,
                )

            def sc(b, qc, kt):
                # scores^T [kj=128, qi=512] per head, heads side by side
                S = ps_s.tile([128, 2 * 512], fp32, tag="S", name="S")
                for h in range(HL):
                    nc.tensor.matmul(
                        S[:, h * 512:(h + 1) * 512],
                        qkT[h * D:(h + 1) * D, b, 1, kt * 128:(kt + 1) * 128],
                        qkT[h * D:(h + 1) * D, b, 0, qc * 512:(qc + 1) * 512],
                        start=True, stop=True,
                    )
                return S

            def vmm_one(tile, c0, P, b, h, qs, kt):
                # o[q-subtile, d+1] += P^T[kj, q-sub]^T @ vaug[kj, d+1]
                nc.tensor.matmul(
                    tile[:, c0:c0 + DV],
                    P[:, h * 512 + qs * 128: h * 512 + (qs + 1) * 128],
                    vaug[:, b, kt, h * DV:(h + 1) * DV],
                    start=(kt == 0), stop=(kt == KT - 1),
                )

            def region_norm(b, qc, tile, c0, h, qs, on_act=False):
                # one region's reciprocal + normalize (tail interleaving)
                rdt = npool.tile([128, 8], fp32, tag="rd", name="rdt")
                nc.vector.reciprocal(
                    rdt[:, h * 4 + qs: h * 4 + qs + 1],
                    tile[:, c0 + D: c0 + DV])
                qt = qc * QS + qs
                if on_act:
                    nc.scalar.activation(
                        o_sb[b][:, qt, h * 64:(h + 1) * 64],
                        tile[:, c0: c0 + D], COPY,
                        scale=rdt[:, h * 4 + qs: h * 4 + qs + 1])
                else:
                    nc.vector.tensor_scalar(
                        o_sb[b][:, qt, h * 64:(h + 1) * 64],
                        tile[:, c0: c0 + D],
                        rdt[:, h * 4 + qs: h * 4 + qs + 1],
                        None, op0=MULT,
                    )

            def finalize(b, qc, ot, via_pe=False, skip_norms=False):
                # All norms (both heads) must precede the transposes: each
                # transpose reads a 128-col window spanning both head
                # slots.  via_pe: for the very last group the DMA
                # transpose+dup chain (~650ns queue overhead per hop) sits
                # exposed on the critical tail, so route it through PE
                # identity-matmuls + DVE/ACT copies instead (idle there).
                rd = npool.tile([128, 8], fp32, tag="rd", name="rd")
                if not skip_norms:
                    for h in range(HL):
                        nc.vector.reciprocal(
                            rd[:, h * 4:(h + 1) * 4].rearrange(
                                "p (a c) -> p a c", c=1),
                            ot[h].rearrange(
                                "p (q c) -> p q c", c=DV)[:, :, D:D + 1],
                        )
                    for qs in range(QS):
                        qt = qc * QS + qs
                        for h in range(HL):
                            nc.vector.tensor_scalar(
                                o_sb[b][:, qt, h * 64:(h + 1) * 64],
                                ot[h][:, qs * DV: qs * DV + D],
                                rd[:, h * 4 + qs: h * 4 + qs + 1],
                                None, op0=MULT,
                            )
                for qs in range(QS):
                    qt = qc * QS + qs
                    for h in range(HL):
                        # o^T; out rows 64-127 get the neighbour head slot /
                        # pad lanes, overwritten by the shifted dup below
                        if via_pe:
                            # bf16 [128,512] = same slot bytes as the sm tag
                            tp = ps_sm.tile([128, 512], bf16, tag="sm",
                                            name="tp")
                            nc.tensor.transpose(
                                tp[:, 0:128],
                                o_sb[b][:, qt, h * 64:h * 64 + 128],
                                ident[:])
                            nc.vector.tensor_copy(
                                oTd[b][0:128, h, qt * 128:(qt + 1) * 128],
                                tp[:, 0:128])
                        else:
                            nc.sync.dma_start_transpose(
                                oTd[b][0:128, h, qt * 128:(qt + 1) * 128],
                                o_sb[b][:, qt, h * 64:h * 64 + 128],
                            )
                for h in range(HL):
                    # shifted dup: via_pe does qc-1 + qc chunks; the DMA
                    # route defers to one whole-row dup after the last qc
                    # (fewer DMA producers -> fewer merged waits downstream)
                    chunks = []
                    if via_pe:
                        # cols 0..1023 were dup'd early (qc==2 below); the
                        # final group owns the last two 512-col pieces
                        chunks = [(1024, 512), (1536, 511)]
                    elif b == B - 1 and qc == QC - 2:
                        # early dup for the tail batch: sources complete
                        # through this group's transposes
                        chunks = [(0, 512), (512, 512)]
                    elif qc == QC - 1:
                        chunks.append((0, T - 1))
                    for c0, cw in chunks:
                        if via_pe:
                            dp = ps_sm.tile([128, 512], fp32, tag="sm",
                                            name="dp")
                            nc.tensor.matmul(
                                dp[0:64, 0:cw], ident[0:64, 0:64],
                                oTd[b][0:64, h, c0 + 1:c0 + cw + 1],
                                start=True, stop=True)
                            if h == 0:
                                nc.vector.tensor_copy(
                                    oTd[b][64:128, h, c0:c0 + cw],
                                    dp[0:64, 0:cw])
                            else:
                                nc.scalar.activation(
                                    oTd[b][64:128, h, c0:c0 + cw],
                                    dp[0:64, 0:cw], COPY)
                        else:
                            nc.sync.dma_start(
                                oTd[b][64:128, h, c0:c0 + cw],
                                oTd[b][0:64, h, c0 + 1:c0 + cw + 1])

            def outproj_piece(b, h, n5, s, split, tag="sm"):
                # one 1/split column chunk of out_proj for (b, h, n5); bias
                # folded in as a ones-row matmul so y DMAs straight out of
                # PSUM (no DVE hop on the critical tail)
                of2 = oTd[b][:, h, :].rearrange("p (t j) -> p j t", j=16)
                w = 512 // split
                c0 = n5 * 512 + s * w
                yp = ps_sm.tile([128, 512], fp32, tag=tag, name="yp") \
                    if tag == "sm" else \
                    ps_s.tile([128, 512], fp32, tag=tag, name="yp")
                for jj in range(8):
                    nc.tensor.matmul(
                        yp[:, 0:w],
                        of2[:, 2 * jj, :],
                        wout_sb[:, jj, c0:c0 + w],
                        start=(jj == 0), stop=False,
                    )
                nc.tensor.matmul(
                    yp[:, 0:w], ones1[0:1, :], bout_bf[0:1, c0:c0 + w],
                    start=False, stop=True,
                )
                # PSUM cannot feed a DMA directly; stage through SBUF on
                # DVE mid-stream / ACT at the tail (idle there)
                ys = ypool.tile([128, 512], fp32, tag="ys", name="ys")
                if b == 0 or (h + n5 + s) % 2:
                    nc.vector.tensor_copy(ys[:, 0:w], yp[:, 0:w])
                else:
                    nc.scalar.activation(ys[:, 0:w], yp[:, 0:w], COPY)
                # mid-stream y-writes ride the gpsimd SWDGE ring: keeps
                # their deep dependency chains off the SP ring's semaphore
                # window (an SP sem recycle once stalled the exp stream
                # 14us).  Tail y-writes (b=1) alternate SP/gpsimd.
                eng = nc.sync if (h + n5 + s) % 2 else nc.gpsimd
                eng.dma_start(y_d[b, h, :, c0:c0 + w], ys[:, 0:w])

            # ---- filler schedule: all pieces ~0.43us (8 accumulating
            # matmuls of F<=130) so the score stream is never delayed by a
            # long chain sitting ahead of it in the PE queue.  Greedy
            # deadline placement, one piece per slot unless a deadline
            # forces more.
            SCHED = {}
            VROWS = {}

            def at_slot(slot, fn, vrow=None):
                key = (slot // 64, (slot % 64) // 16, slot % 16)
                SCHED.setdefault(key, []).append(fn)
                if vrow is not None:
                    VROWS.setdefault(key, []).append(vrow)

            def chunk_slot(c):
                # xt chunk c lands on the gpsimd ring ~2.9us apart; convert
                # to the exp-slot index from which a piece may read it
                if c == 0:
                    return -100
                return int((7814 + 2913 * c + 900 - 11090) / 1038) + 1

            pieces = []  # [deadline, earliest, fn, vrow]
            for b in range(B):
                for kt in range(KT):
                    if b == 0 and kt < 2:
                        continue  # prologue
                    c = (b * T + kt * 128) // 512
                    pieces.append(
                        [64 * b + kt - 2, chunk_slot(c),
                         (lambda bb, kk: lambda: proj_k(bb, kk))(b, kt),
                         None])
                for r in range(KT):
                    c = (b * T + r * 128) // 512
                    pieces.append(
                        [(11 if b == 0 else 52) + r // 2, chunk_slot(c),
                         (lambda bb, rr: lambda: proj_v(bb, rr))(b, r),
                         (b, r)])
                for n in range(QC):
                    if b == 0 and n == 0:
                        continue  # prologue
                    for qs in range(QS):
                        c = (b * T + n * 512 + qs * 128) // 512
                        pieces.append(
                            [64 * b + 16 * n - 3, chunk_slot(c),
                             (lambda bb, nn, qq:
                              lambda: proj_q_qs(bb, nn, qq))(b, n, qs),
                             None])
            for h in range(HL):
                for n5 in range(2):
                    for s in range(2):
                        pieces.append(
                            [100 + 3 * (4 * h + 2 * n5 + s), 95,
                             (lambda hh, nn, ss:
                              lambda: outproj_piece(0, hh, nn, ss, 2))(
                                  h, n5, s),
                             None])
            todo = sorted(pieces, key=lambda p: (p[0], p[1]))
            for slot in range(B * QC * KT):
                # keep group-boundary slots free: the finalize/norm chain
                # must reach the DVE queue head unimpeded.  Deadline-due
                # pieces are placed unconditionally (a qkT piece placed past
                # its deadline would be read stale by the score lookahead).
                boundary = slot % KT in (15, 0)
                n = 1 if boundary else 0
                for p in todo[:]:
                    if p[1] > slot:
                        continue
                    if p[0] <= slot:
                        at_slot(slot, p[2], p[3])
                        todo.remove(p)
                        n += 1
                    elif n == 0:
                        at_slot(slot, p[2], p[3])
                        todo.remove(p)
                        n += 1
                    else:
                        break
            assert not todo, [p[:2] for p in todo]

            # wout/bout on the gpsimd ring once startup DMA traffic is done
            at_slot(33, lambda: nc.gpsimd.dma_start(wout_sb[:], wout_d[:]))
            at_slot(35, lambda: nc.gpsimd.dma_start(bout_sb[:], bout_d[:]))
            at_slot(37, lambda: nc.vector.tensor_copy(
                bout_bf[0:1, :], bout_sb[0:1, :]))

            # ---- unified emission ----
            def _emit_all():
                seq = [(b, qc, kt) for b in range(B) for qc in range(QC)
                       for kt in range(KT)]
                vrows = {0: 0, 1: 0}
                # PSUM accumulation: one pending group per 2KB zero region
                # (bank).  Each o bank (og0=h0, og1=h1) streams its qs=0
                # region kt-incrementally through the group; regions qs=1-3
                # drain region-major after the group's last kt, two regions
                # per slot.
                pmap = {}            # (b,qc) -> {kt: P}
                stream_kts = {}      # (b,qc) -> kts streamed (qs=0 regions)
                drain_q = []         # groups past kt=15, awaiting qs 1-3
                drained = {}         # (b,qc) -> drained region count (0..6)
                otiles = {}          # (b,qc) -> [o_h0, o_h1]
                finalized = []       # groups finalized, in order
                stream_q = []        # groups awaiting/undergoing streaming

                LASTG = (B - 1, QC - 1)

                def alloc_group(g):
                    otiles[g] = [
                        ps_o.tile([128, QS * DV], fp32, tag=f"og{h}",
                                  name=f"og{h}")
                        for h in range(HL)
                    ]
                    if g == LASTG:
                        # two extra banks from the sm tag: four streaming
                        # regions, so only qs=2,3 drain after the last exp
                        otiles[g] += [
                            ps_sm.tile([128, 512], fp32, tag="sm",
                                       name=f"ogx{h}")
                            for h in range(HL)
                        ]

                def rmap(g, h, qs):
                    # region (h, qs) -> (tile, col0).  Last group: qs 0,2 in
                    # og_h (cols 0/65), qs 1,3 in the sm extras.
                    if g == LASTG:
                        if qs % 2 == 0:
                            return otiles[g][h], (qs // 2) * DV
                        return otiles[g][2 + h], (qs // 2) * DV
                    return otiles[g][h], qs * DV

                def group_done(g, skip_norms=False):
                    finalize(g[0], g[1], otiles[g],
                             via_pe=(g == (B - 1, QC - 1)),
                             skip_norms=skip_norms)
                    finalized.append(g)
                    otiles.pop(g)

                def flush(now_i, all_=False):
                    # 1) drain the head of drain_q: regions (qs>=1), two per
                    # call (region = 16 matmuls of F=65 ~ 0.43us each)
                    nreg = 1000 if all_ else 2
                    while drain_q and nreg > 0:
                        g = drain_q[0]
                        last = all_ and g == LASTG
                        qs0 = 2 if g == LASTG else 1
                        ndrain = HL * (QS - qs0)
                        d = drained.get(g, 0)
                        if last and d == 0:
                            # tail: the streamed regions are closed - norm
                            # them while PE drains the rest
                            for qs in range(qs0):
                                for h in range(HL):
                                    t_, c_ = rmap(g, h, qs)
                                    region_norm(g[0], g[1], t_, c_, h, qs,
                                                on_act=(h == 1))
                        take = min(ndrain - d, nreg)
                        for idx in range(d, d + take):
                            qs = qs0 + idx // HL
                            h = idx % HL
                            t_, c_ = rmap(g, h, qs)
                            for kt in range(KT):
                                vmm_one(t_, c_, pmap[g][kt], g[0],
                                        h, qs, kt)
                            if last:
                                region_norm(g[0], g[1], t_, c_, h, qs,
                                            on_act=(idx % 2 == 1))
                        drained[g] = d + take
                        nreg -= take
                        if drained[g] == ndrain:
                            drain_q.pop(0)
                            pmap.pop(g)
                            group_done(g, skip_norms=last)
                    # 2) stream qs=0 regions (head of stream_q) as kts and
                    # vaug rows become available; a group may only take the
                    # o banks once the previous group has fully vacated them
                    while stream_q:
                        g = stream_q[0]
                        if g not in otiles:
                            if otiles:
                                break    # banks still owned by prior group
                            alloc_group(g)
                        hi = min(max(pmap[g].keys()) + 1 if pmap[g] else 0,
                                 vrows[g[0]])
                        sk = stream_kts.get(g, 0)
                        nstream = 2 if g == LASTG else 1
                        for kt in range(sk, hi):
                            for qs in range(nstream):
                                for h in range(HL):
                                    t_, c_ = rmap(g, h, qs)
                                    vmm_one(t_, c_, pmap[g][kt], g[0],
                                            h, qs, kt)
                        stream_kts[g] = max(sk, hi)
                        if stream_kts[g] == KT:
                            drain_q.append(g)
                            stream_q.pop(0)
                            continue
                        break

                # ---- prologue: first score tile in column halves so the
                # exp stream starts as soon as the first xt piece lands ----
                warmup(52)
                proj_q_qs(0, 0, 0)
                proj_q_qs(0, 0, 1)
                proj_k(0, 0)
                proj_k(0, 1)
                S = ps_s.tile([128, 2 * 512], fp32, tag="S", name="S")
                for h in range(HL):
                    nc.tensor.matmul(
                        S[:, h * 512:h * 512 + 256],
                        qkT[h * D:(h + 1) * D, 0, 1, 0:128],
                        qkT[h * D:(h + 1) * D, 0, 0, 0:256],
                        start=True, stop=True)
                proj_q_qs(0, 0, 2)
                proj_q_qs(0, 0, 3)
                for h in range(HL):
                    nc.tensor.matmul(
                        S[:, h * 512 + 256:(h + 1) * 512],
                        qkT[h * D:(h + 1) * D, 0, 1, 0:128],
                        qkT[h * D:(h + 1) * D, 0, 0, 256:512],
                        start=True, stop=True)

                P_last = None
                for i, (b, qc, kt) in enumerate(seq):
                    P = ppool.tile([128, 2 * 512], bf16, tag="P")
                    if i == 0:
                        # split first exp: the first half only needs the
                        # first xt/q piece
                        Sv = S.rearrange("p (h c) -> p h c", h=HL)
                        Pv = P.rearrange("p (h c) -> p h c", h=HL)
                        nc.scalar.activation(Pv[:, :, 0:256], Sv[:, :, 0:256],
                                             EXP, scale=0.125)
                        nc.scalar.activation(Pv[:, :, 256:512],
                                             Sv[:, :, 256:512],
                                             EXP, scale=0.125)
                    else:
                        nc.scalar.activation(P[:], S[:], EXP, scale=0.125)
                    P_last = P
                    if i + 1 < len(seq):
                        S = sc(*seq[i + 1])
                    for fn in SCHED.get((b, qc, kt), []):
                        fn()
                    for (vb, r) in VROWS.get((b, qc, kt), []):
                        vrows[vb] = max(vrows[vb], r + 1)
                    if kt == 0:
                        stream_q.append((b, qc))
                        pmap[(b, qc)] = {}
                    pmap[(b, qc)][kt] = P
                    flush(i)

                # ---- tail ----
                while drain_q or stream_q:
                    flush(len(seq), all_=True)
                assert len(finalized) == B * QC, finalized
                # keep the PE p-state hot across the norm/transpose/dup
                # wait; chained on the last P tile so the scheduler cannot
                # hoist these out of the tail window
                warmup(8, src=P_last)
                for h in range(HL):
                    for n5 in range(2):
                        for s in range(2):
                            outproj_piece(1, h, n5, s, 2,
                                          tag=("S" if (n5 + s) % 2 else "sm"))

            _emit_all()

    nc.compile()
    return nc


def _get_runner():
    """Build + compile once; return a callable(in_maps) -> list of out dicts."""
    global _RUNNER
    if _RUNNER is not None:
        return _RUNNER

    import jax
    import concourse.mybir as mybir
    from concourse import bass2jax
    from jax.experimental.shard_map import shard_map
    from jax.sharding import Mesh, PartitionSpec

    nc = _build_nc()
    bass2jax.install_neuronx_cc_hook()

    partition_name = (nc.partition_id_tensor.name
                      if nc.partition_id_tensor else None)
    in_names, out_names, out_avals = [], [], []
    for alloc in nc.m.functions[0].allocations:
        if not isinstance(alloc, mybir.MemoryLocationSet):
            continue
        name = alloc.memorylocations[0].name
        if alloc.kind == "ExternalInput":
            if name != partition_name:
                in_names.append(name)
        elif alloc.kind == "ExternalOutput":
            out_names.append(name)
            out_avals.append(jax.core.ShapedArray(
                tuple(alloc.tensor_shape), mybir.dt.np(alloc.dtype)))

    n_params, n_outs = len(in_names), len(out_avals)
    all_names = in_names + out_names
    if partition_name is not None:
        all_names = all_names + [partition_name]

    def _body(*args):
        operands = list(args)
        if partition_name is not None:
            operands.append(bass2jax.partition_id_tensor())
        outs = bass2jax._bass_exec_p.bind(
            *operands,
            out_avals=tuple(out_avals),
            in_names=tuple(all_names),
            out_names=tuple(out_names),
            lowering_input_output_aliases=(),
            sim_require_finite=True,
            sim_require_nnan=True,
            nc=nc,
        )
        return tuple(outs)

    devices = jax.devices()[:N_CORES]
    mesh = Mesh(np.asarray(devices), ("core",))
    in_specs = (PartitionSpec("core"),) * (n_params + n_outs)
    out_specs = (PartitionSpec("core"),) * n_outs
    donate = tuple(range(n_params, n_params + n_outs))
    sharded = jax.jit(
        shard_map(_body, mesh=mesh, in_specs=in_specs, out_specs=out_specs,
                  check_rep=False),
        donate_argnums=donate, keep_unused=True,
    )

    def run(in_maps):
        concat_in = [
            np.concatenate([np.asarray(in_maps[c][nm]) for c in range(N_CORES)],
                           axis=0)
            for nm in in_names
        ]
        concat_zeros = [
            np.zeros((N_CORES * a.shape[0], *a.shape[1:]), a.dtype)
            for a in out_avals
        ]
        out_arrs = sharded(*concat_in, *concat_zeros)
        return [
            {nm: np.asarray(out_arrs[i]).reshape(N_CORES, *out_avals[i].shape)[c]
             for i, nm in enumerate(out_names)}
            for c in range(N_CORES)
        ]

    _RUNNER = run
    run._bench_parts = (sharded, mesh, in_names, out_names, out_avals,
                        n_params, _body)
    return run


def _make_bench(in_maps):
    """Device-resident benchmark closure: returns fn() that runs one
    execution with all inputs already on device (no donation)."""
    import jax
    from jax.experimental.shard_map import shard_map
    from jax.sharding import NamedSharding, PartitionSpec

    run = _get_runner()
    sharded, mesh, in_names, out_names, out_avals, n_params, _body = \
        run._bench_parts
    sh = NamedSharding(mesh, PartitionSpec("core"))

    nodonate = jax.jit(
        shard_map(_body, mesh=mesh,
                  in_specs=(PartitionSpec("core"),) * (n_params + len(out_avals)),
                  out_specs=(PartitionSpec("core"),) * len(out_avals),
                  check_rep=False),
        keep_unused=True,
    )
    concat_in = [
        np.concatenate([np.asarray(in_maps[c][nm]) for c in range(N_CORES)], axis=0)
        for nm in in_names
    ]
    concat_zeros = [
        np.zeros((N_CORES * a.shape[0], *a.shape[1:]), a.dtype) for a in out_avals
    ]
    dev_args = [jax.device_put(a, sh) for a in concat_in + concat_zeros]
    for a in dev_args:
        a.block_until_ready()

    def bench_once():
        outs = nodonate(*dev_args)
        for o in outs:
            o.block_until_ready()
        return outs

    bench_once.nodonate = nodonate
    bench_once.dev_args = dev_args
    return bench_once


def _prep_in_maps(x, W_qkv, b_qkv, W_out, b_out):
    bf = ml_dtypes.bfloat16
    xt = np.ascontiguousarray(
        x.reshape(B * T, E).T).astype(bf)                      # [E, B*T]
    wout = np.ascontiguousarray(
        W_out.reshape(8, 128, E).transpose(1, 0, 2).reshape(128, 8 * E)).astype(bf)
    bout = np.ascontiguousarray(
        np.broadcast_to(b_out.astype(np.float32)[None, :], (128, E)))

    in_maps = []
    for c in range(N_CORES):
        hs = [HL * c + i for i in range(HL)]
        qcols = np.concatenate(
            [W_qkv[:, 0 * E + h * D:0 * E + (h + 1) * D] for h in hs], axis=1)
        kcols = np.concatenate(
            [W_qkv[:, 1 * E + h * D:1 * E + (h + 1) * D] for h in hs], axis=1)
        wqk = np.ascontiguousarray(
            np.concatenate([qcols, kcols], axis=1)).astype(bf)  # [E, 256]
        zcol = np.zeros((E, 1), np.float32)
        wv = np.ascontiguousarray(np.concatenate(
            [arr for h in hs
             for arr in (W_qkv[:, 2 * E + h * D:2 * E + (h + 1) * D], zcol)],
            axis=1)).astype(bf)                                 # [E, 130]
        bq = np.concatenate([b_qkv[0 * E + h * D:0 * E + (h + 1) * D] for h in hs])
        bk = np.concatenate([b_qkv[1 * E + h * D:1 * E + (h + 1) * D] for h in hs])
        bqk = np.ascontiguousarray(
            np.stack([bq, bk], axis=1)).astype(np.float32)      # [128, 2]
        one = np.ones(1, np.float32)
        bvv = np.concatenate(
            [a for h in hs
             for a in (b_qkv[2 * E + h * D:2 * E + (h + 1) * D], one)])
        bv = np.ascontiguousarray(
            np.broadcast_to(bvv.astype(np.float32)[None, :], (128, 2 * DV)))
        in_maps.append({
            "xt": xt, "wqk": wqk, "wv": wv, "wout": wout,
            "bqk": bqk, "bv": bv, "bout": bout,
        })
    return in_maps


def kernel(x, W_qkv, b_qkv, W_out, b_out):
    x = np.asarray(x, dtype=np.float32)
    W_qkv = np.asarray(W_qkv, dtype=np.float32)
    b_qkv = np.asarray(b_qkv, dtype=np.float32)
    W_out = np.asarray(W_out, dtype=np.float32)
    b_out = np.asarray(b_out, dtype=np.float32)

    run = _get_runner()
    in_maps = _prep_in_maps(x, W_qkv, b_qkv, W_out, b_out)
    results = run(in_maps)

    out = np.empty((B, T, E), np.float32)
    for c in range(N_CORES):
        y = results[c]["y"]          # [B, HL, 128, E]
        for hl in range(HL):
            hg = HL * c + hl
            out[:, hg * 128:(hg + 1) * 128, :] = y[:, hl]
    return out


# revision 5
# speedup vs baseline: 1.2745x; 1.0062x over previous
"""Multi-head self-attention TRN2 kernel (B=2, T=2048, E=1024, H=16, D=64).

Sharding: tensor-parallel over heads - each of the 8 cores owns 2 heads.
Because the reference reshapes (B,H,T,D)->(B,T,E) with NO transpose, each
head's attention output maps to 128 complete contiguous rows of the
out_proj input, so the whole computation is embarrassingly parallel
across heads (no collectives).

Per-core pipeline (all matmuls bf16, accumulation fp32):
  1. qT/kT projections in [d, T] layout (heads stacked on partitions
     0-63 / 64-127); v in natural [T, d] layout augmented with a ones
     column (softmax denominator for free).
  2. scores^T tiles [kj=128, qi=512x2h] -> exp on ScalarE -> P^T bf16.
  3. attn@v in the cheap orientation: out o[q=128 partitions, d+1=65
     free] accumulated over kj tiles - PE cost 65 free/step instead of
     512 (the cost model charges output free size per accumulation
     step, so small-free x many-partitions wins 8x per instruction).
  4. normalize by per-partition (per-query) reciprocal on VectorE,
     downcast to bf16.
  5. o -> o^T via DMA xbar transposes (off the PE critical path), then
     shifted partition-duplicate copies build the [128=(d,j-parity), T]
     lhsT layout that performs the reference's "faithful reshape" for
     free in out_proj.
  6. out_proj: 8 accumulating matmuls per 512-column chunk.
"""

import numpy as np
import ml_dtypes

B, T, E, H, D = 2, 2048, 1024, 16, 64
N_CORES = 8
HL = H // N_CORES          # heads per core = 2
KP = E // 128              # 8 contraction partition-tiles
KT = T // 128              # 16 kj tiles
QC = T // 512              # 4 qi chunks of 512
QS = 4                     # q subtiles of 128 per chunk
DV = D + 1                 # v width incl. denominator ones column

_RUNNER = None


def _build_nc():
    import concourse.bacc as bacc
    import concourse.tile as tile
    import concourse.bass as bass
    import concourse.mybir as mybir

    from concourse import masks

    fp32 = mybir.dt.float32
    bf16 = mybir.dt.bfloat16
    ADD = mybir.AluOpType.add
    MULT = mybir.AluOpType.mult
    EXP = mybir.ActivationFunctionType.Exp
    COPY = mybir.ActivationFunctionType.Copy

    nc = bacc.Bacc("TRN2", target_bir_lowering=False, debug=False,
                   enable_asserts=True, num_devices=N_CORES)

    xt_d = nc.dram_tensor("xt", [E, B * T], bf16, kind="ExternalInput").ap()
    wqk_d = nc.dram_tensor("wqk", [E, 4 * D], bf16, kind="ExternalInput").ap()
    wv_d = nc.dram_tensor("wv", [E, 2 * DV], bf16, kind="ExternalInput").ap()
    wout_d = nc.dram_tensor("wout", [128, 8 * E], bf16, kind="ExternalInput").ap()
    bqk_d = nc.dram_tensor("bqk", [128, 2], fp32, kind="ExternalInput").ap()
    bv_d = nc.dram_tensor("bv", [128, 2 * DV], fp32, kind="ExternalInput").ap()
    bout_d = nc.dram_tensor("bout", [128, E], fp32, kind="ExternalInput").ap()
    y_d = nc.dram_tensor("y", [B, HL, 128, E], fp32, kind="ExternalOutput").ap()

    with tile.TileContext(nc) as tc:
        with (
            tc.tile_pool(name="const", bufs=1) as cpool,
            tc.tile_pool(name="ppool", bufs=26) as ppool,
            tc.tile_pool(name="npool", bufs=4) as npool,
            tc.tile_pool(name="ypool", bufs=4) as ypool,
            tc.tile_pool(name="ps_s", bufs=2, space=bass.MemorySpace.PSUM) as ps_s,
            tc.tile_pool(name="ps_o", bufs=1, space=bass.MemorySpace.PSUM) as ps_o,
            tc.tile_pool(name="ps_sm", bufs=2, space=bass.MemorySpace.PSUM) as ps_sm,
        ):
            # ---- constants / persistent tiles ----
            xt_sb = cpool.tile([128, KP, B * T], bf16, tag="xt")
            wqk_sb = cpool.tile([128, KP, 4 * D], bf16, tag="wqk")
            wv_sb = cpool.tile([128, KP, 2 * DV], bf16, tag="wv")
            wout_sb = cpool.tile([128, 8, E], bf16, tag="wout")
            bqk_sb = cpool.tile([128, 2], fp32, tag="bqk")
            bv_sb = cpool.tile([128, 2 * DV], fp32, tag="bv")
            bout_sb = cpool.tile([128, E], fp32, tag="bout")
            qkT = cpool.tile([128, B, 2, T], bf16, tag="qkT")
            vaug = cpool.tile([128, B, KT, 2 * DV], bf16, tag="vaug")
            # o natural layout, normalized bf16, 64 pad cols per head slot
            # (the pad transposes into oTd rows 64-127, which the shifted
            # dup copy then overwrites)
            # per-batch tensors (separate tags) so cross-batch false
            # dependencies cannot arise from coarse subtile tracking
            o_sb = [cpool.tile([128, KT, 192], bf16, tag=f"o_sb{bb}",
                               name=f"o_sb{bb}") for bb in range(B)]
            # o^T dup layout per head: rows 0-63 straight (written directly
            # by the xbar transpose), 64-127 shifted by one q (dup DMA)
            oTd = [cpool.tile([128, HL, T], bf16, tag=f"oTd{bb}",
                              name=f"oTd{bb}") for bb in range(B)]
            wub = cpool.tile([1, 128], bf16, tag="wub")
            ones1 = cpool.tile([1, 128], bf16, tag="ones1")
            bout_bf = cpool.tile([1, E], bf16, tag="bout_bf")
            ident = cpool.tile([128, 128], bf16, tag="ident")

            # warmup source (no DMA dependency); zero o_sb's pad lanes once
            nc.gpsimd.memset(wub[:], 0.0)
            nc.gpsimd.memset(ones1[:], 1.0)
            masks.make_identity(nc, ident[:])

            # ---- input DMAs: spread across the three HWDGE-ish rings so
            # queue overheads pipeline.  Transfers serialize on the shared
            # DMA engines; order = priority.  The first score tile needs
            # k cols + q[0:512], so those pieces go first, smallest first.
            wqk_r = wqk_d.rearrange("(a p) n -> p a n", p=128)
            xt_r = xt_d.rearrange("(a p) n -> p a n", p=128)
            nc.sync.dma_start(wqk_sb[:], wqk_r[:])
            # first 512 T-columns on the ACT ring (idle until exps start);
            # bigger slices keep the 1KB contiguous runs (no small-elem
            # DMA penalty)
            nc.scalar.dma_start(xt_sb[:, :, 0:256], xt_r[:, :, 0:256])
            nc.scalar.dma_start(xt_sb[:, :, 256:512], xt_r[:, :, 256:512])
            nc.scalar.dma_start(xt_sb[:, :, 512:1024], xt_r[:, :, 512:1024])
            nc.sync.dma_start(bqk_sb[:], bqk_d[:])
            nc.sync.dma_start(bv_sb[:], bv_d[:])
            # o_sb pad lanes zeroed on the gpsimd engine: doubles as a
            # delay so the Pool ring's chunk DMAs queue at the shared DMA
            # engines AFTER the ACT ring's critical first pieces
            for bb in range(B):
                nc.gpsimd.memset(o_sb[bb][:, :, 128:192], 0.0)
            # rest of x on the gpsimd SWDGE ring (engine otherwise idle);
            # wv mid-way (not needed until ~slot 11)
            for cc in range(2, B * T // 512):
                if cc == 4:
                    nc.gpsimd.dma_start(
                        wv_sb[:], wv_d.rearrange("(a p) n -> p a n", p=128))
                nc.gpsimd.dma_start(xt_sb[:, :, cc * 512:(cc + 1) * 512],
                                    xt_r[:, :, cc * 512:(cc + 1) * 512])

            # ---- PE warmup: keep the clock ramping from t~0 so the first
            # real matmuls run at full p-state.  src defaults to wub (no
            # deps -> the scheduler hoists them to kernel start); pass a
            # late-written AP to pin warmups into a late window instead.
            def warmup(n, src=None):
                s = wub if src is None else src
                for _ in range(n):
                    wt = ps_sm.tile([128, 128], fp32, tag="sm", name="wt")
                    nc.tensor.matmul(wt[:], s[0:1, 0:128], s[0:1, 0:128],
                                     start=True, stop=True)

            # ---- projection pieces ----
            def proj_q(b, n):
                # q^T chunk n: [128=(h0|h1)*d, 512]
                ps = ps_sm.tile([128, 512], fp32, tag="sm", name="psq")
                for k in range(KP):
                    nc.tensor.matmul(
                        ps[:],
                        wqk_sb[:, k, 0:128],
                        xt_sb[:, k, b * T + n * 512: b * T + (n + 1) * 512],
                        start=(k == 0), stop=(k == KP - 1),
                    )
                nc.vector.tensor_scalar(
                    qkT[:, b, 0, n * 512:(n + 1) * 512], ps[:],
                    bqk_sb[:, 0:1], None, op0=ADD,
                )

            def proj_q_half(b, n, hf):
                ps = ps_sm.tile([128, 512], fp32, tag="sm", name="psq")
                c0 = n * 512 + hf * 256
                for k in range(KP):
                    nc.tensor.matmul(
                        ps[:, 0:256],
                        wqk_sb[:, k, 0:128],
                        xt_sb[:, k, b * T + c0: b * T + c0 + 256],
                        start=(k == 0), stop=(k == KP - 1),
                    )
                nc.vector.tensor_scalar(
                    qkT[:, b, 0, c0:c0 + 256], ps[:, 0:256],
                    bqk_sb[:, 0:1], None, op0=ADD,
                )

            def proj_q_qs(b, n, qs):
                ps = ps_sm.tile([128, 512], fp32, tag="sm", name="psq")
                c0 = n * 512 + qs * 128
                for k in range(KP):
                    nc.tensor.matmul(
                        ps[:, 0:128],
                        wqk_sb[:, k, 0:128],
                        xt_sb[:, k, b * T + c0: b * T + c0 + 128],
                        start=(k == 0), stop=(k == KP - 1),
                    )
                nc.vector.tensor_scalar(
                    qkT[:, b, 0, c0:c0 + 128], ps[:, 0:128],
                    bqk_sb[:, 0:1], None, op0=ADD,
                )

            def proj_k(b, kt):
                # k^T tile kt: [128, 128]
                ps = ps_sm.tile([128, 512], fp32, tag="sm", name="psk")
                for k in range(KP):
                    nc.tensor.matmul(
                        ps[:, 0:128],
                        wqk_sb[:, k, 128:256],
                        xt_sb[:, k, b * T + kt * 128: b * T + (kt + 1) * 128],
                        start=(k == 0), stop=(k == KP - 1),
                    )
                nc.vector.tensor_scalar(
                    qkT[:, b, 1, kt * 128:(kt + 1) * 128], ps[:, 0:128],
                    bqk_sb[:, 1:2], None, op0=ADD,
                )

            def proj_v(b, r):
                # v natural [128, 2*DV] row tile r; W_v has zero columns at
                # the two "ones" slots and bv carries 1.0 there
                vp = ps_sm.tile([128, 512], fp32, tag="sm", name="vp")
                for k in range(KP):
                    nc.tensor.matmul(
                        vp[:, 0:2 * DV],
                        xt_sb[:, k, b * T + r * 128: b * T + (r + 1) * 128],
                        wv_sb[:, k, :],
                        start=(k == 0), stop=(k == KP - 1),
                    )
                nc.vector.tensor_tensor(
                    vaug[:, b, r, :], vp[:, 0:2 * DV], bv_sb[:], op=ADD,
                )

            def sc(b, qc, kt):
                # scores^T [kj=128, qi=512] per head, heads side by side
                S = ps_s.tile([128, 2 * 512], fp32, tag="S", name="S")
                for h in range(HL):
                    nc.tensor.matmul(
                        S[:, h * 512:(h + 1) * 512],
                        qkT[h * D:(h + 1) * D, b, 1, kt * 128:(kt + 1) * 128],
                        qkT[h * D:(h + 1) * D, b, 0, qc * 512:(qc + 1) * 512],
                        start=True, stop=True,
                    )
                return S

            def vmm_one(tile, c0, P, b, h, qs, kt):
                # o[q-subtile, d+1] += P^T[kj, q-sub]^T @ vaug[kj, d+1]
                nc.tensor.matmul(
                    tile[:, c0:c0 + DV],
                    P[:, h * 512 + qs * 128: h * 512 + (qs + 1) * 128],
                    vaug[:, b, kt, h * DV:(h + 1) * DV],
                    start=(kt == 0), stop=(kt == KT - 1),
                )

            def region_norm(b, qc, tile, c0, h, qs, on_act=False):
                # one region's reciprocal + normalize (tail interleaving)
                rdt = npool.tile([128, 8], fp32, tag="rd", name="rdt")
                nc.vector.reciprocal(
                    rdt[:, h * 4 + qs: h * 4 + qs + 1],
                    tile[:, c0 + D: c0 + DV])
                qt = qc * QS + qs
                if on_act:
                    nc.scalar.activation(
                        o_sb[b][:, qt, h * 64:(h + 1) * 64],
                        tile[:, c0: c0 + D], COPY,
                        scale=rdt[:, h * 4 + qs: h * 4 + qs + 1])
                else:
                    nc.vector.tensor_scalar(
                        o_sb[b][:, qt, h * 64:(h + 1) * 64],
                        tile[:, c0: c0 + D],
                        rdt[:, h * 4 + qs: h * 4 + qs + 1],
                        None, op0=MULT,
                    )

            def finalize(b, qc, ot, via_pe=False, skip_norms=False):
                # All norms (both heads) must precede the transposes: each
                # transpose reads a 128-col window spanning both head
                # slots.  via_pe: for the very last group the DMA
                # transpose+dup chain (~650ns queue overhead per hop) sits
                # exposed on the critical tail, so route it through PE
                # identity-matmuls + DVE/ACT copies instead (idle there).
                rd = npool.tile([128, 8], fp32, tag="rd", name="rd")
                if not skip_norms:
                    for h in range(HL):
                        nc.vector.reciprocal(
                            rd[:, h * 4:(h + 1) * 4].rearrange(
                                "p (a c) -> p a c", c=1),
                            ot[h].rearrange(
                                "p (q c) -> p q c", c=DV)[:, :, D:D + 1],
                        )
                    for qs in range(QS):
                        qt = qc * QS + qs
                        for h in range(HL):
                            nc.vector.tensor_scalar(
                                o_sb[b][:, qt, h * 64:(h + 1) * 64],
                                ot[h][:, qs * DV: qs * DV + D],
                                rd[:, h * 4 + qs: h * 4 + qs + 1],
                                None, op0=MULT,
                            )
                for qs in range(QS):
                    qt = qc * QS + qs
                    for h in range(HL):
                        # o^T; out rows 64-127 get the neighbour head slot /
                        # pad lanes, overwritten by the shifted dup below
                        if via_pe:
                            # bf16 [128,512] = same slot bytes as the sm tag
                            tp = ps_sm.tile([128, 512], bf16, tag="sm",
                                            name="tp")
                            nc.tensor.transpose(
                                tp[:, 0:128],
                                o_sb[b][:, qt, h * 64:h * 64 + 128],
                                ident[:])
                            nc.vector.tensor_copy(
                                oTd[b][0:128, h, qt * 128:(qt + 1) * 128],
                                tp[:, 0:128])
                        else:
                            nc.sync.dma_start_transpose(
                                oTd[b][0:128, h, qt * 128:(qt + 1) * 128],
                                o_sb[b][:, qt, h * 64:h * 64 + 128],
                            )
                for h in range(HL):
                    # shifted dup: via_pe does qc-1 + qc chunks; the DMA
                    # route defers to one whole-row dup after the last qc
                    # (fewer DMA producers -> fewer merged waits downstream)
                    chunks = []
                    if via_pe:
                        # cols 0..1023 were dup'd early (qc==2 below); the
                        # final group owns the last two 512-col pieces
                        chunks = [(1024, 512), (1536, 511)]
                    elif b == B - 1 and qc == QC - 2:
                        # early dup for the tail batch: sources complete
                        # through this group's transposes
                        chunks = [(0, 512), (512, 512)]
                    elif qc == QC - 1:
                        chunks.append((0, T - 1))
                    for c0, cw in chunks:
                        if via_pe:
                            dp = ps_sm.tile([128, 512], fp32, tag="sm",
                                            name="dp")
                            nc.tensor.matmul(
                                dp[0:64, 0:cw], ident[0:64, 0:64],
                                oTd[b][0:64, h, c0 + 1:c0 + cw + 1],
                                start=True, stop=True)
                            if h == 0:
                                nc.scalar.activation(
                                    oTd[b][64:128, h, c0:c0 + cw],
                                    dp[0:64, 0:cw], COPY)
                            else:
                                nc.vector.tensor_copy(
                                    oTd[b][64:128, h, c0:c0 + cw],
                                    dp[0:64, 0:cw])
                        else:
                            nc.sync.dma_start(
                                oTd[b][64:128, h, c0:c0 + cw],
                                oTd[b][0:64, h, c0 + 1:c0 + cw + 1])

            def outproj_piece(b, h, n5, s, split, tag="sm"):
                # one 1/split column chunk of out_proj for (b, h, n5); bias
                # folded in as a ones-row matmul so y DMAs straight out of
                # PSUM (no DVE hop on the critical tail)
                of2 = oTd[b][:, h, :].rearrange("p (t j) -> p j t", j=16)
                w = 512 // split
                c0 = n5 * 512 + s * w
                yp = ps_sm.tile([128, 512], fp32, tag=tag, name="yp") \
                    if tag == "sm" else \
                    ps_s.tile([128, 512], fp32, tag=tag, name="yp")
                for jj in range(8):
                    nc.tensor.matmul(
                        yp[:, 0:w],
                        of2[:, 2 * jj, :],
                        wout_sb[:, jj, c0:c0 + w],
                        start=(jj == 0), stop=False,
                    )
                nc.tensor.matmul(
                    yp[:, 0:w], ones1[0:1, :], bout_bf[0:1, c0:c0 + w],
                    start=False, stop=True,
                )
                # PSUM cannot feed a DMA directly; stage through SBUF on
                # DVE mid-stream / ACT at the tail (idle there)
                ys = ypool.tile([128, 512], fp32, tag="ys", name="ys")
                if b == 0 or (h + n5 + s) % 2:
                    nc.vector.tensor_copy(ys[:, 0:w], yp[:, 0:w])
                else:
                    nc.scalar.activation(ys[:, 0:w], yp[:, 0:w], COPY)
                # mid-stream y-writes ride the gpsimd SWDGE ring: keeps
                # their deep dependency chains off the SP ring's semaphore
                # window (an SP sem recycle once stalled the exp stream
                # 14us).  Tail y-writes (b=1) alternate SP/gpsimd.
                eng = nc.sync if (h + n5 + s) % 2 else nc.gpsimd
                eng.dma_start(y_d[b, h, :, c0:c0 + w], ys[:, 0:w])

            # ---- filler schedule: all pieces ~0.43us (8 accumulating
            # matmuls of F<=130) so the score stream is never delayed by a
            # long chain sitting ahead of it in the PE queue.  Greedy
            # deadline placement, one piece per slot unless a deadline
            # forces more.
            SCHED = {}
            VROWS = {}

            def at_slot(slot, fn, vrow=None):
                key = (slot // 64, (slot % 64) // 16, slot % 16)
                SCHED.setdefault(key, []).append(fn)
                if vrow is not None:
                    VROWS.setdefault(key, []).append(vrow)

            def chunk_slot(c):
                # xt chunk c lands on the gpsimd ring ~2.9us apart; convert
                # to the exp-slot index from which a piece may read it
                if c == 0:
                    return -100
                return int((7814 + 2913 * c + 900 - 11090) / 1038) + 1

            pieces = []  # [deadline, earliest, fn, vrow]
            for b in range(B):
                for kt in range(KT):
                    if b == 0 and kt < 2:
                        continue  # prologue
                    c = (b * T + kt * 128) // 512
                    pieces.append(
                        [64 * b + kt - 2, chunk_slot(c),
                         (lambda bb, kk: lambda: proj_k(bb, kk))(b, kt),
                         None])
                for r in range(KT):
                    c = (b * T + r * 128) // 512
                    pieces.append(
                        [(11 if b == 0 else 52) + r // 2, chunk_slot(c),
                         (lambda bb, rr: lambda: proj_v(bb, rr))(b, r),
                         (b, r)])
                for n in range(QC):
                    if b == 0 and n == 0:
                        continue  # prologue
                    for qs in range(QS):
                        c = (b * T + n * 512 + qs * 128) // 512
                        pieces.append(
                            [64 * b + 16 * n - 3, chunk_slot(c),
                             (lambda bb, nn, qq:
                              lambda: proj_q_qs(bb, nn, qq))(b, n, qs),
                             None])
            for h in range(HL):
                for n5 in range(2):
                    for s in range(2):
                        pieces.append(
                            [100 + 3 * (4 * h + 2 * n5 + s), 95,
                             (lambda hh, nn, ss:
                              lambda: outproj_piece(0, hh, nn, ss, 2))(
                                  h, n5, s),
                             None])
            todo = sorted(pieces, key=lambda p: (p[0], p[1]))
            for slot in range(B * QC * KT):
                # keep group-boundary slots free: the finalize/norm chain
                # must reach the DVE queue head unimpeded.  Deadline-due
                # pieces are placed unconditionally (a qkT piece placed past
                # its deadline would be read stale by the score lookahead).
                boundary = slot % KT in (15, 0)
                n = 1 if boundary else 0
                for p in todo[:]:
                    if p[1] > slot:
                        continue
                    if p[0] <= slot:
                        at_slot(slot, p[2], p[3])
                        todo.remove(p)
                        n += 1
                    elif n == 0:
                        at_slot(slot, p[2], p[3])
                        todo.remove(p)
                        n += 1
                    else:
                        break
            assert not todo, [p[:2] for p in todo]

            # wout/bout on the gpsimd ring once startup DMA traffic is done
            at_slot(33, lambda: nc.gpsimd.dma_start(wout_sb[:], wout_d[:]))
            at_slot(35, lambda: nc.gpsimd.dma_start(bout_sb[:], bout_d[:]))
            at_slot(37, lambda: nc.vector.tensor_copy(
                bout_bf[0:1, :], bout_sb[0:1, :]))

            # ---- unified emission ----
            def _emit_all():
                seq = [(b, qc, kt) for b in range(B) for qc in range(QC)
                       for kt in range(KT)]
                vrows = {0: 0, 1: 0}
                # PSUM accumulation: one pending group per 2KB zero region
                # (bank).  Each o bank (og0=h0, og1=h1) streams its qs=0
                # region kt-incrementally through the group; regions qs=1-3
                # drain region-major after the group's last kt, two regions
                # per slot.
                pmap = {}            # (b,qc) -> {kt: P}
                stream_kts = {}      # (b,qc) -> kts streamed (qs=0 regions)
                drain_q = []         # groups past kt=15, awaiting qs 1-3
                drained = {}         # (b,qc) -> drained region count (0..6)
                otiles = {}          # (b,qc) -> [o_h0, o_h1]
                finalized = []       # groups finalized, in order
                stream_q = []        # groups awaiting/undergoing streaming

                LASTG = (B - 1, QC - 1)

                def alloc_group(g):
                    otiles[g] = [
                        ps_o.tile([128, QS * DV], fp32, tag=f"og{h}",
                                  name=f"og{h}")
                        for h in range(HL)
                    ]
                    if g == LASTG:
                        # two extra banks from the sm tag: four streaming
                        # regions, so only qs=2,3 drain after the last exp
                        otiles[g] += [
                            ps_sm.tile([128, 512], fp32, tag="sm",
                                       name=f"ogx{h}")
                            for h in range(HL)
                        ]

                def rmap(g, h, qs):
                    # region (h, qs) -> (tile, col0).  Last group: qs 0,2 in
                    # og_h (cols 0/65), qs 1,3 in the sm extras.
                    if g == LASTG:
                        if qs % 2 == 0:
                            return otiles[g][h], (qs // 2) * DV
                        return otiles[g][2 + h], (qs // 2) * DV
                    return otiles[g][h], qs * DV

                def group_done(g, skip_norms=False):
                    finalize(g[0], g[1], otiles[g],
                             via_pe=(g == (B - 1, QC - 1)),
                             skip_norms=skip_norms)
                    finalized.append(g)
                    otiles.pop(g)

                def flush(now_i, all_=False):
                    # 1) drain the head of drain_q: regions (qs>=1), two per
                    # call (region = 16 matmuls of F=65 ~ 0.43us each)
                    nreg = 1000 if all_ else 2
                    while drain_q and nreg > 0:
                        g = drain_q[0]
                        last = all_ and g == LASTG
                        qs0 = 2 if g == LASTG else 1
                        ndrain = HL * (QS - qs0)
                        d = drained.get(g, 0)
                        if last and d == 0:
                            # tail: the streamed regions are closed - norm
                            # them while PE drains the rest
                            for qs in range(qs0):
                                for h in range(HL):
                                    t_, c_ = rmap(g, h, qs)
                                    region_norm(g[0], g[1], t_, c_, h, qs,
                                                on_act=(h == 1))
                        take = min(ndrain - d, nreg)
                        for idx in range(d, d + take):
                            qs = qs0 + idx // HL
                            h = idx % HL
                            t_, c_ = rmap(g, h, qs)
                            for kt in range(KT):
                                vmm_one(t_, c_, pmap[g][kt], g[0],
                                        h, qs, kt)
                            if last:
                                region_norm(g[0], g[1], t_, c_, h, qs,
                                            on_act=(idx % 2 == 1))
                        drained[g] = d + take
                        nreg -= take
                        if drained[g] == ndrain:
                            drain_q.pop(0)
                            pmap.pop(g)
                            group_done(g, skip_norms=last)
                    # 2) stream qs=0 regions (head of stream_q) as kts and
                    # vaug rows become available; a group may only take the
                    # o banks once the previous group has fully vacated them
                    while stream_q:
                        g = stream_q[0]
                        if g not in otiles:
                            if otiles:
                                break    # banks still owned by prior group
                            alloc_group(g)
                        hi = min(max(pmap[g].keys()) + 1 if pmap[g] else 0,
                                 vrows[g[0]])
                        sk = stream_kts.get(g, 0)
                        nstream = 2 if g == LASTG else 1
                        for kt in range(sk, hi):
                            for qs in range(nstream):
                                for h in range(HL):
                                    t_, c_ = rmap(g, h, qs)
                                    vmm_one(t_, c_, pmap[g][kt], g[0],
                                            h, qs, kt)
                        stream_kts[g] = max(sk, hi)
                        if stream_kts[g] == KT:
                            drain_q.append(g)
                            stream_q.pop(0)
                            continue
                        break

                # ---- prologue: first score tile in column halves so the
                # exp stream starts as soon as the first xt piece lands ----
                warmup(52)
                proj_q_qs(0, 0, 0)
                proj_q_qs(0, 0, 1)
                proj_k(0, 0)
                proj_k(0, 1)
                S = ps_s.tile([128, 2 * 512], fp32, tag="S", name="S")
                for h in range(HL):
                    nc.tensor.matmul(
                        S[:, h * 512:h * 512 + 256],
                        qkT[h * D:(h + 1) * D, 0, 1, 0:128],
                        qkT[h * D:(h + 1) * D, 0, 0, 0:256],
                        start=True, stop=True)
                P0 = ppool.tile([128, 2 * 512], bf16, tag="P", name="P0")
                Sv0 = S.rearrange("p (h c) -> p h c", h=HL)
                Pv0 = P0.rearrange("p (h c) -> p h c", h=HL)
                nc.scalar.activation(Pv0[:, :, 0:256], Sv0[:, :, 0:256],
                                     EXP, scale=0.125)
                proj_q_qs(0, 0, 2)
                proj_q_qs(0, 0, 3)
                for h in range(HL):
                    nc.tensor.matmul(
                        S[:, h * 512 + 256:(h + 1) * 512],
                        qkT[h * D:(h + 1) * D, 0, 1, 0:128],
                        qkT[h * D:(h + 1) * D, 0, 0, 256:512],
                        start=True, stop=True)

                P_last = None
                for i, (b, qc, kt) in enumerate(seq):
                    if i == 0:
                        # first half emitted in the prologue on tile P0
                        P = P0
                        nc.scalar.activation(Pv0[:, :, 256:512],
                                             Sv0[:, :, 256:512],
                                             EXP, scale=0.125)
                    elif i == len(seq) - 1:
                        P = ppool.tile([128, 2 * 512], bf16, tag="P")
                        Svl = S.rearrange("p (h c) -> p h c", h=HL)
                        Pvl = P.rearrange("p (h c) -> p h c", h=HL)
                        nc.scalar.activation(Pvl[:, :, 0:256],
                                             Svl[:, :, 0:256],
                                             EXP, scale=0.125)
                        nc.scalar.activation(Pvl[:, :, 256:512],
                                             Svl[:, :, 256:512],
                                             EXP, scale=0.125)
                    else:
                        P = ppool.tile([128, 2 * 512], bf16, tag="P")
                        nc.scalar.activation(P[:], S[:], EXP, scale=0.125)
                    P_last = P
                    if i + 1 < len(seq):
                        S = sc(*seq[i + 1])
                    for fn in SCHED.get((b, qc, kt), []):
                        fn()
                    for (vb, r) in VROWS.get((b, qc, kt), []):
                        vrows[vb] = max(vrows[vb], r + 1)
                    if kt == 0:
                        stream_q.append((b, qc))
                        pmap[(b, qc)] = {}
                    pmap[(b, qc)][kt] = P
                    flush(i)

                # ---- tail ----
                while drain_q or stream_q:
                    flush(len(seq), all_=True)
                assert len(finalized) == B * QC, finalized
                # keep the PE p-state hot across the norm/transpose/dup
                # wait; chained on the last P tile so the scheduler cannot
                # hoist these out of the tail window
                warmup(8, src=P_last)
                for h in range(HL):
                    for n5 in range(2):
                        for s in range(2):
                            outproj_piece(1, h, n5, s, 2,
                                          tag=("S" if (n5 + s) % 2 else "sm"))

            _emit_all()

    nc.compile()
    return nc


def _get_runner():
    """Build + compile once; return a callable(in_maps) -> list of out dicts."""
    global _RUNNER
    if _RUNNER is not None:
        return _RUNNER

    import jax
    import concourse.mybir as mybir
    from concourse import bass2jax
    from jax.experimental.shard_map import shard_map
    from jax.sharding import Mesh, PartitionSpec

    nc = _build_nc()
    bass2jax.install_neuronx_cc_hook()

    partition_name = (nc.partition_id_tensor.name
                      if nc.partition_id_tensor else None)
    in_names, out_names, out_avals = [], [], []
    for alloc in nc.m.functions[0].allocations:
        if not isinstance(alloc, mybir.MemoryLocationSet):
            continue
        name = alloc.memorylocations[0].name
        if alloc.kind == "ExternalInput":
            if name != partition_name:
                in_names.append(name)
        elif alloc.kind == "ExternalOutput":
            out_names.append(name)
            out_avals.append(jax.core.ShapedArray(
                tuple(alloc.tensor_shape), mybir.dt.np(alloc.dtype)))

    n_params, n_outs = len(in_names), len(out_avals)
    all_names = in_names + out_names
    if partition_name is not None:
        all_names = all_names + [partition_name]

    def _body(*args):
        operands = list(args)
        if partition_name is not None:
            operands.append(bass2jax.partition_id_tensor())
        outs = bass2jax._bass_exec_p.bind(
            *operands,
            out_avals=tuple(out_avals),
            in_names=tuple(all_names),
            out_names=tuple(out_names),
            lowering_input_output_aliases=(),
            sim_require_finite=True,
            sim_require_nnan=True,
            nc=nc,
        )
        return tuple(outs)

    devices = jax.devices()[:N_CORES]
    mesh = Mesh(np.asarray(devices), ("core",))
    in_specs = (PartitionSpec("core"),) * (n_params + n_outs)
    out_specs = (PartitionSpec("core"),) * n_outs
    donate = tuple(range(n_params, n_params + n_outs))
    sharded = jax.jit(
        shard_map(_body, mesh=mesh, in_specs=in_specs, out_specs=out_specs,
                  check_rep=False),
        donate_argnums=donate, keep_unused=True,
    )

    def run(in_maps):
        concat_in = [
            np.concatenate([np.asarray(in_maps[c][nm]) for c in range(N_CORES)],
                           axis=0)
            for nm in in_names
        ]
        concat_zeros = [
            np.zeros((N_CORES * a.shape[0], *a.shape[1:]), a.dtype)
            for a in out_avals
        ]
        out_arrs = sharded(*concat_in, *concat_zeros)
        return [
            {nm: np.asarray(out_arrs[i]).reshape(N_CORES, *out_avals[i].shape)[c]
             for i, nm in enumerate(out_names)}
            for c in range(N_CORES)
        ]

    _RUNNER = run
    run._bench_parts = (sharded, mesh, in_names, out_names, out_avals,
                        n_params, _body)
    return run


def _make_bench(in_maps):
    """Device-resident benchmark closure: returns fn() that runs one
    execution with all inputs already on device (no donation)."""
    import jax
    from jax.experimental.shard_map import shard_map
    from jax.sharding import NamedSharding, PartitionSpec

    run = _get_runner()
    sharded, mesh, in_names, out_names, out_avals, n_params, _body = \
        run._bench_parts
    sh = NamedSharding(mesh, PartitionSpec("core"))

    nodonate = jax.jit(
        shard_map(_body, mesh=mesh,
                  in_specs=(PartitionSpec("core"),) * (n_params + len(out_avals)),
                  out_specs=(PartitionSpec("core"),) * len(out_avals),
                  check_rep=False),
        keep_unused=True,
    )
    concat_in = [
        np.concatenate([np.asarray(in_maps[c][nm]) for c in range(N_CORES)], axis=0)
        for nm in in_names
    ]
    concat_zeros = [
        np.zeros((N_CORES * a.shape[0], *a.shape[1:]), a.dtype) for a in out_avals
    ]
    dev_args = [jax.device_put(a, sh) for a in concat_in + concat_zeros]
    for a in dev_args:
        a.block_until_ready()

    def bench_once():
        outs = nodonate(*dev_args)
        for o in outs:
            o.block_until_ready()
        return outs

    bench_once.nodonate = nodonate
    bench_once.dev_args = dev_args
    return bench_once


def _prep_in_maps(x, W_qkv, b_qkv, W_out, b_out):
    bf = ml_dtypes.bfloat16
    xt = np.ascontiguousarray(
        x.reshape(B * T, E).T).astype(bf)                      # [E, B*T]
    wout = np.ascontiguousarray(
        W_out.reshape(8, 128, E).transpose(1, 0, 2).reshape(128, 8 * E)).astype(bf)
    bout = np.ascontiguousarray(
        np.broadcast_to(b_out.astype(np.float32)[None, :], (128, E)))

    in_maps = []
    for c in range(N_CORES):
        hs = [HL * c + i for i in range(HL)]
        qcols = np.concatenate(
            [W_qkv[:, 0 * E + h * D:0 * E + (h + 1) * D] for h in hs], axis=1)
        kcols = np.concatenate(
            [W_qkv[:, 1 * E + h * D:1 * E + (h + 1) * D] for h in hs], axis=1)
        wqk = np.ascontiguousarray(
            np.concatenate([qcols, kcols], axis=1)).astype(bf)  # [E, 256]
        zcol = np.zeros((E, 1), np.float32)
        wv = np.ascontiguousarray(np.concatenate(
            [arr for h in hs
             for arr in (W_qkv[:, 2 * E + h * D:2 * E + (h + 1) * D], zcol)],
            axis=1)).astype(bf)                                 # [E, 130]
        bq = np.concatenate([b_qkv[0 * E + h * D:0 * E + (h + 1) * D] for h in hs])
        bk = np.concatenate([b_qkv[1 * E + h * D:1 * E + (h + 1) * D] for h in hs])
        bqk = np.ascontiguousarray(
            np.stack([bq, bk], axis=1)).astype(np.float32)      # [128, 2]
        one = np.ones(1, np.float32)
        bvv = np.concatenate(
            [a for h in hs
             for a in (b_qkv[2 * E + h * D:2 * E + (h + 1) * D], one)])
        bv = np.ascontiguousarray(
            np.broadcast_to(bvv.astype(np.float32)[None, :], (128, 2 * DV)))
        in_maps.append({
            "xt": xt, "wqk": wqk, "wv": wv, "wout": wout,
            "bqk": bqk, "bv": bv, "bout": bout,
        })
    return in_maps


def kernel(x, W_qkv, b_qkv, W_out, b_out):
    x = np.asarray(x, dtype=np.float32)
    W_qkv = np.asarray(W_qkv, dtype=np.float32)
    b_qkv = np.asarray(b_qkv, dtype=np.float32)
    W_out = np.asarray(W_out, dtype=np.float32)
    b_out = np.asarray(b_out, dtype=np.float32)

    run = _get_runner()
    in_maps = _prep_in_maps(x, W_qkv, b_qkv, W_out, b_out)
    results = run(in_maps)

    out = np.empty((B, T, E), np.float32)
    for c in range(N_CORES):
        y = results[c]["y"]          # [B, HL, 128, E]
        for hl in range(HL):
            hg = HL * c + hl
            out[:, hg * 128:(hg + 1) * 128, :] = y[:, hl]
    return out


# revision 6
# speedup vs baseline: 1.2795x; 1.0039x over previous
"""Multi-head self-attention TRN2 kernel (B=2, T=2048, E=1024, H=16, D=64).

Sharding: tensor-parallel over heads - each of the 8 cores owns 2 heads.
Because the reference reshapes (B,H,T,D)->(B,T,E) with NO transpose, each
head's attention output maps to 128 complete contiguous rows of the
out_proj input, so the whole computation is embarrassingly parallel
across heads (no collectives).

Per-core pipeline (all matmuls bf16, accumulation fp32):
  1. qT/kT projections in [d, T] layout (heads stacked on partitions
     0-63 / 64-127); v in natural [T, d] layout augmented with a ones
     column (softmax denominator for free).
  2. scores^T tiles [kj=128, qi=512x2h] -> exp on ScalarE -> P^T bf16.
  3. attn@v in the cheap orientation: out o[q=128 partitions, d+1=65
     free] accumulated over kj tiles - PE cost 65 free/step instead of
     512 (the cost model charges output free size per accumulation
     step, so small-free x many-partitions wins 8x per instruction).
  4. normalize by per-partition (per-query) reciprocal on VectorE,
     downcast to bf16.
  5. o -> o^T via DMA xbar transposes (off the PE critical path), then
     shifted partition-duplicate copies build the [128=(d,j-parity), T]
     lhsT layout that performs the reference's "faithful reshape" for
     free in out_proj.
  6. out_proj: 8 accumulating matmuls per 512-column chunk.
"""

import numpy as np
import ml_dtypes

B, T, E, H, D = 2, 2048, 1024, 16, 64
N_CORES = 8
HL = H // N_CORES          # heads per core = 2
KP = E // 128              # 8 contraction partition-tiles
KT = T // 128              # 16 kj tiles
QC = T // 512              # 4 qi chunks of 512
QS = 4                     # q subtiles of 128 per chunk
DV = D + 1                 # v width incl. denominator ones column

_RUNNER = None


def _build_nc():
    import concourse.bacc as bacc
    import concourse.tile as tile
    import concourse.bass as bass
    import concourse.mybir as mybir

    from concourse import masks

    fp32 = mybir.dt.float32
    bf16 = mybir.dt.bfloat16
    ADD = mybir.AluOpType.add
    MULT = mybir.AluOpType.mult
    EXP = mybir.ActivationFunctionType.Exp
    COPY = mybir.ActivationFunctionType.Copy

    nc = bacc.Bacc("TRN2", target_bir_lowering=False, debug=False,
                   enable_asserts=True, num_devices=N_CORES)

    xt_d = nc.dram_tensor("xt", [E, B * T], bf16, kind="ExternalInput").ap()
    wqk_d = nc.dram_tensor("wqk", [E, 4 * D], bf16, kind="ExternalInput").ap()
    wv_d = nc.dram_tensor("wv", [E, 2 * DV], bf16, kind="ExternalInput").ap()
    wout_d = nc.dram_tensor("wout", [128, 8 * E], bf16, kind="ExternalInput").ap()
    bqk_d = nc.dram_tensor("bqk", [128, 2], fp32, kind="ExternalInput").ap()
    bv_d = nc.dram_tensor("bv", [128, 2 * DV], fp32, kind="ExternalInput").ap()
    bout_d = nc.dram_tensor("bout", [128, E], fp32, kind="ExternalInput").ap()
    y_d = nc.dram_tensor("y", [B, HL, 128, E], fp32, kind="ExternalOutput").ap()

    with tile.TileContext(nc) as tc:
        with (
            tc.tile_pool(name="const", bufs=1) as cpool,
            tc.tile_pool(name="ppool", bufs=26) as ppool,
            tc.tile_pool(name="npool", bufs=4) as npool,
            tc.tile_pool(name="ypool", bufs=4) as ypool,
            tc.tile_pool(name="ps_s", bufs=2, space=bass.MemorySpace.PSUM) as ps_s,
            tc.tile_pool(name="ps_o", bufs=1, space=bass.MemorySpace.PSUM) as ps_o,
            tc.tile_pool(name="ps_sm", bufs=2, space=bass.MemorySpace.PSUM) as ps_sm,
        ):
            # ---- constants / persistent tiles ----
            xt_sb = cpool.tile([128, KP, B * T], bf16, tag="xt")
            wqk_sb = cpool.tile([128, KP, 4 * D], bf16, tag="wqk")
            wv_sb = cpool.tile([128, KP, 2 * DV], bf16, tag="wv")
            wout_sb = cpool.tile([128, 8, E], bf16, tag="wout")
            bqk_sb = cpool.tile([128, 2], fp32, tag="bqk")
            bv_sb = cpool.tile([128, 2 * DV], fp32, tag="bv")
            bout_sb = cpool.tile([128, E], fp32, tag="bout")
            qkT = cpool.tile([128, B, 2, T], bf16, tag="qkT")
            vaug = cpool.tile([128, B, KT, 2 * DV], bf16, tag="vaug")
            # o natural layout, normalized bf16, 64 pad cols per head slot
            # (the pad transposes into oTd rows 64-127, which the shifted
            # dup copy then overwrites)
            # per-batch tensors (separate tags) so cross-batch false
            # dependencies cannot arise from coarse subtile tracking
            o_sb = [cpool.tile([128, KT, 192], bf16, tag=f"o_sb{bb}",
                               name=f"o_sb{bb}") for bb in range(B)]
            # o^T dup layout per head: rows 0-63 straight (written directly
            # by the xbar transpose), 64-127 shifted by one q (dup DMA)
            oTd = [cpool.tile([128, HL, T], bf16, tag=f"oTd{bb}",
                              name=f"oTd{bb}") for bb in range(B)]
            wub = cpool.tile([1, 128], bf16, tag="wub")
            ones1 = cpool.tile([1, 128], bf16, tag="ones1")
            bout_bf = cpool.tile([1, E], bf16, tag="bout_bf")
            ident = cpool.tile([128, 128], bf16, tag="ident")

            # warmup source (no DMA dependency); zero o_sb's pad lanes once
            nc.gpsimd.memset(wub[:], 0.0)
            nc.gpsimd.memset(ones1[:], 1.0)
            masks.make_identity(nc, ident[:])

            # ---- input DMAs: spread across the three HWDGE-ish rings so
            # queue overheads pipeline.  Transfers serialize on the shared
            # DMA engines; order = priority.  The first score tile needs
            # k cols + q[0:512], so those pieces go first, smallest first.
            wqk_r = wqk_d.rearrange("(a p) n -> p a n", p=128)
            xt_r = xt_d.rearrange("(a p) n -> p a n", p=128)
            nc.sync.dma_start(wqk_sb[:], wqk_r[:])
            # first 512 T-columns on the ACT ring (idle until exps start);
            # bigger slices keep the 1KB contiguous runs (no small-elem
            # DMA penalty)
            nc.scalar.dma_start(xt_sb[:, :, 0:256], xt_r[:, :, 0:256])
            nc.scalar.dma_start(xt_sb[:, :, 256:512], xt_r[:, :, 256:512])
            nc.scalar.dma_start(xt_sb[:, :, 512:1024], xt_r[:, :, 512:1024])
            nc.sync.dma_start(bqk_sb[:], bqk_d[:])
            nc.sync.dma_start(bv_sb[:], bv_d[:])
            # o_sb pad lanes zeroed on the gpsimd engine: doubles as a
            # delay so the Pool ring's chunk DMAs queue at the shared DMA
            # engines AFTER the ACT ring's critical first pieces
            for bb in range(B):
                nc.gpsimd.memset(o_sb[bb][:, :, 128:192], 0.0)
            # rest of x on the gpsimd SWDGE ring (engine otherwise idle);
            # wv mid-way (not needed until ~slot 11)
            for cc in range(2, B * T // 512):
                if cc == 4:
                    nc.gpsimd.dma_start(
                        wv_sb[:], wv_d.rearrange("(a p) n -> p a n", p=128))
                nc.gpsimd.dma_start(xt_sb[:, :, cc * 512:(cc + 1) * 512],
                                    xt_r[:, :, cc * 512:(cc + 1) * 512])

            # ---- PE warmup: keep the clock ramping from t~0 so the first
            # real matmuls run at full p-state.  src defaults to wub (no
            # deps -> the scheduler hoists them to kernel start); pass a
            # late-written AP to pin warmups into a late window instead.
            def warmup(n, src=None):
                s = wub if src is None else src
                for _ in range(n):
                    wt = ps_sm.tile([128, 128], fp32, tag="sm", name="wt")
                    nc.tensor.matmul(wt[:], s[0:1, 0:128], s[0:1, 0:128],
                                     start=True, stop=True)

            # ---- projection pieces ----
            def proj_q(b, n):
                # q^T chunk n: [128=(h0|h1)*d, 512]
                ps = ps_sm.tile([128, 512], fp32, tag="sm", name="psq")
                for k in range(KP):
                    nc.tensor.matmul(
                        ps[:],
                        wqk_sb[:, k, 0:128],
                        xt_sb[:, k, b * T + n * 512: b * T + (n + 1) * 512],
                        start=(k == 0), stop=(k == KP - 1),
                    )
                nc.vector.tensor_scalar(
                    qkT[:, b, 0, n * 512:(n + 1) * 512], ps[:],
                    bqk_sb[:, 0:1], None, op0=ADD,
                )

            def proj_q_half(b, n, hf):
                ps = ps_sm.tile([128, 512], fp32, tag="sm", name="psq")
                c0 = n * 512 + hf * 256
                for k in range(KP):
                    nc.tensor.matmul(
                        ps[:, 0:256],
                        wqk_sb[:, k, 0:128],
                        xt_sb[:, k, b * T + c0: b * T + c0 + 256],
                        start=(k == 0), stop=(k == KP - 1),
                    )
                nc.vector.tensor_scalar(
                    qkT[:, b, 0, c0:c0 + 256], ps[:, 0:256],
                    bqk_sb[:, 0:1], None, op0=ADD,
                )

            def proj_q_qs(b, n, qs):
                ps = ps_sm.tile([128, 512], fp32, tag="sm", name="psq")
                c0 = n * 512 + qs * 128
                for k in range(KP):
                    nc.tensor.matmul(
                        ps[:, 0:128],
                        wqk_sb[:, k, 0:128],
                        xt_sb[:, k, b * T + c0: b * T + c0 + 128],
                        start=(k == 0), stop=(k == KP - 1),
                    )
                nc.vector.tensor_scalar(
                    qkT[:, b, 0, c0:c0 + 128], ps[:, 0:128],
                    bqk_sb[:, 0:1], None, op0=ADD,
                )

            def proj_k(b, kt):
                # k^T tile kt: [128, 128]
                ps = ps_sm.tile([128, 512], fp32, tag="sm", name="psk")
                for k in range(KP):
                    nc.tensor.matmul(
                        ps[:, 0:128],
                        wqk_sb[:, k, 128:256],
                        xt_sb[:, k, b * T + kt * 128: b * T + (kt + 1) * 128],
                        start=(k == 0), stop=(k == KP - 1),
                    )
                nc.vector.tensor_scalar(
                    qkT[:, b, 1, kt * 128:(kt + 1) * 128], ps[:, 0:128],
                    bqk_sb[:, 1:2], None, op0=ADD,
                )

            def proj_v(b, r):
                # v natural [128, 2*DV] row tile r; W_v has zero columns at
                # the two "ones" slots and bv carries 1.0 there
                vp = ps_sm.tile([128, 512], fp32, tag="sm", name="vp")
                for k in range(KP):
                    nc.tensor.matmul(
                        vp[:, 0:2 * DV],
                        xt_sb[:, k, b * T + r * 128: b * T + (r + 1) * 128],
                        wv_sb[:, k, :],
                        start=(k == 0), stop=(k == KP - 1),
                    )
                nc.vector.tensor_tensor(
                    vaug[:, b, r, :], vp[:, 0:2 * DV], bv_sb[:], op=ADD,
                )

            def sc(b, qc, kt):
                # scores^T [kj=128, qi=512] per head, heads side by side
                S = ps_s.tile([128, 2 * 512], fp32, tag="S", name="S")
                for h in range(HL):
                    nc.tensor.matmul(
                        S[:, h * 512:(h + 1) * 512],
                        qkT[h * D:(h + 1) * D, b, 1, kt * 128:(kt + 1) * 128],
                        qkT[h * D:(h + 1) * D, b, 0, qc * 512:(qc + 1) * 512],
                        start=True, stop=True,
                    )
                return S

            def vmm_one(tile, c0, P, b, h, qs, kt):
                # o[q-subtile, d+1] += P^T[kj, q-sub]^T @ vaug[kj, d+1]
                nc.tensor.matmul(
                    tile[:, c0:c0 + DV],
                    P[:, h * 512 + qs * 128: h * 512 + (qs + 1) * 128],
                    vaug[:, b, kt, h * DV:(h + 1) * DV],
                    start=(kt == 0), stop=(kt == KT - 1),
                )

            def region_norm(b, qc, tile, c0, h, qs, on_act=False):
                # one region's reciprocal + normalize (tail interleaving)
                rdt = npool.tile([128, 8], fp32, tag="rd", name="rdt")
                nc.vector.reciprocal(
                    rdt[:, h * 4 + qs: h * 4 + qs + 1],
                    tile[:, c0 + D: c0 + DV])
                qt = qc * QS + qs
                if on_act:
                    nc.scalar.activation(
                        o_sb[b][:, qt, h * 64:(h + 1) * 64],
                        tile[:, c0: c0 + D], COPY,
                        scale=rdt[:, h * 4 + qs: h * 4 + qs + 1])
                else:
                    nc.vector.tensor_scalar(
                        o_sb[b][:, qt, h * 64:(h + 1) * 64],
                        tile[:, c0: c0 + D],
                        rdt[:, h * 4 + qs: h * 4 + qs + 1],
                        None, op0=MULT,
                    )

            def finalize(b, qc, ot, via_pe=False, skip_norms=False):
                # All norms (both heads) must precede the transposes: each
                # transpose reads a 128-col window spanning both head
                # slots.  via_pe: for the very last group the DMA
                # transpose+dup chain (~650ns queue overhead per hop) sits
                # exposed on the critical tail, so route it through PE
                # identity-matmuls + DVE/ACT copies instead (idle there).
                rd = npool.tile([128, 8], fp32, tag="rd", name="rd")
                if not skip_norms:
                    for h in range(HL):
                        nc.vector.reciprocal(
                            rd[:, h * 4:(h + 1) * 4].rearrange(
                                "p (a c) -> p a c", c=1),
                            ot[h].rearrange(
                                "p (q c) -> p q c", c=DV)[:, :, D:D + 1],
                        )
                    for qs in range(QS):
                        qt = qc * QS + qs
                        for h in range(HL):
                            nc.vector.tensor_scalar(
                                o_sb[b][:, qt, h * 64:(h + 1) * 64],
                                ot[h][:, qs * DV: qs * DV + D],
                                rd[:, h * 4 + qs: h * 4 + qs + 1],
                                None, op0=MULT,
                            )
                for qs in range(QS):
                    qt = qc * QS + qs
                    for h in range(HL):
                        # o^T; out rows 64-127 get the neighbour head slot /
                        # pad lanes, overwritten by the shifted dup below
                        if via_pe:
                            # bf16 [128,512] = same slot bytes as the sm tag
                            tp = ps_sm.tile([128, 512], bf16, tag="sm",
                                            name="tp")
                            nc.tensor.transpose(
                                tp[:, 0:128],
                                o_sb[b][:, qt, h * 64:h * 64 + 128],
                                ident[:])
                            nc.vector.tensor_copy(
                                oTd[b][0:128, h, qt * 128:(qt + 1) * 128],
                                tp[:, 0:128])
                        else:
                            nc.sync.dma_start_transpose(
                                oTd[b][0:128, h, qt * 128:(qt + 1) * 128],
                                o_sb[b][:, qt, h * 64:h * 64 + 128],
                            )
                for h in range(HL):
                    # shifted dup: via_pe does qc-1 + qc chunks; the DMA
                    # route defers to one whole-row dup after the last qc
                    # (fewer DMA producers -> fewer merged waits downstream)
                    chunks = []
                    if via_pe:
                        # cols 0..1023 were dup'd early (qc==2 below); the
                        # final group owns the last two 512-col pieces
                        chunks = [(1024, 512), (1536, 511)]
                    elif b == B - 1 and qc == QC - 2:
                        # early dup for the tail batch: sources complete
                        # through this group's transposes
                        chunks = [(0, 512), (512, 512)]
                    elif qc == QC - 1:
                        chunks.append((0, T - 1))
                    for c0, cw in chunks:
                        if via_pe:
                            dp = ps_sm.tile([128, 512], fp32, tag="sm",
                                            name="dp")
                            nc.tensor.matmul(
                                dp[0:64, 0:cw], ident[0:64, 0:64],
                                oTd[b][0:64, h, c0 + 1:c0 + cw + 1],
                                start=True, stop=True)
                            if h == 0:
                                nc.scalar.activation(
                                    oTd[b][64:128, h, c0:c0 + cw],
                                    dp[0:64, 0:cw], COPY)
                            else:
                                nc.vector.tensor_copy(
                                    oTd[b][64:128, h, c0:c0 + cw],
                                    dp[0:64, 0:cw])
                        else:
                            nc.sync.dma_start(
                                oTd[b][64:128, h, c0:c0 + cw],
                                oTd[b][0:64, h, c0 + 1:c0 + cw + 1])

            def outproj_piece(b, h, n5, s, split, tag="sm"):
                # one 1/split column chunk of out_proj for (b, h, n5); bias
                # folded in as a ones-row matmul so y DMAs straight out of
                # PSUM (no DVE hop on the critical tail)
                of2 = oTd[b][:, h, :].rearrange("p (t j) -> p j t", j=16)
                w = 512 // split
                c0 = n5 * 512 + s * w
                yp = ps_sm.tile([128, 512], fp32, tag=tag, name="yp") \
                    if tag == "sm" else \
                    ps_s.tile([128, 512], fp32, tag=tag, name="yp")
                for jj in range(8):
                    nc.tensor.matmul(
                        yp[:, 0:w],
                        of2[:, 2 * jj, :],
                        wout_sb[:, jj, c0:c0 + w],
                        start=(jj == 0), stop=False,
                    )
                nc.tensor.matmul(
                    yp[:, 0:w], ones1[0:1, :], bout_bf[0:1, c0:c0 + w],
                    start=False, stop=True,
                )
                # PSUM cannot feed a DMA directly; stage through SBUF on
                # DVE mid-stream / ACT at the tail (idle there)
                ys = ypool.tile([128, 512], fp32, tag="ys", name="ys")
                if b == 0 or (h + n5 + s) % 2:
                    nc.vector.tensor_copy(ys[:, 0:w], yp[:, 0:w])
                else:
                    nc.scalar.activation(ys[:, 0:w], yp[:, 0:w], COPY)
                # mid-stream y-writes ride the gpsimd SWDGE ring: keeps
                # their deep dependency chains off the SP ring's semaphore
                # window (an SP sem recycle once stalled the exp stream
                # 14us).  Tail y-writes (b=1) alternate SP/gpsimd.
                eng = nc.sync if (h + n5 + s) % 2 else nc.gpsimd
                eng.dma_start(y_d[b, h, :, c0:c0 + w], ys[:, 0:w])

            # ---- filler schedule: all pieces ~0.43us (8 accumulating
            # matmuls of F<=130) so the score stream is never delayed by a
            # long chain sitting ahead of it in the PE queue.  Greedy
            # deadline placement, one piece per slot unless a deadline
            # forces more.
            SCHED = {}
            VROWS = {}

            def at_slot(slot, fn, vrow=None):
                key = (slot // 64, (slot % 64) // 16, slot % 16)
                SCHED.setdefault(key, []).append(fn)
                if vrow is not None:
                    VROWS.setdefault(key, []).append(vrow)

            def chunk_slot(c):
                # xt chunk c lands on the gpsimd ring ~2.9us apart; convert
                # to the exp-slot index from which a piece may read it
                if c == 0:
                    return -100
                return int((7814 + 2913 * c + 900 - 11090) / 1038) + 1

            pieces = []  # [deadline, earliest, fn, vrow]
            for b in range(B):
                for kt in range(KT):
                    if b == 0 and kt < 2:
                        continue  # prologue
                    c = (b * T + kt * 128) // 512
                    pieces.append(
                        [64 * b + kt - 2, chunk_slot(c),
                         (lambda bb, kk: lambda: proj_k(bb, kk))(b, kt),
                         None])
                for r in range(KT):
                    c = (b * T + r * 128) // 512
                    pieces.append(
                        [(11 if b == 0 else 52) + r // 2, chunk_slot(c),
                         (lambda bb, rr: lambda: proj_v(bb, rr))(b, r),
                         (b, r)])
                for n in range(QC):
                    if b == 0 and n == 0:
                        continue  # prologue
                    for qs in range(QS):
                        c = (b * T + n * 512 + qs * 128) // 512
                        pieces.append(
                            [64 * b + 16 * n - 3, chunk_slot(c),
                             (lambda bb, nn, qq:
                              lambda: proj_q_qs(bb, nn, qq))(b, n, qs),
                             None])
            for h in range(HL):
                for n5 in range(2):
                    for s in range(2):
                        pieces.append(
                            [100 + 3 * (4 * h + 2 * n5 + s), 95,
                             (lambda hh, nn, ss:
                              lambda: outproj_piece(0, hh, nn, ss, 2))(
                                  h, n5, s),
                             None])
            todo = sorted(pieces, key=lambda p: (p[0], p[1]))
            for slot in range(B * QC * KT):
                # keep group-boundary slots free: the finalize/norm chain
                # must reach the DVE queue head unimpeded.  Deadline-due
                # pieces are placed unconditionally (a qkT piece placed past
                # its deadline would be read stale by the score lookahead).
                boundary = slot % KT in (15, 0)
                n = 1 if boundary else 0
                for p in todo[:]:
                    if p[1] > slot:
                        continue
                    if p[0] <= slot:
                        at_slot(slot, p[2], p[3])
                        todo.remove(p)
                        n += 1
                    elif n == 0:
                        at_slot(slot, p[2], p[3])
                        todo.remove(p)
                        n += 1
                    else:
                        break
            assert not todo, [p[:2] for p in todo]

            # wout/bout on the gpsimd ring once startup DMA traffic is done
            at_slot(33, lambda: nc.gpsimd.dma_start(wout_sb[:], wout_d[:]))
            at_slot(35, lambda: nc.gpsimd.dma_start(bout_sb[:], bout_d[:]))
            at_slot(37, lambda: nc.vector.tensor_copy(
                bout_bf[0:1, :], bout_sb[0:1, :]))

            # ---- unified emission ----
            def _emit_all():
                seq = [(b, qc, kt) for b in range(B) for qc in range(QC)
                       for kt in range(KT)]
                vrows = {0: 0, 1: 0}
                # PSUM accumulation: one pending group per 2KB zero region
                # (bank).  Each o bank (og0=h0, og1=h1) streams its qs=0
                # region kt-incrementally through the group; regions qs=1-3
                # drain region-major after the group's last kt, two regions
                # per slot.
                pmap = {}            # (b,qc) -> {kt: P}
                stream_kts = {}      # (b,qc) -> kts streamed (qs=0 regions)
                drain_q = []         # groups past kt=15, awaiting qs 1-3
                drained = {}         # (b,qc) -> drained region count (0..6)
                otiles = {}          # (b,qc) -> [o_h0, o_h1]
                finalized = []       # groups finalized, in order
                stream_q = []        # groups awaiting/undergoing streaming

                LASTG = (B - 1, QC - 1)

                def alloc_group(g):
                    otiles[g] = [
                        ps_o.tile([128, QS * DV], fp32, tag=f"og{h}",
                                  name=f"og{h}")
                        for h in range(HL)
                    ]
                    if g == LASTG:
                        # two extra banks from the sm tag: four streaming
                        # regions, so only qs=2,3 drain after the last exp
                        otiles[g] += [
                            ps_sm.tile([128, 512], fp32, tag="sm",
                                       name=f"ogx{h}")
                            for h in range(HL)
                        ]

                def rmap(g, h, qs):
                    # region (h, qs) -> (tile, col0).  Last group: qs 0,2 in
                    # og_h (cols 0/65), qs 1,3 in the sm extras.
                    if g == LASTG:
                        if qs % 2 == 0:
                            return otiles[g][h], (qs // 2) * DV
                        return otiles[g][2 + h], (qs // 2) * DV
                    return otiles[g][h], qs * DV

                def group_done(g, skip_norms=False):
                    finalize(g[0], g[1], otiles[g],
                             via_pe=(g == (B - 1, QC - 1)),
                             skip_norms=skip_norms)
                    finalized.append(g)
                    otiles.pop(g)

                def flush(now_i, all_=False):
                    # 1) drain the head of drain_q: regions (qs>=1), two per
                    # call (region = 16 matmuls of F=65 ~ 0.43us each)
                    nreg = 1000 if all_ else 2
                    while drain_q and nreg > 0:
                        g = drain_q[0]
                        last = all_ and g == LASTG
                        qs0 = 2 if g == LASTG else 1
                        ndrain = HL * (QS - qs0)
                        d = drained.get(g, 0)
                        if last and d == 0:
                            # tail: the streamed regions are closed - norm
                            # them while PE drains the rest
                            for qs in range(qs0):
                                for h in range(HL):
                                    t_, c_ = rmap(g, h, qs)
                                    region_norm(g[0], g[1], t_, c_, h, qs,
                                                on_act=(h == 1))
                        take = min(ndrain - d, nreg)
                        for idx in range(d, d + take):
                            qs = qs0 + idx // HL
                            h = idx % HL
                            t_, c_ = rmap(g, h, qs)
                            for kt in range(KT):
                                vmm_one(t_, c_, pmap[g][kt], g[0],
                                        h, qs, kt)
                            if last:
                                region_norm(g[0], g[1], t_, c_, h, qs,
                                            on_act=(idx % 2 == 1))
                        drained[g] = d + take
                        nreg -= take
                        if drained[g] == ndrain:
                            drain_q.pop(0)
                            pmap.pop(g)
                            group_done(g, skip_norms=last)
                    # 2) stream qs=0 regions (head of stream_q) as kts and
                    # vaug rows become available; a group may only take the
                    # o banks once the previous group has fully vacated them
                    while stream_q:
                        g = stream_q[0]
                        if g not in otiles:
                            if otiles:
                                break    # banks still owned by prior group
                            alloc_group(g)
                        hi = min(max(pmap[g].keys()) + 1 if pmap[g] else 0,
                                 vrows[g[0]])
                        sk = stream_kts.get(g, 0)
                        nstream = 2 if g == LASTG else 1
                        for kt in range(sk, hi):
                            for qs in range(nstream):
                                for h in range(HL):
                                    t_, c_ = rmap(g, h, qs)
                                    vmm_one(t_, c_, pmap[g][kt], g[0],
                                            h, qs, kt)
                        stream_kts[g] = max(sk, hi)
                        if stream_kts[g] == KT:
                            drain_q.append(g)
                            stream_q.pop(0)
                            continue
                        break

                # ---- prologue: first score tile in column halves so the
                # exp stream starts as soon as the first xt piece lands ----
                warmup(52)
                proj_q_qs(0, 0, 0)
                proj_q_qs(0, 0, 1)
                proj_k(0, 0)
                proj_k(0, 1)
                S = ps_s.tile([128, 2 * 512], fp32, tag="S", name="S")
                for h in range(HL):
                    nc.tensor.matmul(
                        S[:, h * 512:h * 512 + 256],
                        qkT[h * D:(h + 1) * D, 0, 1, 0:128],
                        qkT[h * D:(h + 1) * D, 0, 0, 0:256],
                        start=True, stop=True)
                P0 = ppool.tile([128, 2 * 512], bf16, tag="P", name="P0")
                Sv0 = S.rearrange("p (h c) -> p h c", h=HL)
                Pv0 = P0.rearrange("p (h c) -> p h c", h=HL)
                nc.scalar.activation(Pv0[:, :, 0:256], Sv0[:, :, 0:256],
                                     EXP, scale=0.125)
                proj_q_qs(0, 0, 2)
                proj_q_qs(0, 0, 3)
                for h in range(HL):
                    nc.tensor.matmul(
                        S[:, h * 512 + 256:(h + 1) * 512],
                        qkT[h * D:(h + 1) * D, 0, 1, 0:128],
                        qkT[h * D:(h + 1) * D, 0, 0, 256:512],
                        start=True, stop=True)

                P_last = None
                for i, (b, qc, kt) in enumerate(seq):
                    if i == 0:
                        # first half emitted in the prologue on tile P0
                        P = P0
                        nc.scalar.activation(Pv0[:, :, 256:512],
                                             Sv0[:, :, 256:512],
                                             EXP, scale=0.125)
                    elif i == len(seq) - 1:
                        P = ppool.tile([128, 2 * 512], bf16, tag="P")
                        Svl = S.rearrange("p (h c) -> p h c", h=HL)
                        Pvl = P.rearrange("p (h c) -> p h c", h=HL)
                        nc.scalar.activation(Pvl[:, :, 0:256],
                                             Svl[:, :, 0:256],
                                             EXP, scale=0.125)
                        nc.scalar.activation(Pvl[:, :, 256:512],
                                             Svl[:, :, 256:512],
                                             EXP, scale=0.125)
                    else:
                        P = ppool.tile([128, 2 * 512], bf16, tag="P")
                        nc.scalar.activation(P[:], S[:], EXP, scale=0.125)
                    P_last = P
                    if i + 1 < len(seq):
                        S = sc(*seq[i + 1])
                    for fn in SCHED.get((b, qc, kt), []):
                        fn()
                    for (vb, r) in VROWS.get((b, qc, kt), []):
                        vrows[vb] = max(vrows[vb], r + 1)
                    if kt == 0:
                        stream_q.append((b, qc))
                        pmap[(b, qc)] = {}
                    pmap[(b, qc)][kt] = P
                    flush(i)

                # ---- tail ----
                while drain_q or stream_q:
                    flush(len(seq), all_=True)
                assert len(finalized) == B * QC, finalized
                # keep the PE p-state hot across the norm/transpose/dup
                # wait; chained on the last P tile so the scheduler cannot
                # hoist these out of the tail window
                warmup(0, src=P_last)
                for h in range(HL):
                    for n5 in range(2):
                        for s in range(2):
                            outproj_piece(1, h, n5, s, 2,
                                          tag=("S" if (n5 + s) % 2 else "sm"))

            _emit_all()

    nc.compile()
    return nc


def _get_runner():
    """Build + compile once; return a callable(in_maps) -> list of out dicts."""
    global _RUNNER
    if _RUNNER is not None:
        return _RUNNER

    import jax
    import concourse.mybir as mybir
    from concourse import bass2jax
    from jax.experimental.shard_map import shard_map
    from jax.sharding import Mesh, PartitionSpec

    nc = _build_nc()
    bass2jax.install_neuronx_cc_hook()

    partition_name = (nc.partition_id_tensor.name
                      if nc.partition_id_tensor else None)
    in_names, out_names, out_avals = [], [], []
    for alloc in nc.m.functions[0].allocations:
        if not isinstance(alloc, mybir.MemoryLocationSet):
            continue
        name = alloc.memorylocations[0].name
        if alloc.kind == "ExternalInput":
            if name != partition_name:
                in_names.append(name)
        elif alloc.kind == "ExternalOutput":
            out_names.append(name)
            out_avals.append(jax.core.ShapedArray(
                tuple(alloc.tensor_shape), mybir.dt.np(alloc.dtype)))

    n_params, n_outs = len(in_names), len(out_avals)
    all_names = in_names + out_names
    if partition_name is not None:
        all_names = all_names + [partition_name]

    def _body(*args):
        operands = list(args)
        if partition_name is not None:
            operands.append(bass2jax.partition_id_tensor())
        outs = bass2jax._bass_exec_p.bind(
            *operands,
            out_avals=tuple(out_avals),
            in_names=tuple(all_names),
            out_names=tuple(out_names),
            lowering_input_output_aliases=(),
            sim_require_finite=True,
            sim_require_nnan=True,
            nc=nc,
        )
        return tuple(outs)

    devices = jax.devices()[:N_CORES]
    mesh = Mesh(np.asarray(devices), ("core",))
    in_specs = (PartitionSpec("core"),) * (n_params + n_outs)
    out_specs = (PartitionSpec("core"),) * n_outs
    donate = tuple(range(n_params, n_params + n_outs))
    sharded = jax.jit(
        shard_map(_body, mesh=mesh, in_specs=in_specs, out_specs=out_specs,
                  check_rep=False),
        donate_argnums=donate, keep_unused=True,
    )

    def run(in_maps):
        concat_in = [
            np.concatenate([np.asarray(in_maps[c][nm]) for c in range(N_CORES)],
                           axis=0)
            for nm in in_names
        ]
        concat_zeros = [
            np.zeros((N_CORES * a.shape[0], *a.shape[1:]), a.dtype)
            for a in out_avals
        ]
        out_arrs = sharded(*concat_in, *concat_zeros)
        return [
            {nm: np.asarray(out_arrs[i]).reshape(N_CORES, *out_avals[i].shape)[c]
             for i, nm in enumerate(out_names)}
            for c in range(N_CORES)
        ]

    _RUNNER = run
    run._bench_parts = (sharded, mesh, in_names, out_names, out_avals,
                        n_params, _body)
    return run


def _make_bench(in_maps):
    """Device-resident benchmark closure: returns fn() that runs one
    execution with all inputs already on device (no donation)."""
    import jax
    from jax.experimental.shard_map import shard_map
    from jax.sharding import NamedSharding, PartitionSpec

    run = _get_runner()
    sharded, mesh, in_names, out_names, out_avals, n_params, _body = \
        run._bench_parts
    sh = NamedSharding(mesh, PartitionSpec("core"))

    nodonate = jax.jit(
        shard_map(_body, mesh=mesh,
                  in_specs=(PartitionSpec("core"),) * (n_params + len(out_avals)),
                  out_specs=(PartitionSpec("core"),) * len(out_avals),
                  check_rep=False),
        keep_unused=True,
    )
    concat_in = [
        np.concatenate([np.asarray(in_maps[c][nm]) for c in range(N_CORES)], axis=0)
        for nm in in_names
    ]
    concat_zeros = [
        np.zeros((N_CORES * a.shape[0], *a.shape[1:]), a.dtype) for a in out_avals
    ]
    dev_args = [jax.device_put(a, sh) for a in concat_in + concat_zeros]
    for a in dev_args:
        a.block_until_ready()

    def bench_once():
        outs = nodonate(*dev_args)
        for o in outs:
            o.block_until_ready()
        return outs

    bench_once.nodonate = nodonate
    bench_once.dev_args = dev_args
    return bench_once


def _prep_in_maps(x, W_qkv, b_qkv, W_out, b_out):
    bf = ml_dtypes.bfloat16
    xt = np.ascontiguousarray(
        x.reshape(B * T, E).T).astype(bf)                      # [E, B*T]
    wout = np.ascontiguousarray(
        W_out.reshape(8, 128, E).transpose(1, 0, 2).reshape(128, 8 * E)).astype(bf)
    bout = np.ascontiguousarray(
        np.broadcast_to(b_out.astype(np.float32)[None, :], (128, E)))

    in_maps = []
    for c in range(N_CORES):
        hs = [HL * c + i for i in range(HL)]
        qcols = np.concatenate(
            [W_qkv[:, 0 * E + h * D:0 * E + (h + 1) * D] for h in hs], axis=1)
        kcols = np.concatenate(
            [W_qkv[:, 1 * E + h * D:1 * E + (h + 1) * D] for h in hs], axis=1)
        wqk = np.ascontiguousarray(
            np.concatenate([qcols, kcols], axis=1)).astype(bf)  # [E, 256]
        zcol = np.zeros((E, 1), np.float32)
        wv = np.ascontiguousarray(np.concatenate(
            [arr for h in hs
             for arr in (W_qkv[:, 2 * E + h * D:2 * E + (h + 1) * D], zcol)],
            axis=1)).astype(bf)                                 # [E, 130]
        bq = np.concatenate([b_qkv[0 * E + h * D:0 * E + (h + 1) * D] for h in hs])
        bk = np.concatenate([b_qkv[1 * E + h * D:1 * E + (h + 1) * D] for h in hs])
        bqk = np.ascontiguousarray(
            np.stack([bq, bk], axis=1)).astype(np.float32)      # [128, 2]
        one = np.ones(1, np.float32)
        bvv = np.concatenate(
            [a for h in hs
             for a in (b_qkv[2 * E + h * D:2 * E + (h + 1) * D], one)])
        bv = np.ascontiguousarray(
            np.broadcast_to(bvv.astype(np.float32)[None, :], (128, 2 * DV)))
        in_maps.append({
            "xt": xt, "wqk": wqk, "wv": wv, "wout": wout,
            "bqk": bqk, "bv": bv, "bout": bout,
        })
    return in_maps


def kernel(x, W_qkv, b_qkv, W_out, b_out):
    x = np.asarray(x, dtype=np.float32)
    W_qkv = np.asarray(W_qkv, dtype=np.float32)
    b_qkv = np.asarray(b_qkv, dtype=np.float32)
    W_out = np.asarray(W_out, dtype=np.float32)
    b_out = np.asarray(b_out, dtype=np.float32)

    run = _get_runner()
    in_maps = _prep_in_maps(x, W_qkv, b_qkv, W_out, b_out)
    results = run(in_maps)

    out = np.empty((B, T, E), np.float32)
    for c in range(N_CORES):
        y = results[c]["y"]          # [B, HL, 128, E]
        for hl in range(HL):
            hg = HL * c + hl
            out[:, hg * 128:(hg + 1) * 128, :] = y[:, hl]
    return out


# revision 7
# speedup vs baseline: 1.2942x; 1.0114x over previous
"""Multi-head self-attention TRN2 kernel (B=2, T=2048, E=1024, H=16, D=64).

Sharding: tensor-parallel over heads - each of the 8 cores owns 2 heads.
Because the reference reshapes (B,H,T,D)->(B,T,E) with NO transpose, each
head's attention output maps to 128 complete contiguous rows of the
out_proj input, so the whole computation is embarrassingly parallel
across heads (no collectives).

Per-core pipeline (all matmuls bf16, accumulation fp32):
  1. qT/kT projections in [d, T] layout (heads stacked on partitions
     0-63 / 64-127); v in natural [T, d] layout augmented with a ones
     column (softmax denominator for free).
  2. scores^T tiles [kj=128, qi=512x2h] -> exp on ScalarE -> P^T bf16.
  3. attn@v in the cheap orientation: out o[q=128 partitions, d+1=65
     free] accumulated over kj tiles - PE cost 65 free/step instead of
     512 (the cost model charges output free size per accumulation
     step, so small-free x many-partitions wins 8x per instruction).
  4. normalize by per-partition (per-query) reciprocal on VectorE,
     downcast to bf16.
  5. o -> o^T via DMA xbar transposes (off the PE critical path), then
     shifted partition-duplicate copies build the [128=(d,j-parity), T]
     lhsT layout that performs the reference's "faithful reshape" for
     free in out_proj.
  6. out_proj: 8 accumulating matmuls per 512-column chunk.
"""

import numpy as np
import ml_dtypes

B, T, E, H, D = 2, 2048, 1024, 16, 64
N_CORES = 8
HL = H // N_CORES          # heads per core = 2
KP = E // 128              # 8 contraction partition-tiles
KT = T // 128              # 16 kj tiles
QC = T // 512              # 4 qi chunks of 512
QS = 4                     # q subtiles of 128 per chunk
DV = D + 1                 # v width incl. denominator ones column

_RUNNER = None


def _build_nc():
    import concourse.bacc as bacc
    import concourse.tile as tile
    import concourse.bass as bass
    import concourse.mybir as mybir

    from concourse import masks

    fp32 = mybir.dt.float32
    bf16 = mybir.dt.bfloat16
    ADD = mybir.AluOpType.add
    MULT = mybir.AluOpType.mult
    EXP = mybir.ActivationFunctionType.Exp
    COPY = mybir.ActivationFunctionType.Copy

    nc = bacc.Bacc("TRN2", target_bir_lowering=False, debug=False,
                   enable_asserts=True, num_devices=N_CORES)

    xt_d = nc.dram_tensor("xt", [E, B * T], bf16, kind="ExternalInput").ap()
    wqk_d = nc.dram_tensor("wqk", [E, 4 * D], bf16, kind="ExternalInput").ap()
    wv_d = nc.dram_tensor("wv", [E, 2 * DV], bf16, kind="ExternalInput").ap()
    wout_d = nc.dram_tensor("wout", [128, 8 * E], bf16, kind="ExternalInput").ap()
    bqk_d = nc.dram_tensor("bqk", [128, 2], fp32, kind="ExternalInput").ap()
    bv_d = nc.dram_tensor("bv", [128, 2 * DV], fp32, kind="ExternalInput").ap()
    bout_d = nc.dram_tensor("bout", [128, E], fp32, kind="ExternalInput").ap()
    y_d = nc.dram_tensor("y", [B, HL, 128, E], fp32, kind="ExternalOutput").ap()

    with tile.TileContext(nc) as tc:
        with (
            tc.tile_pool(name="const", bufs=1) as cpool,
            tc.tile_pool(name="ppool", bufs=26) as ppool,
            tc.tile_pool(name="npool", bufs=4) as npool,
            tc.tile_pool(name="ypool", bufs=4) as ypool,
            tc.tile_pool(name="ps_s", bufs=2, space=bass.MemorySpace.PSUM) as ps_s,
            tc.tile_pool(name="ps_o", bufs=1, space=bass.MemorySpace.PSUM) as ps_o,
            tc.tile_pool(name="ps_sm", bufs=2, space=bass.MemorySpace.PSUM) as ps_sm,
        ):
            # ---- constants / persistent tiles ----
            xt_sb = cpool.tile([128, KP, B * T], bf16, tag="xt")
            wqk_sb = cpool.tile([128, KP, 4 * D], bf16, tag="wqk")
            wv_sb = cpool.tile([128, KP, 2 * DV], bf16, tag="wv")
            wout_sb = cpool.tile([128, 8, E], bf16, tag="wout")
            bqk_sb = cpool.tile([128, 2], fp32, tag="bqk")
            bv_sb = cpool.tile([128, 2 * DV], fp32, tag="bv")
            bout_sb = cpool.tile([128, E], fp32, tag="bout")
            qkT = cpool.tile([128, B, 2, T], bf16, tag="qkT")
            vaug = cpool.tile([128, B, KT, 2 * DV], bf16, tag="vaug")
            # o natural layout, normalized bf16, 64 pad cols per head slot
            # (the pad transposes into oTd rows 64-127, which the shifted
            # dup copy then overwrites)
            # per-batch tensors (separate tags) so cross-batch false
            # dependencies cannot arise from coarse subtile tracking
            o_sb = [cpool.tile([128, KT, 192], bf16, tag=f"o_sb{bb}",
                               name=f"o_sb{bb}") for bb in range(B)]
            # o^T dup layout per head: rows 0-63 straight (written directly
            # by the xbar transpose), 64-127 shifted by one q (dup DMA)
            oTd = [cpool.tile([128, HL, T], bf16, tag=f"oTd{bb}",
                              name=f"oTd{bb}") for bb in range(B)]
            wub = cpool.tile([1, 128], bf16, tag="wub")
            ones1 = cpool.tile([1, 128], bf16, tag="ones1")
            bout_bf = cpool.tile([1, E], bf16, tag="bout_bf")
            ident = cpool.tile([128, 128], bf16, tag="ident")

            # warmup source (no DMA dependency); zero o_sb's pad lanes once
            nc.gpsimd.memset(wub[:], 0.0)
            nc.gpsimd.memset(ones1[:], 1.0)
            masks.make_identity(nc, ident[:])

            # ---- input DMAs: spread across the three HWDGE-ish rings so
            # queue overheads pipeline.  Transfers serialize on the shared
            # DMA engines; order = priority.  The first score tile needs
            # k cols + q[0:512], so those pieces go first, smallest first.
            wqk_r = wqk_d.rearrange("(a p) n -> p a n", p=128)
            xt_r = xt_d.rearrange("(a p) n -> p a n", p=128)
            nc.sync.dma_start(wqk_sb[:], wqk_r[:])
            # first 512 T-columns on the ACT ring (idle until exps start);
            # bigger slices keep the 1KB contiguous runs (no small-elem
            # DMA penalty)
            nc.scalar.dma_start(xt_sb[:, :, 0:256], xt_r[:, :, 0:256])
            nc.scalar.dma_start(xt_sb[:, :, 256:512], xt_r[:, :, 256:512])
            nc.scalar.dma_start(xt_sb[:, :, 512:1024], xt_r[:, :, 512:1024])
            nc.sync.dma_start(bqk_sb[:], bqk_d[:])
            nc.sync.dma_start(bv_sb[:], bv_d[:])
            # o_sb pad lanes zeroed on the gpsimd engine: doubles as a
            # delay so the Pool ring's chunk DMAs queue at the shared DMA
            # engines AFTER the ACT ring's critical first pieces
            for bb in range(B):
                nc.gpsimd.memset(o_sb[bb][:, :, 128:192], 0.0)
            # rest of x on the gpsimd SWDGE ring (engine otherwise idle);
            # wv mid-way (not needed until ~slot 11)
            for cc in range(2, B * T // 512):
                if cc == 4:
                    nc.gpsimd.dma_start(
                        wv_sb[:], wv_d.rearrange("(a p) n -> p a n", p=128))
                nc.gpsimd.dma_start(xt_sb[:, :, cc * 512:(cc + 1) * 512],
                                    xt_r[:, :, cc * 512:(cc + 1) * 512])

            # ---- PE warmup: keep the clock ramping from t~0 so the first
            # real matmuls run at full p-state.  src defaults to wub (no
            # deps -> the scheduler hoists them to kernel start); pass a
            # late-written AP to pin warmups into a late window instead.
            def warmup(n, src=None):
                s = wub if src is None else src
                for _ in range(n):
                    wt = ps_sm.tile([128, 128], fp32, tag="sm", name="wt")
                    nc.tensor.matmul(wt[:], s[0:1, 0:128], s[0:1, 0:128],
                                     start=True, stop=True)

            # ---- projection pieces ----
            def proj_q(b, n):
                # q^T chunk n: [128=(h0|h1)*d, 512]
                ps = ps_sm.tile([128, 512], fp32, tag="sm", name="psq")
                for k in range(KP):
                    nc.tensor.matmul(
                        ps[:],
                        wqk_sb[:, k, 0:128],
                        xt_sb[:, k, b * T + n * 512: b * T + (n + 1) * 512],
                        start=(k == 0), stop=(k == KP - 1),
                    )
                nc.vector.tensor_scalar(
                    qkT[:, b, 0, n * 512:(n + 1) * 512], ps[:],
                    bqk_sb[:, 0:1], None, op0=ADD,
                )

            def proj_q_half(b, n, hf):
                ps = ps_sm.tile([128, 512], fp32, tag="sm", name="psq")
                c0 = n * 512 + hf * 256
                for k in range(KP):
                    nc.tensor.matmul(
                        ps[:, 0:256],
                        wqk_sb[:, k, 0:128],
                        xt_sb[:, k, b * T + c0: b * T + c0 + 256],
                        start=(k == 0), stop=(k == KP - 1),
                    )
                nc.vector.tensor_scalar(
                    qkT[:, b, 0, c0:c0 + 256], ps[:, 0:256],
                    bqk_sb[:, 0:1], None, op0=ADD,
                )

            def proj_q_qs(b, n, qs):
                ps = ps_sm.tile([128, 512], fp32, tag="sm", name="psq")
                c0 = n * 512 + qs * 128
                for k in range(KP):
                    nc.tensor.matmul(
                        ps[:, 0:128],
                        wqk_sb[:, k, 0:128],
                        xt_sb[:, k, b * T + c0: b * T + c0 + 128],
                        start=(k == 0), stop=(k == KP - 1),
                    )
                nc.vector.tensor_scalar(
                    qkT[:, b, 0, c0:c0 + 128], ps[:, 0:128],
                    bqk_sb[:, 0:1], None, op0=ADD,
                )

            def proj_k(b, kt):
                # k^T tile kt: [128, 128]
                ps = ps_sm.tile([128, 512], fp32, tag="sm", name="psk")
                for k in range(KP):
                    nc.tensor.matmul(
                        ps[:, 0:128],
                        wqk_sb[:, k, 128:256],
                        xt_sb[:, k, b * T + kt * 128: b * T + (kt + 1) * 128],
                        start=(k == 0), stop=(k == KP - 1),
                    )
                nc.vector.tensor_scalar(
                    qkT[:, b, 1, kt * 128:(kt + 1) * 128], ps[:, 0:128],
                    bqk_sb[:, 1:2], None, op0=ADD,
                )

            def proj_v(b, r):
                # v natural [128, 2*DV] row tile r; W_v has zero columns at
                # the two "ones" slots and bv carries 1.0 there
                vp = ps_sm.tile([128, 512], fp32, tag="sm", name="vp")
                for k in range(KP):
                    nc.tensor.matmul(
                        vp[:, 0:2 * DV],
                        xt_sb[:, k, b * T + r * 128: b * T + (r + 1) * 128],
                        wv_sb[:, k, :],
                        start=(k == 0), stop=(k == KP - 1),
                    )
                nc.vector.tensor_tensor(
                    vaug[:, b, r, :], vp[:, 0:2 * DV], bv_sb[:], op=ADD,
                )

            def sc(b, qc, kt):
                # scores^T [kj=128, qi=512] per head, heads side by side
                S = ps_s.tile([128, 2 * 512], fp32, tag="S", name="S")
                for h in range(HL):
                    nc.tensor.matmul(
                        S[:, h * 512:(h + 1) * 512],
                        qkT[h * D:(h + 1) * D, b, 1, kt * 128:(kt + 1) * 128],
                        qkT[h * D:(h + 1) * D, b, 0, qc * 512:(qc + 1) * 512],
                        start=True, stop=True,
                    )
                return S

            def vmm_one(tile, c0, P, b, h, qs, kt):
                # o[q-subtile, d+1] += P^T[kj, q-sub]^T @ vaug[kj, d+1]
                nc.tensor.matmul(
                    tile[:, c0:c0 + DV],
                    P[:, h * 512 + qs * 128: h * 512 + (qs + 1) * 128],
                    vaug[:, b, kt, h * DV:(h + 1) * DV],
                    start=(kt == 0), stop=(kt == KT - 1),
                )

            def region_norm(b, qc, tile, c0, h, qs, on_act=False):
                # one region's reciprocal + normalize (tail interleaving)
                rdt = npool.tile([128, 8], fp32, tag="rd", name="rdt")
                nc.vector.reciprocal(
                    rdt[:, h * 4 + qs: h * 4 + qs + 1],
                    tile[:, c0 + D: c0 + DV])
                qt = qc * QS + qs
                if on_act:
                    nc.scalar.activation(
                        o_sb[b][:, qt, h * 64:(h + 1) * 64],
                        tile[:, c0: c0 + D], COPY,
                        scale=rdt[:, h * 4 + qs: h * 4 + qs + 1])
                else:
                    nc.vector.tensor_scalar(
                        o_sb[b][:, qt, h * 64:(h + 1) * 64],
                        tile[:, c0: c0 + D],
                        rdt[:, h * 4 + qs: h * 4 + qs + 1],
                        None, op0=MULT,
                    )

            def finalize(b, qc, ot, via_pe=False, skip_norms=False):
                # All norms (both heads) must precede the transposes: each
                # transpose reads a 128-col window spanning both head
                # slots.  via_pe: for the very last group the DMA
                # transpose+dup chain (~650ns queue overhead per hop) sits
                # exposed on the critical tail, so route it through PE
                # identity-matmuls + DVE/ACT copies instead (idle there).
                rd = npool.tile([128, 8], fp32, tag="rd", name="rd")
                if not skip_norms:
                    for h in range(HL):
                        nc.vector.reciprocal(
                            rd[:, h * 4:(h + 1) * 4].rearrange(
                                "p (a c) -> p a c", c=1),
                            ot[h].rearrange(
                                "p (q c) -> p q c", c=DV)[:, :, D:D + 1],
                        )
                    for qs in range(QS):
                        qt = qc * QS + qs
                        for h in range(HL):
                            nc.vector.tensor_scalar(
                                o_sb[b][:, qt, h * 64:(h + 1) * 64],
                                ot[h][:, qs * DV: qs * DV + D],
                                rd[:, h * 4 + qs: h * 4 + qs + 1],
                                None, op0=MULT,
                            )
                for qs in range(QS):
                    qt = qc * QS + qs
                    for h in range(HL):
                        # o^T; out rows 64-127 get the neighbour head slot /
                        # pad lanes, overwritten by the shifted dup below
                        if via_pe:
                            # bf16 [128,512] = same slot bytes as the sm tag
                            tp = ps_sm.tile([128, 512], bf16, tag="sm",
                                            name="tp")
                            nc.tensor.transpose(
                                tp[:, 0:128],
                                o_sb[b][:, qt, h * 64:h * 64 + 128],
                                ident[:])
                            nc.vector.tensor_copy(
                                oTd[b][0:128, h, qt * 128:(qt + 1) * 128],
                                tp[:, 0:128])
                        else:
                            nc.sync.dma_start_transpose(
                                oTd[b][0:128, h, qt * 128:(qt + 1) * 128],
                                o_sb[b][:, qt, h * 64:h * 64 + 128],
                            )
                for h in range(HL):
                    # shifted dup: via_pe does qc-1 + qc chunks; the DMA
                    # route defers to one whole-row dup after the last qc
                    # (fewer DMA producers -> fewer merged waits downstream)
                    chunks = []
                    if via_pe:
                        # cols 0..1023 were dup'd early (qc==2 below); the
                        # final group owns the last two 512-col pieces
                        chunks = [(1024, 512), (1536, 511)]
                    elif b == B - 1 and qc == QC - 2:
                        # early dup for the tail batch: sources complete
                        # through this group's transposes
                        chunks = [(0, 512), (512, 512)]
                    elif qc == QC - 1:
                        chunks.append((0, T - 1))
                    for c0, cw in chunks:
                        if via_pe:
                            dp = ps_sm.tile([128, 512], fp32, tag="sm",
                                            name="dp")
                            nc.tensor.matmul(
                                dp[0:64, 0:cw], ident[0:64, 0:64],
                                oTd[b][0:64, h, c0 + 1:c0 + cw + 1],
                                start=True, stop=True)
                            if h == 0:
                                nc.scalar.activation(
                                    oTd[b][64:128, h, c0:c0 + cw],
                                    dp[0:64, 0:cw], COPY)
                            else:
                                nc.vector.tensor_copy(
                                    oTd[b][64:128, h, c0:c0 + cw],
                                    dp[0:64, 0:cw])
                        else:
                            nc.sync.dma_start(
                                oTd[b][64:128, h, c0:c0 + cw],
                                oTd[b][0:64, h, c0 + 1:c0 + cw + 1])

            def outproj_piece(b, h, n5, s, split, tag="sm"):
                # one 1/split column chunk of out_proj for (b, h, n5); bias
                # folded in as a ones-row matmul so y DMAs straight out of
                # PSUM (no DVE hop on the critical tail)
                of2 = oTd[b][:, h, :].rearrange("p (t j) -> p j t", j=16)
                w = 512 // split
                c0 = n5 * 512 + s * w
                yp = ps_sm.tile([128, 512], fp32, tag=tag, name="yp") \
                    if tag == "sm" else \
                    ps_s.tile([128, 512], fp32, tag=tag, name="yp")
                for jj in range(8):
                    nc.tensor.matmul(
                        yp[:, 0:w],
                        of2[:, 2 * jj, :],
                        wout_sb[:, jj, c0:c0 + w],
                        start=(jj == 0), stop=False,
                    )
                nc.tensor.matmul(
                    yp[:, 0:w], ones1[0:1, :], bout_bf[0:1, c0:c0 + w],
                    start=False, stop=True,
                )
                # PSUM cannot feed a DMA directly; stage through SBUF on
                # DVE mid-stream / ACT at the tail (idle there)
                ys = ypool.tile([128, 512], fp32, tag="ys", name="ys")
                if b == 0 or (h + n5 + s) % 2:
                    nc.vector.tensor_copy(ys[:, 0:w], yp[:, 0:w])
                else:
                    nc.scalar.activation(ys[:, 0:w], yp[:, 0:w], COPY)
                # mid-stream y-writes ride the gpsimd SWDGE ring: keeps
                # their deep dependency chains off the SP ring's semaphore
                # window (an SP sem recycle once stalled the exp stream
                # 14us).  Tail y-writes (b=1) alternate SP/gpsimd.
                eng = nc.sync if (h + n5 + s) % 2 else nc.gpsimd
                eng.dma_start(y_d[b, h, :, c0:c0 + w], ys[:, 0:w])

            # ---- filler schedule: all pieces ~0.43us (8 accumulating
            # matmuls of F<=130) so the score stream is never delayed by a
            # long chain sitting ahead of it in the PE queue.  Greedy
            # deadline placement, one piece per slot unless a deadline
            # forces more.
            SCHED = {}
            VROWS = {}

            def at_slot(slot, fn, vrow=None):
                key = (slot // 64, (slot % 64) // 16, slot % 16)
                SCHED.setdefault(key, []).append(fn)
                if vrow is not None:
                    VROWS.setdefault(key, []).append(vrow)

            def chunk_slot(c):
                # xt chunk c lands on the gpsimd ring ~2.9us apart; convert
                # to the exp-slot index from which a piece may read it
                if c == 0:
                    return -100
                return int((7814 + 2913 * c + 900 - 11090) / 1038) + 1

            pieces = []  # [deadline, earliest, fn, vrow]
            for b in range(B):
                for kt in range(KT):
                    if b == 0 and kt < 2:
                        continue  # prologue
                    c = (b * T + kt * 128) // 512
                    pieces.append(
                        [64 * b + kt - 2, chunk_slot(c),
                         (lambda bb, kk: lambda: proj_k(bb, kk))(b, kt),
                         None])
                for r in range(KT):
                    c = (b * T + r * 128) // 512
                    pieces.append(
                        [(14 if b == 0 else 50) + r // 2, chunk_slot(c),
                         (lambda bb, rr: lambda: proj_v(bb, rr))(b, r),
                         (b, r)])
                for n in range(QC):
                    if b == 0 and n == 0:
                        continue  # prologue
                    for qs in range(QS):
                        c = (b * T + n * 512 + qs * 128) // 512
                        pieces.append(
                            [64 * b + 16 * n - 3, chunk_slot(c),
                             (lambda bb, nn, qq:
                              lambda: proj_q_qs(bb, nn, qq))(b, n, qs),
                             None])
            for h in range(HL):
                for n5 in range(2):
                    for s in range(2):
                        pieces.append(
                            [100 + 3 * (4 * h + 2 * n5 + s), 95,
                             (lambda hh, nn, ss:
                              lambda: outproj_piece(0, hh, nn, ss, 2))(
                                  h, n5, s),
                             None])
            todo = sorted(pieces, key=lambda p: (p[0], p[1]))
            for slot in range(B * QC * KT):
                # keep group-boundary slots free: the finalize/norm chain
                # must reach the DVE queue head unimpeded.  Deadline-due
                # pieces are placed unconditionally (a qkT piece placed past
                # its deadline would be read stale by the score lookahead).
                boundary = slot % KT in (15, 0)
                n = 1 if boundary else 0
                for p in todo[:]:
                    if p[1] > slot:
                        continue
                    if p[0] <= slot:
                        at_slot(slot, p[2], p[3])
                        todo.remove(p)
                        n += 1
                    elif n == 0:
                        at_slot(slot, p[2], p[3])
                        todo.remove(p)
                        n += 1
                    else:
                        break
            assert not todo, [p[:2] for p in todo]

            # wout/bout on the gpsimd ring once startup DMA traffic is done
            at_slot(33, lambda: nc.gpsimd.dma_start(wout_sb[:], wout_d[:]))
            at_slot(35, lambda: nc.gpsimd.dma_start(bout_sb[:], bout_d[:]))
            at_slot(37, lambda: nc.vector.tensor_copy(
                bout_bf[0:1, :], bout_sb[0:1, :]))

            # ---- unified emission ----
            def _emit_all():
                seq = [(b, qc, kt) for b in range(B) for qc in range(QC)
                       for kt in range(KT)]
                vrows = {0: 0, 1: 0}
                # PSUM accumulation: one pending group per 2KB zero region
                # (bank).  Each o bank (og0=h0, og1=h1) streams its qs=0
                # region kt-incrementally through the group; regions qs=1-3
                # drain region-major after the group's last kt, two regions
                # per slot.
                pmap = {}            # (b,qc) -> {kt: P}
                stream_kts = {}      # (b,qc) -> kts streamed (qs=0 regions)
                drain_q = []         # groups past kt=15, awaiting qs 1-3
                drained = {}         # (b,qc) -> drained region count (0..6)
                otiles = {}          # (b,qc) -> [o_h0, o_h1]
                finalized = []       # groups finalized, in order
                stream_q = []        # groups awaiting/undergoing streaming

                LASTG = (B - 1, QC - 1)

                def alloc_group(g):
                    otiles[g] = [
                        ps_o.tile([128, QS * DV], fp32, tag=f"og{h}",
                                  name=f"og{h}")
                        for h in range(HL)
                    ]
                    if g == LASTG:
                        # two extra banks from the sm tag: four streaming
                        # regions, so only qs=2,3 drain after the last exp
                        otiles[g] += [
                            ps_sm.tile([128, 512], fp32, tag="sm",
                                       name=f"ogx{h}")
                            for h in range(HL)
                        ]

                def rmap(g, h, qs):
                    # region (h, qs) -> (tile, col0).  Last group: qs 0,2 in
                    # og_h (cols 0/65), qs 1,3 in the sm extras.
                    if g == LASTG:
                        if qs % 2 == 0:
                            return otiles[g][h], (qs // 2) * DV
                        return otiles[g][2 + h], (qs // 2) * DV
                    return otiles[g][h], qs * DV

                def group_done(g, skip_norms=False):
                    finalize(g[0], g[1], otiles[g],
                             via_pe=(g == (B - 1, QC - 1)),
                             skip_norms=skip_norms)
                    finalized.append(g)
                    otiles.pop(g)

                def flush(now_i, all_=False):
                    # 1) drain the head of drain_q: regions (qs>=1), two per
                    # call (region = 16 matmuls of F=65 ~ 0.43us each)
                    nreg = 1000 if all_ else 2
                    while drain_q and nreg > 0:
                        g = drain_q[0]
                        last = all_ and g == LASTG
                        qs0 = 2 if g == LASTG else 1
                        ndrain = HL * (QS - qs0)
                        d = drained.get(g, 0)
                        if last and d == 0:
                            # tail: the streamed regions are closed - norm
                            # them while PE drains the rest
                            for qs in range(qs0):
                                for h in range(HL):
                                    t_, c_ = rmap(g, h, qs)
                                    region_norm(g[0], g[1], t_, c_, h, qs,
                                                on_act=(h == 1))
                        take = min(ndrain - d, nreg)
                        for idx in range(d, d + take):
                            qs = qs0 + idx // HL
                            h = idx % HL
                            t_, c_ = rmap(g, h, qs)
                            for kt in range(KT):
                                vmm_one(t_, c_, pmap[g][kt], g[0],
                                        h, qs, kt)
                            if last:
                                region_norm(g[0], g[1], t_, c_, h, qs,
                                            on_act=(idx % 2 == 1))
                        drained[g] = d + take
                        nreg -= take
                        if drained[g] == ndrain:
                            drain_q.pop(0)
                            pmap.pop(g)
                            group_done(g, skip_norms=last)
                    # 2) stream qs=0 regions (head of stream_q) as kts and
                    # vaug rows become available; a group may only take the
                    # o banks once the previous group has fully vacated them
                    while stream_q:
                        g = stream_q[0]
                        if g not in otiles:
                            if otiles:
                                break    # banks still owned by prior group
                            alloc_group(g)
                        hi = min(max(pmap[g].keys()) + 1 if pmap[g] else 0,
                                 vrows[g[0]])
                        sk = stream_kts.get(g, 0)
                        nstream = 2 if g == LASTG else 1
                        for kt in range(sk, hi):
                            for qs in range(nstream):
                                for h in range(HL):
                                    t_, c_ = rmap(g, h, qs)
                                    vmm_one(t_, c_, pmap[g][kt], g[0],
                                            h, qs, kt)
                        stream_kts[g] = max(sk, hi)
                        if stream_kts[g] == KT:
                            drain_q.append(g)
                            stream_q.pop(0)
                            continue
                        break

                # ---- prologue: first score tile in column halves so the
                # exp stream starts as soon as the first xt piece lands ----
                warmup(52)
                proj_q_qs(0, 0, 0)
                proj_q_qs(0, 0, 1)
                proj_k(0, 0)
                proj_k(0, 1)
                S = ps_s.tile([128, 2 * 512], fp32, tag="S", name="S")
                for h in range(HL):
                    nc.tensor.matmul(
                        S[:, h * 512:h * 512 + 256],
                        qkT[h * D:(h + 1) * D, 0, 1, 0:128],
                        qkT[h * D:(h + 1) * D, 0, 0, 0:256],
                        start=True, stop=True)
                P0 = ppool.tile([128, 2 * 512], bf16, tag="P", name="P0")
                Sv0 = S.rearrange("p (h c) -> p h c", h=HL)
                Pv0 = P0.rearrange("p (h c) -> p h c", h=HL)
                nc.scalar.activation(Pv0[:, :, 0:256], Sv0[:, :, 0:256],
                                     EXP, scale=0.125)
                proj_q_qs(0, 0, 2)
                proj_q_qs(0, 0, 3)
                for h in range(HL):
                    nc.tensor.matmul(
                        S[:, h * 512 + 256:(h + 1) * 512],
                        qkT[h * D:(h + 1) * D, 0, 1, 0:128],
                        qkT[h * D:(h + 1) * D, 0, 0, 256:512],
                        start=True, stop=True)

                P_last = None
                for i, (b, qc, kt) in enumerate(seq):
                    if i == 0:
                        # first half emitted in the prologue on tile P0
                        P = P0
                        nc.scalar.activation(Pv0[:, :, 256:512],
                                             Sv0[:, :, 256:512],
                                             EXP, scale=0.125)
                    elif i == len(seq) - 1:
                        P = ppool.tile([128, 2 * 512], bf16, tag="P")
                        Svl = S.rearrange("p (h c) -> p h c", h=HL)
                        Pvl = P.rearrange("p (h c) -> p h c", h=HL)
                        nc.scalar.activation(Pvl[:, :, 0:256],
                                             Svl[:, :, 0:256],
                                             EXP, scale=0.125)
                        nc.scalar.activation(Pvl[:, :, 256:512],
                                             Svl[:, :, 256:512],
                                             EXP, scale=0.125)
                    else:
                        P = ppool.tile([128, 2 * 512], bf16, tag="P")
                        nc.scalar.activation(P[:], S[:], EXP, scale=0.125)
                    P_last = P
                    if i + 1 < len(seq):
                        S = sc(*seq[i + 1])
                    for fn in SCHED.get((b, qc, kt), []):
                        fn()
                    for (vb, r) in VROWS.get((b, qc, kt), []):
                        vrows[vb] = max(vrows[vb], r + 1)
                    if kt == 0:
                        stream_q.append((b, qc))
                        pmap[(b, qc)] = {}
                    pmap[(b, qc)][kt] = P
                    flush(i)

                # ---- tail ----
                while drain_q or stream_q:
                    flush(len(seq), all_=True)
                assert len(finalized) == B * QC, finalized
                # keep the PE p-state hot across the norm/transpose/dup
                # wait; chained on the last P tile so the scheduler cannot
                # hoist these out of the tail window
                warmup(0, src=P_last)
                for h in range(HL):
                    for n5 in range(2):
                        for s in range(2):
                            outproj_piece(1, h, n5, s, 2,
                                          tag=("S" if (n5 + s) % 2 else "sm"))

            _emit_all()

    nc.compile()
    return nc


def _get_runner():
    """Build + compile once; return a callable(in_maps) -> list of out dicts."""
    global _RUNNER
    if _RUNNER is not None:
        return _RUNNER

    import jax
    import concourse.mybir as mybir
    from concourse import bass2jax
    from jax.experimental.shard_map import shard_map
    from jax.sharding import Mesh, PartitionSpec

    nc = _build_nc()
    bass2jax.install_neuronx_cc_hook()

    partition_name = (nc.partition_id_tensor.name
                      if nc.partition_id_tensor else None)
    in_names, out_names, out_avals = [], [], []
    for alloc in nc.m.functions[0].allocations:
        if not isinstance(alloc, mybir.MemoryLocationSet):
            continue
        name = alloc.memorylocations[0].name
        if alloc.kind == "ExternalInput":
            if name != partition_name:
                in_names.append(name)
        elif alloc.kind == "ExternalOutput":
            out_names.append(name)
            out_avals.append(jax.core.ShapedArray(
                tuple(alloc.tensor_shape), mybir.dt.np(alloc.dtype)))

    n_params, n_outs = len(in_names), len(out_avals)
    all_names = in_names + out_names
    if partition_name is not None:
        all_names = all_names + [partition_name]

    def _body(*args):
        operands = list(args)
        if partition_name is not None:
            operands.append(bass2jax.partition_id_tensor())
        outs = bass2jax._bass_exec_p.bind(
            *operands,
            out_avals=tuple(out_avals),
            in_names=tuple(all_names),
            out_names=tuple(out_names),
            lowering_input_output_aliases=(),
            sim_require_finite=True,
            sim_require_nnan=True,
            nc=nc,
        )
        return tuple(outs)

    devices = jax.devices()[:N_CORES]
    mesh = Mesh(np.asarray(devices), ("core",))
    in_specs = (PartitionSpec("core"),) * (n_params + n_outs)
    out_specs = (PartitionSpec("core"),) * n_outs
    donate = tuple(range(n_params, n_params + n_outs))
    sharded = jax.jit(
        shard_map(_body, mesh=mesh, in_specs=in_specs, out_specs=out_specs,
                  check_rep=False),
        donate_argnums=donate, keep_unused=True,
    )

    def run(in_maps):
        concat_in = [
            np.concatenate([np.asarray(in_maps[c][nm]) for c in range(N_CORES)],
                           axis=0)
            for nm in in_names
        ]
        concat_zeros = [
            np.zeros((N_CORES * a.shape[0], *a.shape[1:]), a.dtype)
            for a in out_avals
        ]
        out_arrs = sharded(*concat_in, *concat_zeros)
        return [
            {nm: np.asarray(out_arrs[i]).reshape(N_CORES, *out_avals[i].shape)[c]
             for i, nm in enumerate(out_names)}
            for c in range(N_CORES)
        ]

    _RUNNER = run
    run._bench_parts = (sharded, mesh, in_names, out_names, out_avals,
                        n_params, _body)
    return run


def _make_bench(in_maps):
    """Device-resident benchmark closure: returns fn() that runs one
    execution with all inputs already on device (no donation)."""
    import jax
    from jax.experimental.shard_map import shard_map
    from jax.sharding import NamedSharding, PartitionSpec

    run = _get_runner()
    sharded, mesh, in_names, out_names, out_avals, n_params, _body = \
        run._bench_parts
    sh = NamedSharding(mesh, PartitionSpec("core"))

    nodonate = jax.jit(
        shard_map(_body, mesh=mesh,
                  in_specs=(PartitionSpec("core"),) * (n_params + len(out_avals)),
                  out_specs=(PartitionSpec("core"),) * len(out_avals),
                  check_rep=False),
        keep_unused=True,
    )
    concat_in = [
        np.concatenate([np.asarray(in_maps[c][nm]) for c in range(N_CORES)], axis=0)
        for nm in in_names
    ]
    concat_zeros = [
        np.zeros((N_CORES * a.shape[0], *a.shape[1:]), a.dtype) for a in out_avals
    ]
    dev_args = [jax.device_put(a, sh) for a in concat_in + concat_zeros]
    for a in dev_args:
        a.block_until_ready()

    def bench_once():
        outs = nodonate(*dev_args)
        for o in outs:
            o.block_until_ready()
        return outs

    bench_once.nodonate = nodonate
    bench_once.dev_args = dev_args
    return bench_once


def _prep_in_maps(x, W_qkv, b_qkv, W_out, b_out):
    bf = ml_dtypes.bfloat16
    xt = np.ascontiguousarray(
        x.reshape(B * T, E).T).astype(bf)                      # [E, B*T]
    wout = np.ascontiguousarray(
        W_out.reshape(8, 128, E).transpose(1, 0, 2).reshape(128, 8 * E)).astype(bf)
    bout = np.ascontiguousarray(
        np.broadcast_to(b_out.astype(np.float32)[None, :], (128, E)))

    in_maps = []
    for c in range(N_CORES):
        hs = [HL * c + i for i in range(HL)]
        qcols = np.concatenate(
            [W_qkv[:, 0 * E + h * D:0 * E + (h + 1) * D] for h in hs], axis=1)
        kcols = np.concatenate(
            [W_qkv[:, 1 * E + h * D:1 * E + (h + 1) * D] for h in hs], axis=1)
        wqk = np.ascontiguousarray(
            np.concatenate([qcols, kcols], axis=1)).astype(bf)  # [E, 256]
        zcol = np.zeros((E, 1), np.float32)
        wv = np.ascontiguousarray(np.concatenate(
            [arr for h in hs
             for arr in (W_qkv[:, 2 * E + h * D:2 * E + (h + 1) * D], zcol)],
            axis=1)).astype(bf)                                 # [E, 130]
        bq = np.concatenate([b_qkv[0 * E + h * D:0 * E + (h + 1) * D] for h in hs])
        bk = np.concatenate([b_qkv[1 * E + h * D:1 * E + (h + 1) * D] for h in hs])
        bqk = np.ascontiguousarray(
            np.stack([bq, bk], axis=1)).astype(np.float32)      # [128, 2]
        one = np.ones(1, np.float32)
        bvv = np.concatenate(
            [a for h in hs
             for a in (b_qkv[2 * E + h * D:2 * E + (h + 1) * D], one)])
        bv = np.ascontiguousarray(
            np.broadcast_to(bvv.astype(np.float32)[None, :], (128, 2 * DV)))
        in_maps.append({
            "xt": xt, "wqk": wqk, "wv": wv, "wout": wout,
            "bqk": bqk, "bv": bv, "bout": bout,
        })
    return in_maps


def kernel(x, W_qkv, b_qkv, W_out, b_out):
    x = np.asarray(x, dtype=np.float32)
    W_qkv = np.asarray(W_qkv, dtype=np.float32)
    b_qkv = np.asarray(b_qkv, dtype=np.float32)
    W_out = np.asarray(W_out, dtype=np.float32)
    b_out = np.asarray(b_out, dtype=np.float32)

    run = _get_runner()
    in_maps = _prep_in_maps(x, W_qkv, b_qkv, W_out, b_out)
    results = run(in_maps)

    out = np.empty((B, T, E), np.float32)
    for c in range(N_CORES):
        y = results[c]["y"]          # [B, HL, 128, E]
        for hl in range(HL):
            hg = HL * c + hl
            out[:, hg * 128:(hg + 1) * 128, :] = y[:, hl]
    return out


# revision 8
# speedup vs baseline: 1.2946x; 1.0003x over previous
"""Multi-head self-attention TRN2 kernel (B=2, T=2048, E=1024, H=16, D=64).

Sharding: tensor-parallel over heads - each of the 8 cores owns 2 heads.
Because the reference reshapes (B,H,T,D)->(B,T,E) with NO transpose, each
head's attention output maps to 128 complete contiguous rows of the
out_proj input, so the whole computation is embarrassingly parallel
across heads (no collectives).

Per-core pipeline (all matmuls bf16, accumulation fp32):
  1. qT/kT projections in [d, T] layout (heads stacked on partitions
     0-63 / 64-127); v in natural [T, d] layout augmented with a ones
     column (softmax denominator for free).
  2. scores^T tiles [kj=128, qi=512x2h] -> exp on ScalarE -> P^T bf16.
  3. attn@v in the cheap orientation: out o[q=128 partitions, d+1=65
     free] accumulated over kj tiles - PE cost 65 free/step instead of
     512 (the cost model charges output free size per accumulation
     step, so small-free x many-partitions wins 8x per instruction).
  4. normalize by per-partition (per-query) reciprocal on VectorE,
     downcast to bf16.
  5. o -> o^T via DMA xbar transposes (off the PE critical path), then
     shifted partition-duplicate copies build the [128=(d,j-parity), T]
     lhsT layout that performs the reference's "faithful reshape" for
     free in out_proj.
  6. out_proj: 8 accumulating matmuls per 512-column chunk.
"""

import numpy as np
import ml_dtypes

B, T, E, H, D = 2, 2048, 1024, 16, 64
N_CORES = 8
HL = H // N_CORES          # heads per core = 2
KP = E // 128              # 8 contraction partition-tiles
KT = T // 128              # 16 kj tiles
QC = T // 512              # 4 qi chunks of 512
QS = 4                     # q subtiles of 128 per chunk
DV = D + 1                 # v width incl. denominator ones column

_RUNNER = None


def _build_nc():
    import concourse.bacc as bacc
    import concourse.tile as tile
    import concourse.bass as bass
    import concourse.mybir as mybir

    from concourse import masks

    fp32 = mybir.dt.float32
    bf16 = mybir.dt.bfloat16
    ADD = mybir.AluOpType.add
    MULT = mybir.AluOpType.mult
    EXP = mybir.ActivationFunctionType.Exp
    COPY = mybir.ActivationFunctionType.Copy

    nc = bacc.Bacc("TRN2", target_bir_lowering=False, debug=False,
                   enable_asserts=True, num_devices=N_CORES)

    xt_d = nc.dram_tensor("xt", [E, B * T], bf16, kind="ExternalInput").ap()
    wqk_d = nc.dram_tensor("wqk", [E, 4 * D], bf16, kind="ExternalInput").ap()
    wv_d = nc.dram_tensor("wv", [E, 2 * DV], bf16, kind="ExternalInput").ap()
    wout_d = nc.dram_tensor("wout", [128, 8 * E], bf16, kind="ExternalInput").ap()
    bqk_d = nc.dram_tensor("bqk", [128, 2], fp32, kind="ExternalInput").ap()
    bv_d = nc.dram_tensor("bv", [128, 2 * DV], fp32, kind="ExternalInput").ap()
    bout_d = nc.dram_tensor("bout", [128, E], fp32, kind="ExternalInput").ap()
    y_d = nc.dram_tensor("y", [B, HL, 128, E], fp32, kind="ExternalOutput").ap()

    with tile.TileContext(nc) as tc:
        with (
            tc.tile_pool(name="const", bufs=1) as cpool,
            tc.tile_pool(name="ppool", bufs=26) as ppool,
            tc.tile_pool(name="npool", bufs=4) as npool,
            tc.tile_pool(name="ypool", bufs=4) as ypool,
            tc.tile_pool(name="ps_s", bufs=2, space=bass.MemorySpace.PSUM) as ps_s,
            tc.tile_pool(name="ps_o", bufs=1, space=bass.MemorySpace.PSUM) as ps_o,
            tc.tile_pool(name="ps_sm", bufs=2, space=bass.MemorySpace.PSUM) as ps_sm,
        ):
            # ---- constants / persistent tiles ----
            xt_sb = cpool.tile([128, KP, B * T], bf16, tag="xt")
            wqk_sb = cpool.tile([128, KP, 4 * D], bf16, tag="wqk")
            wv_sb = cpool.tile([128, KP, 2 * DV], bf16, tag="wv")
            wout_sb = cpool.tile([128, 8, E], bf16, tag="wout")
            bqk_sb = cpool.tile([128, 2], fp32, tag="bqk")
            bv_sb = cpool.tile([128, 2 * DV], fp32, tag="bv")
            bout_sb = cpool.tile([128, E], fp32, tag="bout")
            qkT = cpool.tile([128, B, 2, T], bf16, tag="qkT")
            vaug = cpool.tile([128, B, KT, 2 * DV], bf16, tag="vaug")
            # o natural layout, normalized bf16, 64 pad cols per head slot
            # (the pad transposes into oTd rows 64-127, which the shifted
            # dup copy then overwrites)
            # per-batch tensors (separate tags) so cross-batch false
            # dependencies cannot arise from coarse subtile tracking
            o_sb = [cpool.tile([128, KT, 192], bf16, tag=f"o_sb{bb}",
                               name=f"o_sb{bb}") for bb in range(B)]
            # o^T dup layout per head: rows 0-63 straight (written directly
            # by the xbar transpose), 64-127 shifted by one q (dup DMA)
            oTd = [cpool.tile([128, HL, T], bf16, tag=f"oTd{bb}",
                              name=f"oTd{bb}") for bb in range(B)]
            wub = cpool.tile([1, 128], bf16, tag="wub")
            ones1 = cpool.tile([1, 128], bf16, tag="ones1")
            bout_bf = cpool.tile([1, E], bf16, tag="bout_bf")
            ident = cpool.tile([128, 128], bf16, tag="ident")

            # warmup source (no DMA dependency); zero o_sb's pad lanes once
            nc.gpsimd.memset(wub[:], 0.0)
            nc.gpsimd.memset(ones1[:], 1.0)
            masks.make_identity(nc, ident[:])

            # ---- input DMAs: spread across the three HWDGE-ish rings so
            # queue overheads pipeline.  Transfers serialize on the shared
            # DMA engines; order = priority.  The first score tile needs
            # k cols + q[0:512], so those pieces go first, smallest first.
            wqk_r = wqk_d.rearrange("(a p) n -> p a n", p=128)
            xt_r = xt_d.rearrange("(a p) n -> p a n", p=128)
            nc.sync.dma_start(wqk_sb[:], wqk_r[:])
            # first 512 T-columns on the ACT ring (idle until exps start);
            # bigger slices keep the 1KB contiguous runs (no small-elem
            # DMA penalty)
            nc.scalar.dma_start(xt_sb[:, :, 0:256], xt_r[:, :, 0:256])
            nc.scalar.dma_start(xt_sb[:, :, 256:512], xt_r[:, :, 256:512])
            nc.scalar.dma_start(xt_sb[:, :, 512:1024], xt_r[:, :, 512:1024])
            nc.sync.dma_start(bqk_sb[:], bqk_d[:])
            nc.sync.dma_start(bv_sb[:], bv_d[:])
            # o_sb pad lanes zeroed on the gpsimd engine: doubles as a
            # delay so the Pool ring's chunk DMAs queue at the shared DMA
            # engines AFTER the ACT ring's critical first pieces
            for bb in range(B):
                nc.gpsimd.memset(o_sb[bb][:, :, 128:192], 0.0)
            # rest of x on the gpsimd SWDGE ring (engine otherwise idle);
            # wv mid-way (not needed until ~slot 11)
            for cc in range(2, B * T // 512):
                if cc == 4:
                    nc.gpsimd.dma_start(
                        wv_sb[:], wv_d.rearrange("(a p) n -> p a n", p=128))
                nc.gpsimd.dma_start(xt_sb[:, :, cc * 512:(cc + 1) * 512],
                                    xt_r[:, :, cc * 512:(cc + 1) * 512])

            # ---- PE warmup: keep the clock ramping from t~0 so the first
            # real matmuls run at full p-state.  src defaults to wub (no
            # deps -> the scheduler hoists them to kernel start); pass a
            # late-written AP to pin warmups into a late window instead.
            def warmup(n, src=None):
                s = wub if src is None else src
                for _ in range(n):
                    wt = ps_sm.tile([128, 128], fp32, tag="sm", name="wt")
                    nc.tensor.matmul(wt[:], s[0:1, 0:128], s[0:1, 0:128],
                                     start=True, stop=True)

            # ---- projection pieces ----
            def proj_q(b, n):
                # q^T chunk n: [128=(h0|h1)*d, 512]
                ps = ps_sm.tile([128, 512], fp32, tag="sm", name="psq")
                for k in range(KP):
                    nc.tensor.matmul(
                        ps[:],
                        wqk_sb[:, k, 0:128],
                        xt_sb[:, k, b * T + n * 512: b * T + (n + 1) * 512],
                        start=(k == 0), stop=(k == KP - 1),
                    )
                nc.vector.tensor_scalar(
                    qkT[:, b, 0, n * 512:(n + 1) * 512], ps[:],
                    bqk_sb[:, 0:1], None, op0=ADD,
                )

            def proj_q_half(b, n, hf):
                ps = ps_sm.tile([128, 512], fp32, tag="sm", name="psq")
                c0 = n * 512 + hf * 256
                for k in range(KP):
                    nc.tensor.matmul(
                        ps[:, 0:256],
                        wqk_sb[:, k, 0:128],
                        xt_sb[:, k, b * T + c0: b * T + c0 + 256],
                        start=(k == 0), stop=(k == KP - 1),
                    )
                nc.vector.tensor_scalar(
                    qkT[:, b, 0, c0:c0 + 256], ps[:, 0:256],
                    bqk_sb[:, 0:1], None, op0=ADD,
                )

            def proj_q_qs(b, n, qs):
                ps = ps_sm.tile([128, 512], fp32, tag="sm", name="psq")
                c0 = n * 512 + qs * 128
                for k in range(KP):
                    nc.tensor.matmul(
                        ps[:, 0:128],
                        wqk_sb[:, k, 0:128],
                        xt_sb[:, k, b * T + c0: b * T + c0 + 128],
                        start=(k == 0), stop=(k == KP - 1),
                    )
                nc.vector.tensor_scalar(
                    qkT[:, b, 0, c0:c0 + 128], ps[:, 0:128],
                    bqk_sb[:, 0:1], None, op0=ADD,
                )

            def proj_k(b, kt):
                # k^T tile kt: [128, 128]
                ps = ps_sm.tile([128, 512], fp32, tag="sm", name="psk")
                for k in range(KP):
                    nc.tensor.matmul(
                        ps[:, 0:128],
                        wqk_sb[:, k, 128:256],
                        xt_sb[:, k, b * T + kt * 128: b * T + (kt + 1) * 128],
                        start=(k == 0), stop=(k == KP - 1),
                    )
                nc.vector.tensor_scalar(
                    qkT[:, b, 1, kt * 128:(kt + 1) * 128], ps[:, 0:128],
                    bqk_sb[:, 1:2], None, op0=ADD,
                )

            def proj_v(b, r):
                # v natural [128, 2*DV] row tile r; W_v has zero columns at
                # the two "ones" slots and bv carries 1.0 there
                vp = ps_sm.tile([128, 512], fp32, tag="sm", name="vp")
                for k in range(KP):
                    nc.tensor.matmul(
                        vp[:, 0:2 * DV],
                        xt_sb[:, k, b * T + r * 128: b * T + (r + 1) * 128],
                        wv_sb[:, k, :],
                        start=(k == 0), stop=(k == KP - 1),
                    )
                nc.vector.tensor_tensor(
                    vaug[:, b, r, :], vp[:, 0:2 * DV], bv_sb[:], op=ADD,
                )

            def sc(b, qc, kt):
                # scores^T [kj=128, qi=512] per head, heads side by side
                S = ps_s.tile([128, 2 * 512], fp32, tag="S", name="S")
                for h in range(HL):
                    nc.tensor.matmul(
                        S[:, h * 512:(h + 1) * 512],
                        qkT[h * D:(h + 1) * D, b, 1, kt * 128:(kt + 1) * 128],
                        qkT[h * D:(h + 1) * D, b, 0, qc * 512:(qc + 1) * 512],
                        start=True, stop=True,
                    )
                return S

            def vmm_one(tile, c0, P, b, h, qs, kt):
                # o[q-subtile, d+1] += P^T[kj, q-sub]^T @ vaug[kj, d+1]
                nc.tensor.matmul(
                    tile[:, c0:c0 + DV],
                    P[:, h * 512 + qs * 128: h * 512 + (qs + 1) * 128],
                    vaug[:, b, kt, h * DV:(h + 1) * DV],
                    start=(kt == 0), stop=(kt == KT - 1),
                )

            def region_norm(b, qc, tile, c0, h, qs, on_act=False):
                # one region's reciprocal + normalize (tail interleaving)
                rdt = npool.tile([128, 8], fp32, tag="rd", name="rdt")
                nc.vector.reciprocal(
                    rdt[:, h * 4 + qs: h * 4 + qs + 1],
                    tile[:, c0 + D: c0 + DV])
                qt = qc * QS + qs
                if on_act:
                    nc.scalar.activation(
                        o_sb[b][:, qt, h * 64:(h + 1) * 64],
                        tile[:, c0: c0 + D], COPY,
                        scale=rdt[:, h * 4 + qs: h * 4 + qs + 1])
                else:
                    nc.vector.tensor_scalar(
                        o_sb[b][:, qt, h * 64:(h + 1) * 64],
                        tile[:, c0: c0 + D],
                        rdt[:, h * 4 + qs: h * 4 + qs + 1],
                        None, op0=MULT,
                    )

            def finalize(b, qc, ot, via_pe=False, skip_norms=False):
                # All norms (both heads) must precede the transposes: each
                # transpose reads a 128-col window spanning both head
                # slots.  via_pe: for the very last group the DMA
                # transpose+dup chain (~650ns queue overhead per hop) sits
                # exposed on the critical tail, so route it through PE
                # identity-matmuls + DVE/ACT copies instead (idle there).
                rd = npool.tile([128, 8], fp32, tag="rd", name="rd")
                if not skip_norms:
                    for h in range(HL):
                        nc.vector.reciprocal(
                            rd[:, h * 4:(h + 1) * 4].rearrange(
                                "p (a c) -> p a c", c=1),
                            ot[h].rearrange(
                                "p (q c) -> p q c", c=DV)[:, :, D:D + 1],
                        )
                    for qs in range(QS):
                        qt = qc * QS + qs
                        for h in range(HL):
                            nc.vector.tensor_scalar(
                                o_sb[b][:, qt, h * 64:(h + 1) * 64],
                                ot[h][:, qs * DV: qs * DV + D],
                                rd[:, h * 4 + qs: h * 4 + qs + 1],
                                None, op0=MULT,
                            )
                for qs in range(QS):
                    qt = qc * QS + qs
                    for h in range(HL):
                        # o^T; out rows 64-127 get the neighbour head slot /
                        # pad lanes, overwritten by the shifted dup below
                        if via_pe:
                            # bf16 [128,512] = same slot bytes as the sm tag
                            tp = ps_sm.tile([128, 512], bf16, tag="sm",
                                            name="tp")
                            nc.tensor.transpose(
                                tp[:, 0:128],
                                o_sb[b][:, qt, h * 64:h * 64 + 128],
                                ident[:])
                            nc.vector.tensor_copy(
                                oTd[b][0:128, h, qt * 128:(qt + 1) * 128],
                                tp[:, 0:128])
                        else:
                            nc.sync.dma_start_transpose(
                                oTd[b][0:128, h, qt * 128:(qt + 1) * 128],
                                o_sb[b][:, qt, h * 64:h * 64 + 128],
                            )
                for h in range(HL):
                    # shifted dup: via_pe does qc-1 + qc chunks; the DMA
                    # route defers to one whole-row dup after the last qc
                    # (fewer DMA producers -> fewer merged waits downstream)
                    chunks = []
                    if via_pe:
                        # cols 0..1023 were dup'd early (qc==2 below); the
                        # final group owns the last two 512-col pieces
                        chunks = [(1024, 512), (1536, 511)]
                    elif b == B - 1 and qc == QC - 2:
                        # early dup for the tail batch: sources complete
                        # through this group's transposes
                        chunks = [(0, 512), (512, 512)]
                    elif qc == QC - 1:
                        chunks.append((0, T - 1))
                    for c0, cw in chunks:
                        if via_pe:
                            dp = ps_sm.tile([128, 512], fp32, tag="sm",
                                            name="dp")
                            nc.tensor.matmul(
                                dp[0:64, 0:cw], ident[0:64, 0:64],
                                oTd[b][0:64, h, c0 + 1:c0 + cw + 1],
                                start=True, stop=True)
                            if h == 0:
                                nc.scalar.activation(
                                    oTd[b][64:128, h, c0:c0 + cw],
                                    dp[0:64, 0:cw], COPY)
                            else:
                                nc.vector.tensor_copy(
                                    oTd[b][64:128, h, c0:c0 + cw],
                                    dp[0:64, 0:cw])
                        else:
                            nc.sync.dma_start(
                                oTd[b][64:128, h, c0:c0 + cw],
                                oTd[b][0:64, h, c0 + 1:c0 + cw + 1])

            def outproj_piece(b, h, n5, s, split, tag="sm"):
                # one 1/split column chunk of out_proj for (b, h, n5); bias
                # folded in as a ones-row matmul so y DMAs straight out of
                # PSUM (no DVE hop on the critical tail)
                of2 = oTd[b][:, h, :].rearrange("p (t j) -> p j t", j=16)
                w = 512 // split
                c0 = n5 * 512 + s * w
                yp = ps_sm.tile([128, 512], fp32, tag=tag, name="yp") \
                    if tag == "sm" else \
                    ps_s.tile([128, 512], fp32, tag=tag, name="yp")
                for jj in range(8):
                    nc.tensor.matmul(
                        yp[:, 0:w],
                        of2[:, 2 * jj, :],
                        wout_sb[:, jj, c0:c0 + w],
                        start=(jj == 0), stop=False,
                    )
                nc.tensor.matmul(
                    yp[:, 0:w], ones1[0:1, :], bout_bf[0:1, c0:c0 + w],
                    start=False, stop=True,
                )
                # PSUM cannot feed a DMA directly; stage through SBUF on
                # DVE mid-stream / ACT at the tail (idle there)
                ys = ypool.tile([128, 512], fp32, tag="ys", name="ys")
                if b == 0 or (h + n5 + s) % 2 == 0:
                    nc.vector.tensor_copy(ys[:, 0:w], yp[:, 0:w])
                else:
                    nc.scalar.activation(ys[:, 0:w], yp[:, 0:w], COPY)
                # mid-stream y-writes ride the gpsimd SWDGE ring: keeps
                # their deep dependency chains off the SP ring's semaphore
                # window (an SP sem recycle once stalled the exp stream
                # 14us).  Tail y-writes (b=1) alternate SP/gpsimd.
                eng = nc.sync
                eng.dma_start(y_d[b, h, :, c0:c0 + w], ys[:, 0:w])

            # ---- filler schedule: all pieces ~0.43us (8 accumulating
            # matmuls of F<=130) so the score stream is never delayed by a
            # long chain sitting ahead of it in the PE queue.  Greedy
            # deadline placement, one piece per slot unless a deadline
            # forces more.
            SCHED = {}
            VROWS = {}

            def at_slot(slot, fn, vrow=None):
                key = (slot // 64, (slot % 64) // 16, slot % 16)
                SCHED.setdefault(key, []).append(fn)
                if vrow is not None:
                    VROWS.setdefault(key, []).append(vrow)

            def chunk_slot(c):
                # xt chunk c lands on the gpsimd ring ~2.9us apart; convert
                # to the exp-slot index from which a piece may read it
                if c == 0:
                    return -100
                return int((7814 + 2913 * c + 900 - 11090) / 1038) + 1

            pieces = []  # [deadline, earliest, fn, vrow]
            for b in range(B):
                for kt in range(KT):
                    if b == 0 and kt < 2:
                        continue  # prologue
                    c = (b * T + kt * 128) // 512
                    pieces.append(
                        [64 * b + kt - 2, chunk_slot(c),
                         (lambda bb, kk: lambda: proj_k(bb, kk))(b, kt),
                         None])
                for r in range(KT):
                    c = (b * T + r * 128) // 512
                    pieces.append(
                        [(14 if b == 0 else 50) + r // 2, chunk_slot(c),
                         (lambda bb, rr: lambda: proj_v(bb, rr))(b, r),
                         (b, r)])
                for n in range(QC):
                    if b == 0 and n == 0:
                        continue  # prologue
                    for qs in range(QS):
                        c = (b * T + n * 512 + qs * 128) // 512
                        pieces.append(
                            [64 * b + 16 * n - 3, chunk_slot(c),
                             (lambda bb, nn, qq:
                              lambda: proj_q_qs(bb, nn, qq))(b, n, qs),
                             None])
            for h in range(HL):
                for n5 in range(2):
                    for s in range(2):
                        pieces.append(
                            [100 + 3 * (4 * h + 2 * n5 + s), 95,
                             (lambda hh, nn, ss:
                              lambda: outproj_piece(0, hh, nn, ss, 2))(
                                  h, n5, s),
                             None])
            todo = sorted(pieces, key=lambda p: (p[0], p[1]))
            for slot in range(B * QC * KT):
                # keep group-boundary slots free: the finalize/norm chain
                # must reach the DVE queue head unimpeded.  Deadline-due
                # pieces are placed unconditionally (a qkT piece placed past
                # its deadline would be read stale by the score lookahead).
                boundary = slot % KT in (15, 0)
                n = 1 if boundary else 0
                for p in todo[:]:
                    if p[1] > slot:
                        continue
                    if p[0] <= slot:
                        at_slot(slot, p[2], p[3])
                        todo.remove(p)
                        n += 1
                    elif n == 0:
                        at_slot(slot, p[2], p[3])
                        todo.remove(p)
                        n += 1
                    else:
                        break
            assert not todo, [p[:2] for p in todo]

            # wout/bout on the gpsimd ring once startup DMA traffic is done
            at_slot(33, lambda: nc.gpsimd.dma_start(wout_sb[:], wout_d[:]))
            at_slot(35, lambda: nc.gpsimd.dma_start(bout_sb[:], bout_d[:]))
            at_slot(37, lambda: nc.vector.tensor_copy(
                bout_bf[0:1, :], bout_sb[0:1, :]))

            # ---- unified emission ----
            def _emit_all():
                seq = [(b, qc, kt) for b in range(B) for qc in range(QC)
                       for kt in range(KT)]
                vrows = {0: 0, 1: 0}
                # PSUM accumulation: one pending group per 2KB zero region
                # (bank).  Each o bank (og0=h0, og1=h1) streams its qs=0
                # region kt-incrementally through the group; regions qs=1-3
                # drain region-major after the group's last kt, two regions
                # per slot.
                pmap = {}            # (b,qc) -> {kt: P}
                stream_kts = {}      # (b,qc) -> kts streamed (qs=0 regions)
                drain_q = []         # groups past kt=15, awaiting qs 1-3
                drained = {}         # (b,qc) -> drained region count (0..6)
                otiles = {}          # (b,qc) -> [o_h0, o_h1]
                finalized = []       # groups finalized, in order
                stream_q = []        # groups awaiting/undergoing streaming

                LASTG = (B - 1, QC - 1)

                def alloc_group(g):
                    otiles[g] = [
                        ps_o.tile([128, QS * DV], fp32, tag=f"og{h}",
                                  name=f"og{h}")
                        for h in range(HL)
                    ]
                    if g == LASTG:
                        # two extra banks from the sm tag: four streaming
                        # regions, so only qs=2,3 drain after the last exp
                        otiles[g] += [
                            ps_sm.tile([128, 512], fp32, tag="sm",
                                       name=f"ogx{h}")
                            for h in range(HL)
                        ]

                def rmap(g, h, qs):
                    # region (h, qs) -> (tile, col0).  Last group: qs 0,2 in
                    # og_h (cols 0/65), qs 1,3 in the sm extras.
                    if g == LASTG:
                        if qs % 2 == 0:
                            return otiles[g][h], (qs // 2) * DV
                        return otiles[g][2 + h], (qs // 2) * DV
                    return otiles[g][h], qs * DV

                def group_done(g, skip_norms=False):
                    finalize(g[0], g[1], otiles[g],
                             via_pe=(g == (B - 1, QC - 1)),
                             skip_norms=skip_norms)
                    finalized.append(g)
                    otiles.pop(g)

                def flush(now_i, all_=False):
                    # 1) drain the head of drain_q: regions (qs>=1), two per
                    # call (region = 16 matmuls of F=65 ~ 0.43us each)
                    nreg = 1000 if all_ else 2
                    while drain_q and nreg > 0:
                        g = drain_q[0]
                        last = all_ and g == LASTG
                        qs0 = 2 if g == LASTG else 1
                        ndrain = HL * (QS - qs0)
                        d = drained.get(g, 0)
                        if last and d == 0:
                            # tail: the streamed regions are closed - norm
                            # them while PE drains the rest
                            for qs in range(qs0):
                                for h in range(HL):
                                    t_, c_ = rmap(g, h, qs)
                                    region_norm(g[0], g[1], t_, c_, h, qs,
                                                on_act=(h == 1))
                        take = min(ndrain - d, nreg)
                        for idx in range(d, d + take):
                            qs = qs0 + idx // HL
                            h = idx % HL
                            t_, c_ = rmap(g, h, qs)
                            for kt in range(KT):
                                vmm_one(t_, c_, pmap[g][kt], g[0],
                                        h, qs, kt)
                            if last:
                                region_norm(g[0], g[1], t_, c_, h, qs,
                                            on_act=(idx % 2 == 1))
                        drained[g] = d + take
                        nreg -= take
                        if drained[g] == ndrain:
                            drain_q.pop(0)
                            pmap.pop(g)
                            group_done(g, skip_norms=last)
                    # 2) stream qs=0 regions (head of stream_q) as kts and
                    # vaug rows become available; a group may only take the
                    # o banks once the previous group has fully vacated them
                    while stream_q:
                        g = stream_q[0]
                        if g not in otiles:
                            if otiles:
                                break    # banks still owned by prior group
                            alloc_group(g)
                        hi = min(max(pmap[g].keys()) + 1 if pmap[g] else 0,
                                 vrows[g[0]])
                        sk = stream_kts.get(g, 0)
                        nstream = 2 if g == LASTG else 1
                        for kt in range(sk, hi):
                            for qs in range(nstream):
                                for h in range(HL):
                                    t_, c_ = rmap(g, h, qs)
                                    vmm_one(t_, c_, pmap[g][kt], g[0],
                                            h, qs, kt)
                        stream_kts[g] = max(sk, hi)
                        if stream_kts[g] == KT:
                            drain_q.append(g)
                            stream_q.pop(0)
                            continue
                        break

                # ---- prologue: first score tile in column halves so the
                # exp stream starts as soon as the first xt piece lands ----
                warmup(52)
                proj_q_qs(0, 0, 0)
                proj_k(0, 0)
                S = ps_s.tile([128, 2 * 512], fp32, tag="S", name="S")
                for h in range(HL):
                    nc.tensor.matmul(
                        S[:, h * 512:h * 512 + 128],
                        qkT[h * D:(h + 1) * D, 0, 1, 0:128],
                        qkT[h * D:(h + 1) * D, 0, 0, 0:128],
                        start=True, stop=True)
                P0 = ppool.tile([128, 2 * 512], bf16, tag="P", name="P0")
                Sv0 = S.rearrange("p (h c) -> p h c", h=HL)
                Pv0 = P0.rearrange("p (h c) -> p h c", h=HL)
                nc.scalar.activation(Pv0[:, :, 0:128], Sv0[:, :, 0:128],
                                     EXP, scale=0.125)
                proj_q_qs(0, 0, 1)
                for h in range(HL):
                    nc.tensor.matmul(
                        S[:, h * 512 + 128:h * 512 + 256],
                        qkT[h * D:(h + 1) * D, 0, 1, 0:128],
                        qkT[h * D:(h + 1) * D, 0, 0, 128:256],
                        start=True, stop=True)
                nc.scalar.activation(Pv0[:, :, 128:256], Sv0[:, :, 128:256],
                                     EXP, scale=0.125)
                proj_k(0, 1)
                proj_q_qs(0, 0, 2)
                proj_q_qs(0, 0, 3)
                for h in range(HL):
                    nc.tensor.matmul(
                        S[:, h * 512 + 256:(h + 1) * 512],
                        qkT[h * D:(h + 1) * D, 0, 1, 0:128],
                        qkT[h * D:(h + 1) * D, 0, 0, 256:512],
                        start=True, stop=True)

                P_last = None
                for i, (b, qc, kt) in enumerate(seq):
                    if i == 0:
                        # first half emitted in the prologue on tile P0
                        P = P0
                        nc.scalar.activation(Pv0[:, :, 256:512],
                                             Sv0[:, :, 256:512],
                                             EXP, scale=0.125)
                    elif i == len(seq) - 1:
                        P = ppool.tile([128, 2 * 512], bf16, tag="P")
                        Svl = S.rearrange("p (h c) -> p h c", h=HL)
                        Pvl = P.rearrange("p (h c) -> p h c", h=HL)
                        nc.scalar.activation(Pvl[:, :, 0:256],
                                             Svl[:, :, 0:256],
                                             EXP, scale=0.125)
                        nc.scalar.activation(Pvl[:, :, 256:512],
                                             Svl[:, :, 256:512],
                                             EXP, scale=0.125)
                    else:
                        P = ppool.tile([128, 2 * 512], bf16, tag="P")
                        nc.scalar.activation(P[:], S[:], EXP, scale=0.125)
                    P_last = P
                    if i + 1 < len(seq):
                        S = sc(*seq[i + 1])
                    for fn in SCHED.get((b, qc, kt), []):
                        fn()
                    for (vb, r) in VROWS.get((b, qc, kt), []):
                        vrows[vb] = max(vrows[vb], r + 1)
                    if kt == 0:
                        stream_q.append((b, qc))
                        pmap[(b, qc)] = {}
                    pmap[(b, qc)][kt] = P
                    flush(i)

                # ---- tail ----
                while drain_q or stream_q:
                    flush(len(seq), all_=True)
                assert len(finalized) == B * QC, finalized
                # keep the PE p-state hot across the norm/transpose/dup
                # wait; chained on the last P tile so the scheduler cannot
                # hoist these out of the tail window
                warmup(0, src=P_last)
                for h in range(HL):
                    for n5 in range(2):
                        for s in range(2):
                            outproj_piece(1, h, n5, s, 2,
                                          tag=("S" if (n5 + s) % 2 else "sm"))

            _emit_all()

    nc.compile()
    return nc


def _get_runner():
    """Build + compile once; return a callable(in_maps) -> list of out dicts."""
    global _RUNNER
    if _RUNNER is not None:
        return _RUNNER

    import jax
    import concourse.mybir as mybir
    from concourse import bass2jax
    from jax.experimental.shard_map import shard_map
    from jax.sharding import Mesh, PartitionSpec

    nc = _build_nc()
    bass2jax.install_neuronx_cc_hook()

    partition_name = (nc.partition_id_tensor.name
                      if nc.partition_id_tensor else None)
    in_names, out_names, out_avals = [], [], []
    for alloc in nc.m.functions[0].allocations:
        if not isinstance(alloc, mybir.MemoryLocationSet):
            continue
        name = alloc.memorylocations[0].name
        if alloc.kind == "ExternalInput":
            if name != partition_name:
                in_names.append(name)
        elif alloc.kind == "ExternalOutput":
            out_names.append(name)
            out_avals.append(jax.core.ShapedArray(
                tuple(alloc.tensor_shape), mybir.dt.np(alloc.dtype)))

    n_params, n_outs = len(in_names), len(out_avals)
    all_names = in_names + out_names
    if partition_name is not None:
        all_names = all_names + [partition_name]

    def _body(*args):
        operands = list(args)
        if partition_name is not None:
            operands.append(bass2jax.partition_id_tensor())
        outs = bass2jax._bass_exec_p.bind(
            *operands,
            out_avals=tuple(out_avals),
            in_names=tuple(all_names),
            out_names=tuple(out_names),
            lowering_input_output_aliases=(),
            sim_require_finite=True,
            sim_require_nnan=True,
            nc=nc,
        )
        return tuple(outs)

    devices = jax.devices()[:N_CORES]
    mesh = Mesh(np.asarray(devices), ("core",))
    in_specs = (PartitionSpec("core"),) * (n_params + n_outs)
    out_specs = (PartitionSpec("core"),) * n_outs
    donate = tuple(range(n_params, n_params + n_outs))
    sharded = jax.jit(
        shard_map(_body, mesh=mesh, in_specs=in_specs, out_specs=out_specs,
                  check_rep=False),
        donate_argnums=donate, keep_unused=True,
    )

    def run(in_maps):
        concat_in = [
            np.concatenate([np.asarray(in_maps[c][nm]) for c in range(N_CORES)],
                           axis=0)
            for nm in in_names
        ]
        concat_zeros = [
            np.zeros((N_CORES * a.shape[0], *a.shape[1:]), a.dtype)
            for a in out_avals
        ]
        out_arrs = sharded(*concat_in, *concat_zeros)
        return [
            {nm: np.asarray(out_arrs[i]).reshape(N_CORES, *out_avals[i].shape)[c]
             for i, nm in enumerate(out_names)}
            for c in range(N_CORES)
        ]

    _RUNNER = run
    run._bench_parts = (sharded, mesh, in_names, out_names, out_avals,
                        n_params, _body)
    return run


def _make_bench(in_maps):
    """Device-resident benchmark closure: returns fn() that runs one
    execution with all inputs already on device (no donation)."""
    import jax
    from jax.experimental.shard_map import shard_map
    from jax.sharding import NamedSharding, PartitionSpec

    run = _get_runner()
    sharded, mesh, in_names, out_names, out_avals, n_params, _body = \
        run._bench_parts
    sh = NamedSharding(mesh, PartitionSpec("core"))

    nodonate = jax.jit(
        shard_map(_body, mesh=mesh,
                  in_specs=(PartitionSpec("core"),) * (n_params + len(out_avals)),
                  out_specs=(PartitionSpec("core"),) * len(out_avals),
                  check_rep=False),
        keep_unused=True,
    )
    concat_in = [
        np.concatenate([np.asarray(in_maps[c][nm]) for c in range(N_CORES)], axis=0)
        for nm in in_names
    ]
    concat_zeros = [
        np.zeros((N_CORES * a.shape[0], *a.shape[1:]), a.dtype) for a in out_avals
    ]
    dev_args = [jax.device_put(a, sh) for a in concat_in + concat_zeros]
    for a in dev_args:
        a.block_until_ready()

    def bench_once():
        outs = nodonate(*dev_args)
        for o in outs:
            o.block_until_ready()
        return outs

    bench_once.nodonate = nodonate
    bench_once.dev_args = dev_args
    return bench_once


def _prep_in_maps(x, W_qkv, b_qkv, W_out, b_out):
    bf = ml_dtypes.bfloat16
    xt = np.ascontiguousarray(
        x.reshape(B * T, E).T).astype(bf)                      # [E, B*T]
    wout = np.ascontiguousarray(
        W_out.reshape(8, 128, E).transpose(1, 0, 2).reshape(128, 8 * E)).astype(bf)
    bout = np.ascontiguousarray(
        np.broadcast_to(b_out.astype(np.float32)[None, :], (128, E)))

    in_maps = []
    for c in range(N_CORES):
        hs = [HL * c + i for i in range(HL)]
        qcols = np.concatenate(
            [W_qkv[:, 0 * E + h * D:0 * E + (h + 1) * D] for h in hs], axis=1)
        kcols = np.concatenate(
            [W_qkv[:, 1 * E + h * D:1 * E + (h + 1) * D] for h in hs], axis=1)
        wqk = np.ascontiguousarray(
            np.concatenate([qcols, kcols], axis=1)).astype(bf)  # [E, 256]
        zcol = np.zeros((E, 1), np.float32)
        wv = np.ascontiguousarray(np.concatenate(
            [arr for h in hs
             for arr in (W_qkv[:, 2 * E + h * D:2 * E + (h + 1) * D], zcol)],
            axis=1)).astype(bf)                                 # [E, 130]
        bq = np.concatenate([b_qkv[0 * E + h * D:0 * E + (h + 1) * D] for h in hs])
        bk = np.concatenate([b_qkv[1 * E + h * D:1 * E + (h + 1) * D] for h in hs])
        bqk = np.ascontiguousarray(
            np.stack([bq, bk], axis=1)).astype(np.float32)      # [128, 2]
        one = np.ones(1, np.float32)
        bvv = np.concatenate(
            [a for h in hs
             for a in (b_qkv[2 * E + h * D:2 * E + (h + 1) * D], one)])
        bv = np.ascontiguousarray(
            np.broadcast_to(bvv.astype(np.float32)[None, :], (128, 2 * DV)))
        in_maps.append({
            "xt": xt, "wqk": wqk, "wv": wv, "wout": wout,
            "bqk": bqk, "bv": bv, "bout": bout,
        })
    return in_maps


def kernel(x, W_qkv, b_qkv, W_out, b_out):
    x = np.asarray(x, dtype=np.float32)
    W_qkv = np.asarray(W_qkv, dtype=np.float32)
    b_qkv = np.asarray(b_qkv, dtype=np.float32)
    W_out = np.asarray(W_out, dtype=np.float32)
    b_out = np.asarray(b_out, dtype=np.float32)

    run = _get_runner()
    in_maps = _prep_in_maps(x, W_qkv, b_qkv, W_out, b_out)
    results = run(in_maps)

    out = np.empty((B, T, E), np.float32)
    for c in range(N_CORES):
        y = results[c]["y"]          # [B, HL, 128, E]
        for hl in range(HL):
            hg = HL * c + hl
            out[:, hg * 128:(hg + 1) * 128, :] = y[:, hl]
    return out


# revision 9
# speedup vs baseline: 1.2974x; 1.0021x over previous
"""Multi-head self-attention TRN2 kernel (B=2, T=2048, E=1024, H=16, D=64).

Sharding: tensor-parallel over heads - each of the 8 cores owns 2 heads.
Because the reference reshapes (B,H,T,D)->(B,T,E) with NO transpose, each
head's attention output maps to 128 complete contiguous rows of the
out_proj input, so the whole computation is embarrassingly parallel
across heads (no collectives).

Per-core pipeline (all matmuls bf16, accumulation fp32):
  1. qT/kT projections in [d, T] layout (heads stacked on partitions
     0-63 / 64-127); v in natural [T, d] layout augmented with a ones
     column (softmax denominator for free).
  2. scores^T tiles [kj=128, qi=512x2h] -> exp on ScalarE -> P^T bf16.
  3. attn@v in the cheap orientation: out o[q=128 partitions, d+1=65
     free] accumulated over kj tiles - PE cost 65 free/step instead of
     512 (the cost model charges output free size per accumulation
     step, so small-free x many-partitions wins 8x per instruction).
  4. normalize by per-partition (per-query) reciprocal on VectorE,
     downcast to bf16.
  5. o -> o^T via DMA xbar transposes (off the PE critical path), then
     shifted partition-duplicate copies build the [128=(d,j-parity), T]
     lhsT layout that performs the reference's "faithful reshape" for
     free in out_proj.
  6. out_proj: 8 accumulating matmuls per 512-column chunk.
"""

import numpy as np
import ml_dtypes

B, T, E, H, D = 2, 2048, 1024, 16, 64
N_CORES = 8
HL = H // N_CORES          # heads per core = 2
KP = E // 128              # 8 contraction partition-tiles
KT = T // 128              # 16 kj tiles
QC = T // 512              # 4 qi chunks of 512
QS = 4                     # q subtiles of 128 per chunk
DV = D + 1                 # v width incl. denominator ones column

_RUNNER = None


def _build_nc():
    import concourse.bacc as bacc
    import concourse.tile as tile
    import concourse.bass as bass
    import concourse.mybir as mybir

    from concourse import masks

    fp32 = mybir.dt.float32
    bf16 = mybir.dt.bfloat16
    ADD = mybir.AluOpType.add
    MULT = mybir.AluOpType.mult
    EXP = mybir.ActivationFunctionType.Exp
    COPY = mybir.ActivationFunctionType.Copy

    nc = bacc.Bacc("TRN2", target_bir_lowering=False, debug=False,
                   enable_asserts=True, num_devices=N_CORES)

    xt_d = nc.dram_tensor("xt", [E, B * T], bf16, kind="ExternalInput").ap()
    wqk_d = nc.dram_tensor("wqk", [E, 4 * D], bf16, kind="ExternalInput").ap()
    wv_d = nc.dram_tensor("wv", [E, 2 * DV], bf16, kind="ExternalInput").ap()
    wout_d = nc.dram_tensor("wout", [128, 8 * E], bf16, kind="ExternalInput").ap()
    bqk_d = nc.dram_tensor("bqk", [128, 2], fp32, kind="ExternalInput").ap()
    bv_d = nc.dram_tensor("bv", [128, 2 * DV], fp32, kind="ExternalInput").ap()
    bout_d = nc.dram_tensor("bout", [128, E], fp32, kind="ExternalInput").ap()
    y_d = nc.dram_tensor("y", [B, HL, 128, E], fp32, kind="ExternalOutput").ap()

    with tile.TileContext(nc) as tc:
        with (
            tc.tile_pool(name="const", bufs=1) as cpool,
            tc.tile_pool(name="ppool", bufs=26) as ppool,
            tc.tile_pool(name="npool", bufs=4) as npool,
            tc.tile_pool(name="ypool", bufs=4) as ypool,
            tc.tile_pool(name="ps_s", bufs=2, space=bass.MemorySpace.PSUM) as ps_s,
            tc.tile_pool(name="ps_o", bufs=1, space=bass.MemorySpace.PSUM) as ps_o,
            tc.tile_pool(name="ps_sm", bufs=2, space=bass.MemorySpace.PSUM) as ps_sm,
        ):
            # ---- constants / persistent tiles ----
            xt_sb = cpool.tile([128, KP, B * T], bf16, tag="xt")
            wqk_sb = cpool.tile([128, KP, 4 * D], bf16, tag="wqk")
            wv_sb = cpool.tile([128, KP, 2 * DV], bf16, tag="wv")
            wout_sb = cpool.tile([128, 8, E], bf16, tag="wout")
            bqk_sb = cpool.tile([128, 2], fp32, tag="bqk")
            bv_sb = cpool.tile([128, 2 * DV], fp32, tag="bv")
            bout_sb = cpool.tile([128, E], fp32, tag="bout")
            qkT = cpool.tile([128, B, 2, T], bf16, tag="qkT")
            vaug = cpool.tile([128, B, KT, 2 * DV], bf16, tag="vaug")
            # o natural layout, normalized bf16, 64 pad cols per head slot
            # (the pad transposes into oTd rows 64-127, which the shifted
            # dup copy then overwrites)
            # per-batch tensors (separate tags) so cross-batch false
            # dependencies cannot arise from coarse subtile tracking
            o_sb = [cpool.tile([128, KT, 192], bf16, tag=f"o_sb{bb}",
                               name=f"o_sb{bb}") for bb in range(B)]
            # o^T dup layout per head: rows 0-63 straight (written directly
            # by the xbar transpose), 64-127 shifted by one q (dup DMA)
            oTd = [cpool.tile([128, HL, T], bf16, tag=f"oTd{bb}",
                              name=f"oTd{bb}") for bb in range(B)]
            wub = cpool.tile([1, 128], bf16, tag="wub")
            ones1 = cpool.tile([1, 128], bf16, tag="ones1")
            bout_bf = cpool.tile([1, E], bf16, tag="bout_bf")
            ident = cpool.tile([128, 128], bf16, tag="ident")

            # warmup source (no DMA dependency); zero o_sb's pad lanes once
            nc.gpsimd.memset(wub[:], 0.0)
            nc.gpsimd.memset(ones1[:], 1.0)
            masks.make_identity(nc, ident[:])

            # ---- input DMAs: spread across the three HWDGE-ish rings so
            # queue overheads pipeline.  Transfers serialize on the shared
            # DMA engines; order = priority.  The first score tile needs
            # k cols + q[0:512], so those pieces go first, smallest first.
            wqk_r = wqk_d.rearrange("(a p) n -> p a n", p=128)
            xt_r = xt_d.rearrange("(a p) n -> p a n", p=128)
            nc.sync.dma_start(wqk_sb[:], wqk_r[:])
            # first 512 T-columns on the ACT ring (idle until exps start);
            # bigger slices keep the 1KB contiguous runs (no small-elem
            # DMA penalty)
            nc.scalar.dma_start(xt_sb[:, :, 0:256], xt_r[:, :, 0:256])
            nc.scalar.dma_start(xt_sb[:, :, 256:512], xt_r[:, :, 256:512])
            nc.scalar.dma_start(xt_sb[:, :, 512:1024], xt_r[:, :, 512:1024])
            nc.sync.dma_start(bqk_sb[:], bqk_d[:])
            nc.sync.dma_start(bv_sb[:], bv_d[:])
            # o_sb pad lanes zeroed on the gpsimd engine: doubles as a
            # delay so the Pool ring's chunk DMAs queue at the shared DMA
            # engines AFTER the ACT ring's critical first pieces
            for bb in range(B):
                nc.gpsimd.memset(o_sb[bb][:, :, 128:192], 0.0)
            # rest of x on the gpsimd SWDGE ring (engine otherwise idle);
            # wv mid-way (not needed until ~slot 11)
            for cc in range(2, B * T // 512):
                if cc == 4:
                    nc.gpsimd.dma_start(
                        wv_sb[:], wv_d.rearrange("(a p) n -> p a n", p=128))
                nc.gpsimd.dma_start(xt_sb[:, :, cc * 512:(cc + 1) * 512],
                                    xt_r[:, :, cc * 512:(cc + 1) * 512])

            # ---- PE warmup: keep the clock ramping from t~0 so the first
            # real matmuls run at full p-state.  src defaults to wub (no
            # deps -> the scheduler hoists them to kernel start); pass a
            # late-written AP to pin warmups into a late window instead.
            def warmup(n, src=None):
                s = wub if src is None else src
                for _ in range(n):
                    wt = ps_sm.tile([128, 128], fp32, tag="sm", name="wt")
                    nc.tensor.matmul(wt[:], s[0:1, 0:128], s[0:1, 0:128],
                                     start=True, stop=True)

            # ---- projection pieces ----
            def proj_q(b, n):
                # q^T chunk n: [128=(h0|h1)*d, 512]
                ps = ps_sm.tile([128, 512], fp32, tag="sm", name="psq")
                for k in range(KP):
                    nc.tensor.matmul(
                        ps[:],
                        wqk_sb[:, k, 0:128],
                        xt_sb[:, k, b * T + n * 512: b * T + (n + 1) * 512],
                        start=(k == 0), stop=(k == KP - 1),
                    )
                nc.vector.tensor_scalar(
                    qkT[:, b, 0, n * 512:(n + 1) * 512], ps[:],
                    bqk_sb[:, 0:1], None, op0=ADD,
                )

            def proj_q_half(b, n, hf):
                ps = ps_sm.tile([128, 512], fp32, tag="sm", name="psq")
                c0 = n * 512 + hf * 256
                for k in range(KP):
                    nc.tensor.matmul(
                        ps[:, 0:256],
                        wqk_sb[:, k, 0:128],
                        xt_sb[:, k, b * T + c0: b * T + c0 + 256],
                        start=(k == 0), stop=(k == KP - 1),
                    )
                nc.vector.tensor_scalar(
                    qkT[:, b, 0, c0:c0 + 256], ps[:, 0:256],
                    bqk_sb[:, 0:1], None, op0=ADD,
                )

            def proj_q_qs(b, n, qs):
                ps = ps_sm.tile([128, 512], fp32, tag="sm", name="psq")
                c0 = n * 512 + qs * 128
                for k in range(KP):
                    nc.tensor.matmul(
                        ps[:, 0:128],
                        wqk_sb[:, k, 0:128],
                        xt_sb[:, k, b * T + c0: b * T + c0 + 128],
                        start=(k == 0), stop=(k == KP - 1),
                    )
                nc.vector.tensor_scalar(
                    qkT[:, b, 0, c0:c0 + 128], ps[:, 0:128],
                    bqk_sb[:, 0:1], None, op0=ADD,
                )

            def proj_k(b, kt):
                # k^T tile kt: [128, 128]
                ps = ps_sm.tile([128, 512], fp32, tag="sm", name="psk")
                for k in range(KP):
                    nc.tensor.matmul(
                        ps[:, 0:128],
                        wqk_sb[:, k, 128:256],
                        xt_sb[:, k, b * T + kt * 128: b * T + (kt + 1) * 128],
                        start=(k == 0), stop=(k == KP - 1),
                    )
                nc.vector.tensor_scalar(
                    qkT[:, b, 1, kt * 128:(kt + 1) * 128], ps[:, 0:128],
                    bqk_sb[:, 1:2], None, op0=ADD,
                )

            def proj_v(b, r):
                # v natural [128, 2*DV] row tile r; W_v has zero columns at
                # the two "ones" slots and bv carries 1.0 there
                vp = ps_sm.tile([128, 512], fp32, tag="sm", name="vp")
                for k in range(KP):
                    nc.tensor.matmul(
                        vp[:, 0:2 * DV],
                        xt_sb[:, k, b * T + r * 128: b * T + (r + 1) * 128],
                        wv_sb[:, k, :],
                        start=(k == 0), stop=(k == KP - 1),
                    )
                nc.vector.tensor_tensor(
                    vaug[:, b, r, :], vp[:, 0:2 * DV], bv_sb[:], op=ADD,
                )

            def sc(b, qc, kt):
                # scores^T [kj=128, qi=512] per head, heads side by side
                S = ps_s.tile([128, 2 * 512], fp32, tag="S", name="S")
                for h in range(HL):
                    nc.tensor.matmul(
                        S[:, h * 512:(h + 1) * 512],
                        qkT[h * D:(h + 1) * D, b, 1, kt * 128:(kt + 1) * 128],
                        qkT[h * D:(h + 1) * D, b, 0, qc * 512:(qc + 1) * 512],
                        start=True, stop=True,
                    )
                return S

            def vmm_one(tile, c0, P, b, h, qs, kt):
                # o[q-subtile, d+1] += P^T[kj, q-sub]^T @ vaug[kj, d+1]
                nc.tensor.matmul(
                    tile[:, c0:c0 + DV],
                    P[:, h * 512 + qs * 128: h * 512 + (qs + 1) * 128],
                    vaug[:, b, kt, h * DV:(h + 1) * DV],
                    start=(kt == 0), stop=(kt == KT - 1),
                )

            def region_norm(b, qc, tile, c0, h, qs, on_act=False):
                # one region's reciprocal + normalize (tail interleaving)
                rdt = npool.tile([128, 8], fp32, tag="rd", name="rdt")
                nc.vector.reciprocal(
                    rdt[:, h * 4 + qs: h * 4 + qs + 1],
                    tile[:, c0 + D: c0 + DV])
                qt = qc * QS + qs
                if on_act:
                    nc.scalar.activation(
                        o_sb[b][:, qt, h * 64:(h + 1) * 64],
                        tile[:, c0: c0 + D], COPY,
                        scale=rdt[:, h * 4 + qs: h * 4 + qs + 1])
                else:
                    nc.vector.tensor_scalar(
                        o_sb[b][:, qt, h * 64:(h + 1) * 64],
                        tile[:, c0: c0 + D],
                        rdt[:, h * 4 + qs: h * 4 + qs + 1],
                        None, op0=MULT,
                    )

            def finalize(b, qc, ot, via_pe=False, skip_norms=False):
                # All norms (both heads) must precede the transposes: each
                # transpose reads a 128-col window spanning both head
                # slots.  via_pe: for the very last group the DMA
                # transpose+dup chain (~650ns queue overhead per hop) sits
                # exposed on the critical tail, so route it through PE
                # identity-matmuls + DVE/ACT copies instead (idle there).
                rd = npool.tile([128, 8], fp32, tag="rd", name="rd")
                if not skip_norms:
                    for h in range(HL):
                        nc.vector.reciprocal(
                            rd[:, h * 4:(h + 1) * 4].rearrange(
                                "p (a c) -> p a c", c=1),
                            ot[h].rearrange(
                                "p (q c) -> p q c", c=DV)[:, :, D:D + 1],
                        )
                    for qs in range(QS):
                        qt = qc * QS + qs
                        for h in range(HL):
                            nc.vector.tensor_scalar(
                                o_sb[b][:, qt, h * 64:(h + 1) * 64],
                                ot[h][:, qs * DV: qs * DV + D],
                                rd[:, h * 4 + qs: h * 4 + qs + 1],
                                None, op0=MULT,
                            )
                for qs in range(QS):
                    qt = qc * QS + qs
                    for h in range(HL):
                        # o^T; out rows 64-127 get the neighbour head slot /
                        # pad lanes, overwritten by the shifted dup below
                        if via_pe:
                            # bf16 [128,512] = same slot bytes as the sm tag
                            tp = ps_sm.tile([128, 512], bf16, tag="sm",
                                            name="tp")
                            nc.tensor.transpose(
                                tp[:, 0:128],
                                o_sb[b][:, qt, h * 64:h * 64 + 128],
                                ident[:])
                            nc.vector.tensor_copy(
                                oTd[b][0:128, h, qt * 128:(qt + 1) * 128],
                                tp[:, 0:128])
                        else:
                            nc.sync.dma_start_transpose(
                                oTd[b][0:128, h, qt * 128:(qt + 1) * 128],
                                o_sb[b][:, qt, h * 64:h * 64 + 128],
                            )
                for h in range(HL):
                    # shifted dup: via_pe does qc-1 + qc chunks; the DMA
                    # route defers to one whole-row dup after the last qc
                    # (fewer DMA producers -> fewer merged waits downstream)
                    chunks = []
                    if via_pe:
                        # cols 0..1023 were dup'd early (qc==2 below); the
                        # final group owns the last two 512-col pieces
                        chunks = [(1024, 512), (1536, 511)]
                    elif b == B - 1 and qc == QC - 2:
                        # early dup for the tail batch: sources complete
                        # through this group's transposes
                        chunks = [(0, 512), (512, 512)]
                    elif qc == QC - 1:
                        chunks.append((0, T - 1))
                    for c0, cw in chunks:
                        if via_pe:
                            dp = ps_sm.tile([128, 512], fp32, tag="sm",
                                            name="dp")
                            nc.tensor.matmul(
                                dp[0:64, 0:cw], ident[0:64, 0:64],
                                oTd[b][0:64, h, c0 + 1:c0 + cw + 1],
                                start=True, stop=True)
                            if h == 0:
                                nc.scalar.activation(
                                    oTd[b][64:128, h, c0:c0 + cw],
                                    dp[0:64, 0:cw], COPY)
                            else:
                                nc.vector.tensor_copy(
                                    oTd[b][64:128, h, c0:c0 + cw],
                                    dp[0:64, 0:cw])
                        else:
                            nc.sync.dma_start(
                                oTd[b][64:128, h, c0:c0 + cw],
                                oTd[b][0:64, h, c0 + 1:c0 + cw + 1])

            def outproj_piece(b, h, n5, s, split, tag="sm"):
                # one 1/split column chunk of out_proj for (b, h, n5); bias
                # folded in as a ones-row matmul so y DMAs straight out of
                # PSUM (no DVE hop on the critical tail)
                of2 = oTd[b][:, h, :].rearrange("p (t j) -> p j t", j=16)
                w = 512 // split
                c0 = n5 * 512 + s * w
                yp = ps_sm.tile([128, 512], fp32, tag=tag, name="yp") \
                    if tag == "sm" else \
                    ps_s.tile([128, 512], fp32, tag=tag, name="yp")
                for jj in range(8):
                    nc.tensor.matmul(
                        yp[:, 0:w],
                        of2[:, 2 * jj, :],
                        wout_sb[:, jj, c0:c0 + w],
                        start=(jj == 0), stop=False,
                    )
                nc.tensor.matmul(
                    yp[:, 0:w], ones1[0:1, :], bout_bf[0:1, c0:c0 + w],
                    start=False, stop=True,
                )
                # PSUM cannot feed a DMA directly; stage through SBUF on
                # DVE mid-stream / ACT at the tail (idle there)
                ys = ypool.tile([128, 512], fp32, tag="ys", name="ys")
                if b == 0 or (h + n5 + s) % 2 == 0:
                    nc.vector.tensor_copy(ys[:, 0:w], yp[:, 0:w])
                else:
                    nc.scalar.activation(ys[:, 0:w], yp[:, 0:w], COPY)
                # mid-stream y-writes ride the gpsimd SWDGE ring: keeps
                # their deep dependency chains off the SP ring's semaphore
                # window (an SP sem recycle once stalled the exp stream
                # 14us).  Tail y-writes (b=1) alternate SP/gpsimd.
                eng = nc.sync
                eng.dma_start(y_d[b, h, :, c0:c0 + w], ys[:, 0:w])

            # ---- filler schedule: all pieces ~0.43us (8 accumulating
            # matmuls of F<=130) so the score stream is never delayed by a
            # long chain sitting ahead of it in the PE queue.  Greedy
            # deadline placement, one piece per slot unless a deadline
            # forces more.
            SCHED = {}
            VROWS = {}

            def at_slot(slot, fn, vrow=None):
                key = (slot // 64, (slot % 64) // 16, slot % 16)
                SCHED.setdefault(key, []).append(fn)
                if vrow is not None:
                    VROWS.setdefault(key, []).append(vrow)

            def chunk_slot(c):
                # xt chunk c lands on the gpsimd ring ~2.9us apart; convert
                # to the exp-slot index from which a piece may read it
                if c == 0:
                    return -100
                return int((7814 + 2913 * c + 900 - 11090) / 1038) + 1

            pieces = []  # [deadline, earliest, fn, vrow]
            for b in range(B):
                for kt in range(KT):
                    if b == 0 and kt < 2:
                        continue  # prologue
                    c = (b * T + kt * 128) // 512
                    pieces.append(
                        [64 * b + kt - 2, chunk_slot(c),
                         (lambda bb, kk: lambda: proj_k(bb, kk))(b, kt),
                         None])
                for r in range(KT):
                    c = (b * T + r * 128) // 512
                    pieces.append(
                        [(14 if b == 0 else 50) + r // 2, chunk_slot(c),
                         (lambda bb, rr: lambda: proj_v(bb, rr))(b, r),
                         (b, r)])
                for n in range(QC):
                    if b == 0 and n == 0:
                        continue  # prologue
                    for qs in range(QS):
                        c = (b * T + n * 512 + qs * 128) // 512
                        pieces.append(
                            [64 * b + 16 * n - 3, chunk_slot(c),
                             (lambda bb, nn, qq:
                              lambda: proj_q_qs(bb, nn, qq))(b, n, qs),
                             None])
            for h in range(HL):
                for n5 in range(2):
                    for s in range(2):
                        pieces.append(
                            [100 + 3 * (4 * h + 2 * n5 + s), 95,
                             (lambda hh, nn, ss:
                              lambda: outproj_piece(0, hh, nn, ss, 2))(
                                  h, n5, s),
                             None])
            todo = sorted(pieces, key=lambda p: (p[0], p[1]))
            for slot in range(B * QC * KT):
                # keep group-boundary slots free: the finalize/norm chain
                # must reach the DVE queue head unimpeded.  Deadline-due
                # pieces are placed unconditionally (a qkT piece placed past
                # its deadline would be read stale by the score lookahead).
                boundary = slot % KT in (15, 0)
                n = 1 if boundary else 0
                for p in todo[:]:
                    if p[1] > slot:
                        continue
                    if p[0] <= slot:
                        at_slot(slot, p[2], p[3])
                        todo.remove(p)
                        n += 1
                    elif n == 0:
                        at_slot(slot, p[2], p[3])
                        todo.remove(p)
                        n += 1
                    else:
                        break
            assert not todo, [p[:2] for p in todo]

            # wout/bout on the gpsimd ring once startup DMA traffic is done
            at_slot(33, lambda: nc.gpsimd.dma_start(wout_sb[:], wout_d[:]))
            at_slot(35, lambda: nc.gpsimd.dma_start(bout_sb[:], bout_d[:]))
            at_slot(37, lambda: nc.vector.tensor_copy(
                bout_bf[0:1, :], bout_sb[0:1, :]))

            # ---- unified emission ----
            def _emit_all():
                seq = [(b, qc, kt) for b in range(B) for qc in range(QC)
                       for kt in range(KT)]
                vrows = {0: 0, 1: 0}
                # PSUM accumulation: one pending group per 2KB zero region
                # (bank).  Each o bank (og0=h0, og1=h1) streams its qs=0
                # region kt-incrementally through the group; regions qs=1-3
                # drain region-major after the group's last kt, two regions
                # per slot.
                pmap = {}            # (b,qc) -> {kt: P}
                stream_kts = {}      # (b,qc) -> kts streamed (qs=0 regions)
                drain_q = []         # groups past kt=15, awaiting qs 1-3
                drained = {}         # (b,qc) -> drained region count (0..6)
                otiles = {}          # (b,qc) -> [o_h0, o_h1]
                finalized = []       # groups finalized, in order
                stream_q = []        # groups awaiting/undergoing streaming

                LASTG = (B - 1, QC - 1)

                def alloc_group(g):
                    otiles[g] = [
                        ps_o.tile([128, QS * DV], fp32, tag=f"og{h}",
                                  name=f"og{h}")
                        for h in range(HL)
                    ]
                    if g == LASTG:
                        # two extra banks from the sm tag: four streaming
                        # regions, so only qs=2,3 drain after the last exp
                        otiles[g] += [
                            ps_sm.tile([128, 512], fp32, tag="sm",
                                       name=f"ogx{h}")
                            for h in range(HL)
                        ]

                def rmap(g, h, qs):
                    # region (h, qs) -> (tile, col0).  Last group: qs 0,2 in
                    # og_h (cols 0/65), qs 1,3 in the sm extras.
                    if g == LASTG:
                        if qs % 2 == 0:
                            return otiles[g][h], (qs // 2) * DV
                        return otiles[g][2 + h], (qs // 2) * DV
                    return otiles[g][h], qs * DV

                def group_done(g, skip_norms=False):
                    finalize(g[0], g[1], otiles[g],
                             via_pe=(g == (B - 1, QC - 1)),
                             skip_norms=skip_norms)
                    finalized.append(g)
                    otiles.pop(g)

                def flush(now_i, all_=False):
                    # 1) drain the head of drain_q: regions (qs>=1), two per
                    # call (region = 16 matmuls of F=65 ~ 0.43us each)
                    nreg = 1000 if all_ else 2
                    while drain_q and nreg > 0:
                        g = drain_q[0]
                        last = all_ and g == LASTG
                        qs0 = 2 if g == LASTG else 1
                        ndrain = HL * (QS - qs0)
                        d = drained.get(g, 0)
                        if last and d == 0:
                            # tail: the streamed regions are closed - norm
                            # them while PE drains the rest
                            for qs in range(qs0):
                                for h in range(HL):
                                    t_, c_ = rmap(g, h, qs)
                                    region_norm(g[0], g[1], t_, c_, h, qs,
                                                on_act=(h == 1))
                        take = min(ndrain - d, nreg)
                        for idx in range(d, d + take):
                            qs = qs0 + idx // HL
                            h = idx % HL
                            t_, c_ = rmap(g, h, qs)
                            for kt in range(KT):
                                vmm_one(t_, c_, pmap[g][kt], g[0],
                                        h, qs, kt)
                            if last:
                                region_norm(g[0], g[1], t_, c_, h, qs,
                                            on_act=(idx % 2 == 1))
                        drained[g] = d + take
                        nreg -= take
                        if drained[g] == ndrain:
                            drain_q.pop(0)
                            pmap.pop(g)
                            group_done(g, skip_norms=last)
                    # 2) stream qs=0 regions (head of stream_q) as kts and
                    # vaug rows become available; a group may only take the
                    # o banks once the previous group has fully vacated them
                    while stream_q:
                        g = stream_q[0]
                        if g not in otiles:
                            if otiles:
                                break    # banks still owned by prior group
                            alloc_group(g)
                        hi = min(max(pmap[g].keys()) + 1 if pmap[g] else 0,
                                 vrows[g[0]])
                        sk = stream_kts.get(g, 0)
                        nstream = 2 if g == LASTG else 1
                        for kt in range(sk, hi):
                            for qs in range(nstream):
                                for h in range(HL):
                                    t_, c_ = rmap(g, h, qs)
                                    vmm_one(t_, c_, pmap[g][kt], g[0],
                                            h, qs, kt)
                        stream_kts[g] = max(sk, hi)
                        if stream_kts[g] == KT:
                            drain_q.append(g)
                            stream_q.pop(0)
                            continue
                        break

                # ---- prologue: first score tile in column halves so the
                # exp stream starts as soon as the first xt piece lands ----
                warmup(48)
                proj_q_qs(0, 0, 0)
                proj_k(0, 0)
                S = ps_s.tile([128, 2 * 512], fp32, tag="S", name="S")
                for h in range(HL):
                    nc.tensor.matmul(
                        S[:, h * 512:h * 512 + 128],
                        qkT[h * D:(h + 1) * D, 0, 1, 0:128],
                        qkT[h * D:(h + 1) * D, 0, 0, 0:128],
                        start=True, stop=True)
                P0 = ppool.tile([128, 2 * 512], bf16, tag="P", name="P0")
                Sv0 = S.rearrange("p (h c) -> p h c", h=HL)
                Pv0 = P0.rearrange("p (h c) -> p h c", h=HL)
                nc.scalar.activation(Pv0[:, :, 0:128], Sv0[:, :, 0:128],
                                     EXP, scale=0.125)
                proj_q_qs(0, 0, 1)
                for h in range(HL):
                    nc.tensor.matmul(
                        S[:, h * 512 + 128:h * 512 + 256],
                        qkT[h * D:(h + 1) * D, 0, 1, 0:128],
                        qkT[h * D:(h + 1) * D, 0, 0, 128:256],
                        start=True, stop=True)
                nc.scalar.activation(Pv0[:, :, 128:256], Sv0[:, :, 128:256],
                                     EXP, scale=0.125)
                proj_k(0, 1)
                proj_q_qs(0, 0, 2)
                proj_q_qs(0, 0, 3)
                for h in range(HL):
                    nc.tensor.matmul(
                        S[:, h * 512 + 256:(h + 1) * 512],
                        qkT[h * D:(h + 1) * D, 0, 1, 0:128],
                        qkT[h * D:(h + 1) * D, 0, 0, 256:512],
                        start=True, stop=True)

                P_last = None
                for i, (b, qc, kt) in enumerate(seq):
                    if i == 0:
                        # first half emitted in the prologue on tile P0
                        P = P0
                        nc.scalar.activation(Pv0[:, :, 256:512],
                                             Sv0[:, :, 256:512],
                                             EXP, scale=0.125)
                    elif i == len(seq) - 1:
                        P = ppool.tile([128, 2 * 512], bf16, tag="P")
                        Svl = S.rearrange("p (h c) -> p h c", h=HL)
                        Pvl = P.rearrange("p (h c) -> p h c", h=HL)
                        nc.scalar.activation(Pvl[:, :, 0:256],
                                             Svl[:, :, 0:256],
                                             EXP, scale=0.125)
                        nc.scalar.activation(Pvl[:, :, 256:512],
                                             Svl[:, :, 256:512],
                                             EXP, scale=0.125)
                    else:
                        P = ppool.tile([128, 2 * 512], bf16, tag="P")
                        nc.scalar.activation(P[:], S[:], EXP, scale=0.125)
                    P_last = P
                    if i + 1 < len(seq):
                        S = sc(*seq[i + 1])
                    for fn in SCHED.get((b, qc, kt), []):
                        fn()
                    for (vb, r) in VROWS.get((b, qc, kt), []):
                        vrows[vb] = max(vrows[vb], r + 1)
                    if kt == 0:
                        stream_q.append((b, qc))
                        pmap[(b, qc)] = {}
                    pmap[(b, qc)][kt] = P
                    flush(i)

                # ---- tail ----
                while drain_q or stream_q:
                    flush(len(seq), all_=True)
                assert len(finalized) == B * QC, finalized
                # keep the PE p-state hot across the norm/transpose/dup
                # wait; chained on the last P tile so the scheduler cannot
                # hoist these out of the tail window
                warmup(0, src=P_last)
                for h in range(HL):
                    for n5 in range(2):
                        for s in range(2):
                            outproj_piece(1, h, n5, s, 2,
                                          tag=("S" if (n5 + s) % 2 else "sm"))

            _emit_all()

    nc.compile()
    return nc


def _get_runner():
    """Build + compile once; return a callable(in_maps) -> list of out dicts."""
    global _RUNNER
    if _RUNNER is not None:
        return _RUNNER

    import jax
    import concourse.mybir as mybir
    from concourse import bass2jax
    from jax.experimental.shard_map import shard_map
    from jax.sharding import Mesh, PartitionSpec

    nc = _build_nc()
    bass2jax.install_neuronx_cc_hook()

    partition_name = (nc.partition_id_tensor.name
                      if nc.partition_id_tensor else None)
    in_names, out_names, out_avals = [], [], []
    for alloc in nc.m.functions[0].allocations:
        if not isinstance(alloc, mybir.MemoryLocationSet):
            continue
        name = alloc.memorylocations[0].name
        if alloc.kind == "ExternalInput":
            if name != partition_name:
                in_names.append(name)
        elif alloc.kind == "ExternalOutput":
            out_names.append(name)
            out_avals.append(jax.core.ShapedArray(
                tuple(alloc.tensor_shape), mybir.dt.np(alloc.dtype)))

    n_params, n_outs = len(in_names), len(out_avals)
    all_names = in_names + out_names
    if partition_name is not None:
        all_names = all_names + [partition_name]

    def _body(*args):
        operands = list(args)
        if partition_name is not None:
            operands.append(bass2jax.partition_id_tensor())
        outs = bass2jax._bass_exec_p.bind(
            *operands,
            out_avals=tuple(out_avals),
            in_names=tuple(all_names),
            out_names=tuple(out_names),
            lowering_input_output_aliases=(),
            sim_require_finite=True,
            sim_require_nnan=True,
            nc=nc,
        )
        return tuple(outs)

    devices = jax.devices()[:N_CORES]
    mesh = Mesh(np.asarray(devices), ("core",))
    in_specs = (PartitionSpec("core"),) * (n_params + n_outs)
    out_specs = (PartitionSpec("core"),) * n_outs
    donate = tuple(range(n_params, n_params + n_outs))
    sharded = jax.jit(
        shard_map(_body, mesh=mesh, in_specs=in_specs, out_specs=out_specs,
                  check_rep=False),
        donate_argnums=donate, keep_unused=True,
    )

    def run(in_maps):
        concat_in = [
            np.concatenate([np.asarray(in_maps[c][nm]) for c in range(N_CORES)],
                           axis=0)
            for nm in in_names
        ]
        concat_zeros = [
            np.zeros((N_CORES * a.shape[0], *a.shape[1:]), a.dtype)
            for a in out_avals
        ]
        out_arrs = sharded(*concat_in, *concat_zeros)
        return [
            {nm: np.asarray(out_arrs[i]).reshape(N_CORES, *out_avals[i].shape)[c]
             for i, nm in enumerate(out_names)}
            for c in range(N_CORES)
        ]

    _RUNNER = run
    run._bench_parts = (sharded, mesh, in_names, out_names, out_avals,
                        n_params, _body)
    return run


def _make_bench(in_maps):
    """Device-resident benchmark closure: returns fn() that runs one
    execution with all inputs already on device (no donation)."""
    import jax
    from jax.experimental.shard_map import shard_map
    from jax.sharding import NamedSharding, PartitionSpec

    run = _get_runner()
    sharded, mesh, in_names, out_names, out_avals, n_params, _body = \
        run._bench_parts
    sh = NamedSharding(mesh, PartitionSpec("core"))

    nodonate = jax.jit(
        shard_map(_body, mesh=mesh,
                  in_specs=(PartitionSpec("core"),) * (n_params + len(out_avals)),
                  out_specs=(PartitionSpec("core"),) * len(out_avals),
                  check_rep=False),
        keep_unused=True,
    )
    concat_in = [
        np.concatenate([np.asarray(in_maps[c][nm]) for c in range(N_CORES)], axis=0)
        for nm in in_names
    ]
    concat_zeros = [
        np.zeros((N_CORES * a.shape[0], *a.shape[1:]), a.dtype) for a in out_avals
    ]
    dev_args = [jax.device_put(a, sh) for a in concat_in + concat_zeros]
    for a in dev_args:
        a.block_until_ready()

    def bench_once():
        outs = nodonate(*dev_args)
        for o in outs:
            o.block_until_ready()
        return outs

    bench_once.nodonate = nodonate
    bench_once.dev_args = dev_args
    return bench_once


def _prep_in_maps(x, W_qkv, b_qkv, W_out, b_out):
    bf = ml_dtypes.bfloat16
    xt = np.ascontiguousarray(
        x.reshape(B * T, E).T).astype(bf)                      # [E, B*T]
    wout = np.ascontiguousarray(
        W_out.reshape(8, 128, E).transpose(1, 0, 2).reshape(128, 8 * E)).astype(bf)
    bout = np.ascontiguousarray(
        np.broadcast_to(b_out.astype(np.float32)[None, :], (128, E)))

    in_maps = []
    for c in range(N_CORES):
        hs = [HL * c + i for i in range(HL)]
        qcols = np.concatenate(
            [W_qkv[:, 0 * E + h * D:0 * E + (h + 1) * D] for h in hs], axis=1)
        kcols = np.concatenate(
            [W_qkv[:, 1 * E + h * D:1 * E + (h + 1) * D] for h in hs], axis=1)
        wqk = np.ascontiguousarray(
            np.concatenate([qcols, kcols], axis=1)).astype(bf)  # [E, 256]
        zcol = np.zeros((E, 1), np.float32)
        wv = np.ascontiguousarray(np.concatenate(
            [arr for h in hs
             for arr in (W_qkv[:, 2 * E + h * D:2 * E + (h + 1) * D], zcol)],
            axis=1)).astype(bf)                                 # [E, 130]
        bq = np.concatenate([b_qkv[0 * E + h * D:0 * E + (h + 1) * D] for h in hs])
        bk = np.concatenate([b_qkv[1 * E + h * D:1 * E + (h + 1) * D] for h in hs])
        bqk = np.ascontiguousarray(
            np.stack([bq, bk], axis=1)).astype(np.float32)      # [128, 2]
        one = np.ones(1, np.float32)
        bvv = np.concatenate(
            [a for h in hs
             for a in (b_qkv[2 * E + h * D:2 * E + (h + 1) * D], one)])
        bv = np.ascontiguousarray(
            np.broadcast_to(bvv.astype(np.float32)[None, :], (128, 2 * DV)))
        in_maps.append({
            "xt": xt, "wqk": wqk, "wv": wv, "wout": wout,
            "bqk": bqk, "bv": bv, "bout": bout,
        })
    return in_maps


def kernel(x, W_qkv, b_qkv, W_out, b_out):
    x = np.asarray(x, dtype=np.float32)
    W_qkv = np.asarray(W_qkv, dtype=np.float32)
    b_qkv = np.asarray(b_qkv, dtype=np.float32)
    W_out = np.asarray(W_out, dtype=np.float32)
    b_out = np.asarray(b_out, dtype=np.float32)

    run = _get_runner()
    in_maps = _prep_in_maps(x, W_qkv, b_qkv, W_out, b_out)
    results = run(in_maps)

    out = np.empty((B, T, E), np.float32)
    for c in range(N_CORES):
        y = results[c]["y"]          # [B, HL, 128, E]
        for hl in range(HL):
            hg = HL * c + hl
            out[:, hg * 128:(hg + 1) * 128, :] = y[:, hl]
    return out


# revision 10
# speedup vs baseline: 1.2984x; 1.0008x over previous
"""Multi-head self-attention TRN2 kernel (B=2, T=2048, E=1024, H=16, D=64).

Sharding: tensor-parallel over heads - each of the 8 cores owns 2 heads.
Because the reference reshapes (B,H,T,D)->(B,T,E) with NO transpose, each
head's attention output maps to 128 complete contiguous rows of the
out_proj input, so the whole computation is embarrassingly parallel
across heads (no collectives).

Per-core pipeline (all matmuls bf16, accumulation fp32):
  1. qT/kT projections in [d, T] layout (heads stacked on partitions
     0-63 / 64-127); v in natural [T, d] layout augmented with a ones
     column (softmax denominator for free).
  2. scores^T tiles [kj=128, qi=512x2h] -> exp on ScalarE -> P^T bf16.
  3. attn@v in the cheap orientation: out o[q=128 partitions, d+1=65
     free] accumulated over kj tiles - PE cost 65 free/step instead of
     512 (the cost model charges output free size per accumulation
     step, so small-free x many-partitions wins 8x per instruction).
  4. normalize by per-partition (per-query) reciprocal on VectorE,
     downcast to bf16.
  5. o -> o^T via DMA xbar transposes (off the PE critical path), then
     shifted partition-duplicate copies build the [128=(d,j-parity), T]
     lhsT layout that performs the reference's "faithful reshape" for
     free in out_proj.
  6. out_proj: 8 accumulating matmuls per 512-column chunk.
"""

import numpy as np
import ml_dtypes

B, T, E, H, D = 2, 2048, 1024, 16, 64
N_CORES = 8
HL = H // N_CORES          # heads per core = 2
KP = E // 128              # 8 contraction partition-tiles
KT = T // 128              # 16 kj tiles
QC = T // 512              # 4 qi chunks of 512
QS = 4                     # q subtiles of 128 per chunk
DV = D + 1                 # v width incl. denominator ones column

_RUNNER = None


def _build_nc():
    import concourse.bacc as bacc
    import concourse.tile as tile
    import concourse.bass as bass
    import concourse.mybir as mybir

    from concourse import masks

    fp32 = mybir.dt.float32
    bf16 = mybir.dt.bfloat16
    ADD = mybir.AluOpType.add
    MULT = mybir.AluOpType.mult
    EXP = mybir.ActivationFunctionType.Exp
    COPY = mybir.ActivationFunctionType.Copy

    nc = bacc.Bacc("TRN2", target_bir_lowering=False, debug=False,
                   enable_asserts=True, num_devices=N_CORES)

    xt_d = nc.dram_tensor("xt", [E, B * T], bf16, kind="ExternalInput").ap()
    wqk_d = nc.dram_tensor("wqk", [E, 4 * D], bf16, kind="ExternalInput").ap()
    wv_d = nc.dram_tensor("wv", [E, 2 * DV], bf16, kind="ExternalInput").ap()
    wout_d = nc.dram_tensor("wout", [128, 8 * E], bf16, kind="ExternalInput").ap()
    bqk_d = nc.dram_tensor("bqk", [128, 2], fp32, kind="ExternalInput").ap()
    bv_d = nc.dram_tensor("bv", [128, 2 * DV], fp32, kind="ExternalInput").ap()
    bout_d = nc.dram_tensor("bout", [128, E], fp32, kind="ExternalInput").ap()
    y_d = nc.dram_tensor("y", [B, HL, 128, E], fp32, kind="ExternalOutput").ap()

    with tile.TileContext(nc) as tc:
        with (
            tc.tile_pool(name="const", bufs=1) as cpool,
            tc.tile_pool(name="ppool", bufs=26) as ppool,
            tc.tile_pool(name="npool", bufs=4) as npool,
            tc.tile_pool(name="ypool", bufs=4) as ypool,
            tc.tile_pool(name="ps_s", bufs=2, space=bass.MemorySpace.PSUM) as ps_s,
            tc.tile_pool(name="ps_o", bufs=1, space=bass.MemorySpace.PSUM) as ps_o,
            tc.tile_pool(name="ps_sm", bufs=2, space=bass.MemorySpace.PSUM) as ps_sm,
        ):
            # ---- constants / persistent tiles ----
            xt_sb = cpool.tile([128, KP, B * T], bf16, tag="xt")
            wqk_sb = cpool.tile([128, KP, 4 * D], bf16, tag="wqk")
            wv_sb = cpool.tile([128, KP, 2 * DV], bf16, tag="wv")
            wout_sb = cpool.tile([128, 8, E], bf16, tag="wout")
            bqk_sb = cpool.tile([128, 2], fp32, tag="bqk")
            bv_sb = cpool.tile([128, 2 * DV], fp32, tag="bv")
            bout_sb = cpool.tile([128, E], fp32, tag="bout")
            qkT = cpool.tile([128, B, 2, T], bf16, tag="qkT")
            vaug = cpool.tile([128, B, KT, 2 * DV], bf16, tag="vaug")
            # o natural layout, normalized bf16, 64 pad cols per head slot
            # (the pad transposes into oTd rows 64-127, which the shifted
            # dup copy then overwrites)
            # per-batch tensors (separate tags) so cross-batch false
            # dependencies cannot arise from coarse subtile tracking
            o_sb = [cpool.tile([128, KT, 192], bf16, tag=f"o_sb{bb}",
                               name=f"o_sb{bb}") for bb in range(B)]
            # o^T dup layout per head: rows 0-63 straight (written directly
            # by the xbar transpose), 64-127 shifted by one q (dup DMA)
            oTd = [cpool.tile([128, HL, T], bf16, tag=f"oTd{bb}",
                              name=f"oTd{bb}") for bb in range(B)]
            wub = cpool.tile([1, 128], bf16, tag="wub")
            ones1 = cpool.tile([1, 128], bf16, tag="ones1")
            bout_bf = cpool.tile([1, E], bf16, tag="bout_bf")
            ident = cpool.tile([128, 128], bf16, tag="ident")

            # warmup source (no DMA dependency); zero o_sb's pad lanes once
            nc.gpsimd.memset(wub[:], 0.0)
            nc.gpsimd.memset(ones1[:], 1.0)
            masks.make_identity(nc, ident[:])

            # ---- input DMAs: spread across the three HWDGE-ish rings so
            # queue overheads pipeline.  Transfers serialize on the shared
            # DMA engines; order = priority.  The first score tile needs
            # k cols + q[0:512], so those pieces go first, smallest first.
            wqk_r = wqk_d.rearrange("(a p) n -> p a n", p=128)
            xt_r = xt_d.rearrange("(a p) n -> p a n", p=128)
            nc.sync.dma_start(wqk_sb[:], wqk_r[:])
            # first 512 T-columns on the ACT ring (idle until exps start);
            # bigger slices keep the 1KB contiguous runs (no small-elem
            # DMA penalty)
            nc.scalar.dma_start(xt_sb[:, :, 0:256], xt_r[:, :, 0:256])
            nc.scalar.dma_start(xt_sb[:, :, 256:512], xt_r[:, :, 256:512])
            nc.scalar.dma_start(xt_sb[:, :, 512:1024], xt_r[:, :, 512:1024])
            nc.sync.dma_start(bqk_sb[:], bqk_d[:])
            nc.sync.dma_start(bv_sb[:], bv_d[:])
            # o_sb pad lanes zeroed on the gpsimd engine: doubles as a
            # delay so the Pool ring's chunk DMAs queue at the shared DMA
            # engines AFTER the ACT ring's critical first pieces
            for bb in range(B):
                nc.gpsimd.memset(o_sb[bb][:, :, 128:192], 0.0)
            # rest of x on the gpsimd SWDGE ring (engine otherwise idle);
            # wv mid-way (not needed until ~slot 11)
            for cc in range(2, B * T // 512):
                if cc == 4:
                    nc.gpsimd.dma_start(
                        wv_sb[:], wv_d.rearrange("(a p) n -> p a n", p=128))
                nc.gpsimd.dma_start(xt_sb[:, :, cc * 512:(cc + 1) * 512],
                                    xt_r[:, :, cc * 512:(cc + 1) * 512])

            # ---- PE warmup: keep the clock ramping from t~0 so the first
            # real matmuls run at full p-state.  src defaults to wub (no
            # deps -> the scheduler hoists them to kernel start); pass a
            # late-written AP to pin warmups into a late window instead.
            def warmup(n, src=None):
                s = wub if src is None else src
                for _ in range(n):
                    wt = ps_sm.tile([128, 128], fp32, tag="sm", name="wt")
                    nc.tensor.matmul(wt[:], s[0:1, 0:128], s[0:1, 0:128],
                                     start=True, stop=True)

            # ---- projection pieces ----
            def proj_q(b, n):
                # q^T chunk n: [128=(h0|h1)*d, 512]
                ps = ps_sm.tile([128, 512], fp32, tag="sm", name="psq")
                for k in range(KP):
                    nc.tensor.matmul(
                        ps[:],
                        wqk_sb[:, k, 0:128],
                        xt_sb[:, k, b * T + n * 512: b * T + (n + 1) * 512],
                        start=(k == 0), stop=(k == KP - 1),
                    )
                nc.vector.tensor_scalar(
                    qkT[:, b, 0, n * 512:(n + 1) * 512], ps[:],
                    bqk_sb[:, 0:1], None, op0=ADD,
                )

            def proj_q_half(b, n, hf):
                ps = ps_sm.tile([128, 512], fp32, tag="sm", name="psq")
                c0 = n * 512 + hf * 256
                for k in range(KP):
                    nc.tensor.matmul(
                        ps[:, 0:256],
                        wqk_sb[:, k, 0:128],
                        xt_sb[:, k, b * T + c0: b * T + c0 + 256],
                        start=(k == 0), stop=(k == KP - 1),
                    )
                nc.vector.tensor_scalar(
                    qkT[:, b, 0, c0:c0 + 256], ps[:, 0:256],
                    bqk_sb[:, 0:1], None, op0=ADD,
                )

            def proj_q_qs(b, n, qs):
                ps = ps_sm.tile([128, 512], fp32, tag="sm", name="psq")
                c0 = n * 512 + qs * 128
                for k in range(KP):
                    nc.tensor.matmul(
                        ps[:, 0:128],
                        wqk_sb[:, k, 0:128],
                        xt_sb[:, k, b * T + c0: b * T + c0 + 128],
                        start=(k == 0), stop=(k == KP - 1),
                    )
                nc.vector.tensor_scalar(
                    qkT[:, b, 0, c0:c0 + 128], ps[:, 0:128],
                    bqk_sb[:, 0:1], None, op0=ADD,
                )

            def proj_k(b, kt):
                # k^T tile kt: [128, 128]
                ps = ps_sm.tile([128, 512], fp32, tag="sm", name="psk")
                for k in range(KP):
                    nc.tensor.matmul(
                        ps[:, 0:128],
                        wqk_sb[:, k, 128:256],
                        xt_sb[:, k, b * T + kt * 128: b * T + (kt + 1) * 128],
                        start=(k == 0), stop=(k == KP - 1),
                    )
                nc.vector.tensor_scalar(
                    qkT[:, b, 1, kt * 128:(kt + 1) * 128], ps[:, 0:128],
                    bqk_sb[:, 1:2], None, op0=ADD,
                )

            def proj_v(b, r):
                # v natural [128, 2*DV] row tile r; W_v has zero columns at
                # the two "ones" slots and bv carries 1.0 there
                vp = ps_sm.tile([128, 512], fp32, tag="sm", name="vp")
                for k in range(KP):
                    nc.tensor.matmul(
                        vp[:, 0:2 * DV],
                        xt_sb[:, k, b * T + r * 128: b * T + (r + 1) * 128],
                        wv_sb[:, k, :],
                        start=(k == 0), stop=(k == KP - 1),
                    )
                nc.vector.tensor_tensor(
                    vaug[:, b, r, :], vp[:, 0:2 * DV], bv_sb[:], op=ADD,
                )

            def sc(b, qc, kt):
                # scores^T [kj=128, qi=512] per head, heads side by side
                S = ps_s.tile([128, 2 * 512], fp32, tag="S", name="S")
                for h in range(HL):
                    nc.tensor.matmul(
                        S[:, h * 512:(h + 1) * 512],
                        qkT[h * D:(h + 1) * D, b, 1, kt * 128:(kt + 1) * 128],
                        qkT[h * D:(h + 1) * D, b, 0, qc * 512:(qc + 1) * 512],
                        start=True, stop=True,
                    )
                return S

            def vmm_one(tile, c0, P, b, h, qs, kt):
                # o[q-subtile, d+1] += P^T[kj, q-sub]^T @ vaug[kj, d+1]
                nc.tensor.matmul(
                    tile[:, c0:c0 + DV],
                    P[:, h * 512 + qs * 128: h * 512 + (qs + 1) * 128],
                    vaug[:, b, kt, h * DV:(h + 1) * DV],
                    start=(kt == 0), stop=(kt == KT - 1),
                )

            def region_norm(b, qc, tile, c0, h, qs, on_act=False):
                # one region's reciprocal + normalize (tail interleaving)
                rdt = npool.tile([128, 8], fp32, tag="rd", name="rdt")
                nc.vector.reciprocal(
                    rdt[:, h * 4 + qs: h * 4 + qs + 1],
                    tile[:, c0 + D: c0 + DV])
                qt = qc * QS + qs
                if on_act:
                    nc.scalar.activation(
                        o_sb[b][:, qt, h * 64:(h + 1) * 64],
                        tile[:, c0: c0 + D], COPY,
                        scale=rdt[:, h * 4 + qs: h * 4 + qs + 1])
                else:
                    nc.vector.tensor_scalar(
                        o_sb[b][:, qt, h * 64:(h + 1) * 64],
                        tile[:, c0: c0 + D],
                        rdt[:, h * 4 + qs: h * 4 + qs + 1],
                        None, op0=MULT,
                    )

            def finalize(b, qc, ot, via_pe=False, skip_norms=False):
                # All norms (both heads) must precede the transposes: each
                # transpose reads a 128-col window spanning both head
                # slots.  via_pe: for the very last group the DMA
                # transpose+dup chain (~650ns queue overhead per hop) sits
                # exposed on the critical tail, so route it through PE
                # identity-matmuls + DVE/ACT copies instead (idle there).
                rd = npool.tile([128, 8], fp32, tag="rd", name="rd")
                if not skip_norms:
                    for h in range(HL):
                        nc.vector.reciprocal(
                            rd[:, h * 4:(h + 1) * 4].rearrange(
                                "p (a c) -> p a c", c=1),
                            ot[h].rearrange(
                                "p (q c) -> p q c", c=DV)[:, :, D:D + 1],
                        )
                    for qs in range(QS):
                        qt = qc * QS + qs
                        for h in range(HL):
                            nc.vector.tensor_scalar(
                                o_sb[b][:, qt, h * 64:(h + 1) * 64],
                                ot[h][:, qs * DV: qs * DV + D],
                                rd[:, h * 4 + qs: h * 4 + qs + 1],
                                None, op0=MULT,
                            )
                for qs in range(QS):
                    qt = qc * QS + qs
                    for h in range(HL):
                        # o^T; out rows 64-127 get the neighbour head slot /
                        # pad lanes, overwritten by the shifted dup below
                        if via_pe:
                            # bf16 [128,512] = same slot bytes as the sm tag
                            tp = ps_sm.tile([128, 512], bf16, tag="sm",
                                            name="tp")
                            nc.tensor.transpose(
                                tp[:, 0:128],
                                o_sb[b][:, qt, h * 64:h * 64 + 128],
                                ident[:])
                            nc.vector.tensor_copy(
                                oTd[b][0:128, h, qt * 128:(qt + 1) * 128],
                                tp[:, 0:128])
                        else:
                            nc.sync.dma_start_transpose(
                                oTd[b][0:128, h, qt * 128:(qt + 1) * 128],
                                o_sb[b][:, qt, h * 64:h * 64 + 128],
                            )
                for h in range(HL):
                    # shifted dup: via_pe does qc-1 + qc chunks; the DMA
                    # route defers to one whole-row dup after the last qc
                    # (fewer DMA producers -> fewer merged waits downstream)
                    chunks = []
                    if via_pe:
                        # cols 0..1023 were dup'd early (qc==2 below); the
                        # final group owns the last two 512-col pieces
                        chunks = [(1024, 512), (1536, 511)]
                    elif b == B - 1 and qc == QC - 2:
                        # early dup for the tail batch: sources complete
                        # through this group's transposes
                        chunks = [(0, 512), (512, 512)]
                    elif qc == QC - 1:
                        chunks.append((0, T - 1))
                    for c0, cw in chunks:
                        if via_pe:
                            dp = ps_sm.tile([128, 512], fp32, tag="sm",
                                            name="dp")
                            nc.tensor.matmul(
                                dp[0:64, 0:cw], ident[0:64, 0:64],
                                oTd[b][0:64, h, c0 + 1:c0 + cw + 1],
                                start=True, stop=True)
                            if h == 0:
                                nc.scalar.activation(
                                    oTd[b][64:128, h, c0:c0 + cw],
                                    dp[0:64, 0:cw], COPY)
                            else:
                                nc.vector.tensor_copy(
                                    oTd[b][64:128, h, c0:c0 + cw],
                                    dp[0:64, 0:cw])
                        else:
                            nc.sync.dma_start(
                                oTd[b][64:128, h, c0:c0 + cw],
                                oTd[b][0:64, h, c0 + 1:c0 + cw + 1])

            def outproj_piece(b, h, n5, s, split, tag="sm"):
                # one 1/split column chunk of out_proj for (b, h, n5); bias
                # folded in as a ones-row matmul so y DMAs straight out of
                # PSUM (no DVE hop on the critical tail)
                of2 = oTd[b][:, h, :].rearrange("p (t j) -> p j t", j=16)
                w = 512 // split
                c0 = n5 * 512 + s * w
                yp = ps_sm.tile([128, 512], fp32, tag=tag, name="yp") \
                    if tag == "sm" else \
                    ps_s.tile([128, 512], fp32, tag=tag, name="yp")
                for jj in range(8):
                    nc.tensor.matmul(
                        yp[:, 0:w],
                        of2[:, 2 * jj, :],
                        wout_sb[:, jj, c0:c0 + w],
                        start=(jj == 0), stop=False,
                    )
                nc.tensor.matmul(
                    yp[:, 0:w], ones1[0:1, :], bout_bf[0:1, c0:c0 + w],
                    start=False, stop=True,
                )
                # PSUM cannot feed a DMA directly; stage through SBUF on
                # DVE mid-stream / ACT at the tail (idle there)
                ys = ypool.tile([128, 512], fp32, tag="ys", name="ys")
                if b == 0 or (h + n5 + s) % 2 == 0:
                    nc.vector.tensor_copy(ys[:, 0:w], yp[:, 0:w])
                else:
                    nc.scalar.activation(ys[:, 0:w], yp[:, 0:w], COPY)
                # mid-stream y-writes ride the gpsimd SWDGE ring: keeps
                # their deep dependency chains off the SP ring's semaphore
                # window (an SP sem recycle once stalled the exp stream
                # 14us).  Tail y-writes (b=1) alternate SP/gpsimd.
                eng = nc.sync
                eng.dma_start(y_d[b, h, :, c0:c0 + w], ys[:, 0:w])

            # ---- filler schedule: all pieces ~0.43us (8 accumulating
            # matmuls of F<=130) so the score stream is never delayed by a
            # long chain sitting ahead of it in the PE queue.  Greedy
            # deadline placement, one piece per slot unless a deadline
            # forces more.
            SCHED = {}
            VROWS = {}

            def at_slot(slot, fn, vrow=None):
                key = (slot // 64, (slot % 64) // 16, slot % 16)
                SCHED.setdefault(key, []).append(fn)
                if vrow is not None:
                    VROWS.setdefault(key, []).append(vrow)

            def chunk_slot(c):
                # xt chunk c lands on the gpsimd ring ~2.9us apart; convert
                # to the exp-slot index from which a piece may read it
                if c == 0:
                    return -100
                return int((7814 + 2913 * c + 900 - 11090) / 1038) + 1

            pieces = []  # [deadline, earliest, fn, vrow]
            for b in range(B):
                for kt in range(KT):
                    if b == 0 and kt < 2:
                        continue  # prologue
                    c = (b * T + kt * 128) // 512
                    pieces.append(
                        [64 * b + kt - 2, chunk_slot(c),
                         (lambda bb, kk: lambda: proj_k(bb, kk))(b, kt),
                         None])
                for r in range(KT):
                    c = (b * T + r * 128) // 512
                    pieces.append(
                        [(13 if b == 0 else 50) + r // 2, chunk_slot(c),
                         (lambda bb, rr: lambda: proj_v(bb, rr))(b, r),
                         (b, r)])
                for n in range(QC):
                    if b == 0 and n == 0:
                        continue  # prologue
                    for qs in range(QS):
                        c = (b * T + n * 512 + qs * 128) // 512
                        pieces.append(
                            [64 * b + 16 * n - 3, chunk_slot(c),
                             (lambda bb, nn, qq:
                              lambda: proj_q_qs(bb, nn, qq))(b, n, qs),
                             None])
            for h in range(HL):
                for n5 in range(2):
                    for s in range(2):
                        pieces.append(
                            [100 + 3 * (4 * h + 2 * n5 + s), 95,
                             (lambda hh, nn, ss:
                              lambda: outproj_piece(0, hh, nn, ss, 2))(
                                  h, n5, s),
                             None])
            todo = sorted(pieces, key=lambda p: (p[0], p[1]))
            for slot in range(B * QC * KT):
                # keep group-boundary slots free: the finalize/norm chain
                # must reach the DVE queue head unimpeded.  Deadline-due
                # pieces are placed unconditionally (a qkT piece placed past
                # its deadline would be read stale by the score lookahead).
                boundary = slot % KT in (15, 0)
                n = 1 if boundary else 0
                for p in todo[:]:
                    if p[1] > slot:
                        continue
                    if p[0] <= slot:
                        at_slot(slot, p[2], p[3])
                        todo.remove(p)
                        n += 1
                    elif n == 0:
                        at_slot(slot, p[2], p[3])
                        todo.remove(p)
                        n += 1
                    else:
                        break
            assert not todo, [p[:2] for p in todo]

            # wout/bout on the gpsimd ring once startup DMA traffic is done
            at_slot(33, lambda: nc.gpsimd.dma_start(wout_sb[:], wout_d[:]))
            at_slot(35, lambda: nc.gpsimd.dma_start(bout_sb[:], bout_d[:]))
            at_slot(37, lambda: nc.vector.tensor_copy(
                bout_bf[0:1, :], bout_sb[0:1, :]))

            # ---- unified emission ----
            def _emit_all():
                seq = [(b, qc, kt) for b in range(B) for qc in range(QC)
                       for kt in range(KT)]
                vrows = {0: 0, 1: 0}
                # PSUM accumulation: one pending group per 2KB zero region
                # (bank).  Each o bank (og0=h0, og1=h1) streams its qs=0
                # region kt-incrementally through the group; regions qs=1-3
                # drain region-major after the group's last kt, two regions
                # per slot.
                pmap = {}            # (b,qc) -> {kt: P}
                stream_kts = {}      # (b,qc) -> kts streamed (qs=0 regions)
                drain_q = []         # groups past kt=15, awaiting qs 1-3
                drained = {}         # (b,qc) -> drained region count (0..6)
                otiles = {}          # (b,qc) -> [o_h0, o_h1]
                finalized = []       # groups finalized, in order
                stream_q = []        # groups awaiting/undergoing streaming

                LASTG = (B - 1, QC - 1)

                def alloc_group(g):
                    otiles[g] = [
                        ps_o.tile([128, QS * DV], fp32, tag=f"og{h}",
                                  name=f"og{h}")
                        for h in range(HL)
                    ]
                    if g == LASTG:
                        # two extra banks from the sm tag: four streaming
                        # regions, so only qs=2,3 drain after the last exp
                        otiles[g] += [
                            ps_sm.tile([128, 512], fp32, tag="sm",
                                       name=f"ogx{h}")
                            for h in range(HL)
                        ]

                def rmap(g, h, qs):
                    # region (h, qs) -> (tile, col0).  Last group: qs 0,2 in
                    # og_h (cols 0/65), qs 1,3 in the sm extras.
                    if g == LASTG:
                        if qs % 2 == 0:
                            return otiles[g][h], (qs // 2) * DV
                        return otiles[g][2 + h], (qs // 2) * DV
                    return otiles[g][h], qs * DV

                def group_done(g, skip_norms=False):
                    finalize(g[0], g[1], otiles[g],
                             via_pe=(g == (B - 1, QC - 1)),
                             skip_norms=skip_norms)
                    finalized.append(g)
                    otiles.pop(g)

                def flush(now_i, all_=False):
                    # 1) drain the head of drain_q: regions (qs>=1), two per
                    # call (region = 16 matmuls of F=65 ~ 0.43us each)
                    nreg = 1000 if all_ else 2
                    while drain_q and nreg > 0:
                        g = drain_q[0]
                        last = all_ and g == LASTG
                        qs0 = 2 if g == LASTG else 1
                        ndrain = HL * (QS - qs0)
                        d = drained.get(g, 0)
                        if last and d == 0:
                            # tail: the streamed regions are closed - norm
                            # them while PE drains the rest
                            for qs in range(qs0):
                                for h in range(HL):
                                    t_, c_ = rmap(g, h, qs)
                                    region_norm(g[0], g[1], t_, c_, h, qs,
                                                on_act=(h == 1))
                        take = min(ndrain - d, nreg)
                        for idx in range(d, d + take):
                            qs = qs0 + idx // HL
                            h = idx % HL
                            t_, c_ = rmap(g, h, qs)
                            for kt in range(KT):
                                vmm_one(t_, c_, pmap[g][kt], g[0],
                                        h, qs, kt)
                            if last:
                                region_norm(g[0], g[1], t_, c_, h, qs,
                                            on_act=(idx % 2 == 1))
                        drained[g] = d + take
                        nreg -= take
                        if drained[g] == ndrain:
                            drain_q.pop(0)
                            pmap.pop(g)
                            group_done(g, skip_norms=last)
                    # 2) stream qs=0 regions (head of stream_q) as kts and
                    # vaug rows become available; a group may only take the
                    # o banks once the previous group has fully vacated them
                    while stream_q:
                        g = stream_q[0]
                        if g not in otiles:
                            if otiles:
                                break    # banks still owned by prior group
                            alloc_group(g)
                        hi = min(max(pmap[g].keys()) + 1 if pmap[g] else 0,
                                 vrows[g[0]])
                        sk = stream_kts.get(g, 0)
                        nstream = 2 if g == LASTG else 1
                        for kt in range(sk, hi):
                            for qs in range(nstream):
                                for h in range(HL):
                                    t_, c_ = rmap(g, h, qs)
                                    vmm_one(t_, c_, pmap[g][kt], g[0],
                                            h, qs, kt)
                        stream_kts[g] = max(sk, hi)
                        if stream_kts[g] == KT:
                            drain_q.append(g)
                            stream_q.pop(0)
                            continue
                        break

                # ---- prologue: first score tile in column halves so the
                # exp stream starts as soon as the first xt piece lands ----
                warmup(48)
                proj_q_qs(0, 0, 0)
                proj_k(0, 0)
                S = ps_s.tile([128, 2 * 512], fp32, tag="S", name="S")
                for h in range(HL):
                    nc.tensor.matmul(
                        S[:, h * 512:h * 512 + 128],
                        qkT[h * D:(h + 1) * D, 0, 1, 0:128],
                        qkT[h * D:(h + 1) * D, 0, 0, 0:128],
                        start=True, stop=True)
                P0 = ppool.tile([128, 2 * 512], bf16, tag="P", name="P0")
                Sv0 = S.rearrange("p (h c) -> p h c", h=HL)
                Pv0 = P0.rearrange("p (h c) -> p h c", h=HL)
                nc.scalar.activation(Pv0[:, :, 0:128], Sv0[:, :, 0:128],
                                     EXP, scale=0.125)
                proj_q_qs(0, 0, 1)
                for h in range(HL):
                    nc.tensor.matmul(
                        S[:, h * 512 + 128:h * 512 + 256],
                        qkT[h * D:(h + 1) * D, 0, 1, 0:128],
                        qkT[h * D:(h + 1) * D, 0, 0, 128:256],
                        start=True, stop=True)
                nc.scalar.activation(Pv0[:, :, 128:256], Sv0[:, :, 128:256],
                                     EXP, scale=0.125)
                proj_k(0, 1)
                proj_q_qs(0, 0, 2)
                proj_q_qs(0, 0, 3)
                for h in range(HL):
                    nc.tensor.matmul(
                        S[:, h * 512 + 256:(h + 1) * 512],
                        qkT[h * D:(h + 1) * D, 0, 1, 0:128],
                        qkT[h * D:(h + 1) * D, 0, 0, 256:512],
                        start=True, stop=True)

                P_last = None
                for i, (b, qc, kt) in enumerate(seq):
                    if i == 0:
                        # first half emitted in the prologue on tile P0
                        P = P0
                        nc.scalar.activation(Pv0[:, :, 256:512],
                                             Sv0[:, :, 256:512],
                                             EXP, scale=0.125)
                    elif i == len(seq) - 1:
                        P = ppool.tile([128, 2 * 512], bf16, tag="P")
                        Svl = S.rearrange("p (h c) -> p h c", h=HL)
                        Pvl = P.rearrange("p (h c) -> p h c", h=HL)
                        nc.scalar.activation(Pvl[:, :, 0:256],
                                             Svl[:, :, 0:256],
                                             EXP, scale=0.125)
                        nc.scalar.activation(Pvl[:, :, 256:512],
                                             Svl[:, :, 256:512],
                                             EXP, scale=0.125)
                    else:
                        P = ppool.tile([128, 2 * 512], bf16, tag="P")
                        nc.scalar.activation(P[:], S[:], EXP, scale=0.125)
                    P_last = P
                    if i + 1 < len(seq):
                        S = sc(*seq[i + 1])
                    for fn in SCHED.get((b, qc, kt), []):
                        fn()
                    for (vb, r) in VROWS.get((b, qc, kt), []):
                        vrows[vb] = max(vrows[vb], r + 1)
                    if kt == 0:
                        stream_q.append((b, qc))
                        pmap[(b, qc)] = {}
                    pmap[(b, qc)][kt] = P
                    flush(i)

                # ---- tail ----
                while drain_q or stream_q:
                    flush(len(seq), all_=True)
                assert len(finalized) == B * QC, finalized
                # keep the PE p-state hot across the norm/transpose/dup
                # wait; chained on the last P tile so the scheduler cannot
                # hoist these out of the tail window
                warmup(0, src=P_last)
                for h in range(HL):
                    for n5 in range(2):
                        for s in range(2):
                            outproj_piece(1, h, n5, s, 2,
                                          tag=("S" if (n5 + s) % 2 else "sm"))

            _emit_all()

    nc.compile()
    return nc


def _get_runner():
    """Build + compile once; return a callable(in_maps) -> list of out dicts."""
    global _RUNNER
    if _RUNNER is not None:
        return _RUNNER

    import jax
    import concourse.mybir as mybir
    from concourse import bass2jax
    from jax.experimental.shard_map import shard_map
    from jax.sharding import Mesh, PartitionSpec

    nc = _build_nc()
    bass2jax.install_neuronx_cc_hook()

    partition_name = (nc.partition_id_tensor.name
                      if nc.partition_id_tensor else None)
    in_names, out_names, out_avals = [], [], []
    for alloc in nc.m.functions[0].allocations:
        if not isinstance(alloc, mybir.MemoryLocationSet):
            continue
        name = alloc.memorylocations[0].name
        if alloc.kind == "ExternalInput":
            if name != partition_name:
                in_names.append(name)
        elif alloc.kind == "ExternalOutput":
            out_names.append(name)
            out_avals.append(jax.core.ShapedArray(
                tuple(alloc.tensor_shape), mybir.dt.np(alloc.dtype)))

    n_params, n_outs = len(in_names), len(out_avals)
    all_names = in_names + out_names
    if partition_name is not None:
        all_names = all_names + [partition_name]

    def _body(*args):
        operands = list(args)
        if partition_name is not None:
            operands.append(bass2jax.partition_id_tensor())
        outs = bass2jax._bass_exec_p.bind(
            *operands,
            out_avals=tuple(out_avals),
            in_names=tuple(all_names),
            out_names=tuple(out_names),
            lowering_input_output_aliases=(),
            sim_require_finite=True,
            sim_require_nnan=True,
            nc=nc,
        )
        return tuple(outs)

    devices = jax.devices()[:N_CORES]
    mesh = Mesh(np.asarray(devices), ("core",))
    in_specs = (PartitionSpec("core"),) * (n_params + n_outs)
    out_specs = (PartitionSpec("core"),) * n_outs
    donate = tuple(range(n_params, n_params + n_outs))
    sharded = jax.jit(
        shard_map(_body, mesh=mesh, in_specs=in_specs, out_specs=out_specs,
                  check_rep=False),
        donate_argnums=donate, keep_unused=True,
    )

    def run(in_maps):
        concat_in = [
            np.concatenate([np.asarray(in_maps[c][nm]) for c in range(N_CORES)],
                           axis=0)
            for nm in in_names
        ]
        concat_zeros = [
            np.zeros((N_CORES * a.shape[0], *a.shape[1:]), a.dtype)
            for a in out_avals
        ]
        out_arrs = sharded(*concat_in, *concat_zeros)
        return [
            {nm: np.asarray(out_arrs[i]).reshape(N_CORES, *out_avals[i].shape)[c]
             for i, nm in enumerate(out_names)}
            for c in range(N_CORES)
        ]

    _RUNNER = run
    run._bench_parts = (sharded, mesh, in_names, out_names, out_avals,
                        n_params, _body)
    return run


def _make_bench(in_maps):
    """Device-resident benchmark closure: returns fn() that runs one
    execution with all inputs already on device (no donation)."""
    import jax
    from jax.experimental.shard_map import shard_map
    from jax.sharding import NamedSharding, PartitionSpec

    run = _get_runner()
    sharded, mesh, in_names, out_names, out_avals, n_params, _body = \
        run._bench_parts
    sh = NamedSharding(mesh, PartitionSpec("core"))

    nodonate = jax.jit(
        shard_map(_body, mesh=mesh,
                  in_specs=(PartitionSpec("core"),) * (n_params + len(out_avals)),
                  out_specs=(PartitionSpec("core"),) * len(out_avals),
                  check_rep=False),
        keep_unused=True,
    )
    concat_in = [
        np.concatenate([np.asarray(in_maps[c][nm]) for c in range(N_CORES)], axis=0)
        for nm in in_names
    ]
    concat_zeros = [
        np.zeros((N_CORES * a.shape[0], *a.shape[1:]), a.dtype) for a in out_avals
    ]
    dev_args = [jax.device_put(a, sh) for a in concat_in + concat_zeros]
    for a in dev_args:
        a.block_until_ready()

    def bench_once():
        outs = nodonate(*dev_args)
        for o in outs:
            o.block_until_ready()
        return outs

    bench_once.nodonate = nodonate
    bench_once.dev_args = dev_args
    return bench_once


def _prep_in_maps(x, W_qkv, b_qkv, W_out, b_out):
    bf = ml_dtypes.bfloat16
    xt = np.ascontiguousarray(
        x.reshape(B * T, E).T).astype(bf)                      # [E, B*T]
    wout = np.ascontiguousarray(
        W_out.reshape(8, 128, E).transpose(1, 0, 2).reshape(128, 8 * E)).astype(bf)
    bout = np.ascontiguousarray(
        np.broadcast_to(b_out.astype(np.float32)[None, :], (128, E)))

    in_maps = []
    for c in range(N_CORES):
        hs = [HL * c + i for i in range(HL)]
        qcols = np.concatenate(
            [W_qkv[:, 0 * E + h * D:0 * E + (h + 1) * D] for h in hs], axis=1)
        kcols = np.concatenate(
            [W_qkv[:, 1 * E + h * D:1 * E + (h + 1) * D] for h in hs], axis=1)
        wqk = np.ascontiguousarray(
            np.concatenate([qcols, kcols], axis=1)).astype(bf)  # [E, 256]
        zcol = np.zeros((E, 1), np.float32)
        wv = np.ascontiguousarray(np.concatenate(
            [arr for h in hs
             for arr in (W_qkv[:, 2 * E + h * D:2 * E + (h + 1) * D], zcol)],
            axis=1)).astype(bf)                                 # [E, 130]
        bq = np.concatenate([b_qkv[0 * E + h * D:0 * E + (h + 1) * D] for h in hs])
        bk = np.concatenate([b_qkv[1 * E + h * D:1 * E + (h + 1) * D] for h in hs])
        bqk = np.ascontiguousarray(
            np.stack([bq, bk], axis=1)).astype(np.float32)      # [128, 2]
        one = np.ones(1, np.float32)
        bvv = np.concatenate(
            [a for h in hs
             for a in (b_qkv[2 * E + h * D:2 * E + (h + 1) * D], one)])
        bv = np.ascontiguousarray(
            np.broadcast_to(bvv.astype(np.float32)[None, :], (128, 2 * DV)))
        in_maps.append({
            "xt": xt, "wqk": wqk, "wv": wv, "wout": wout,
            "bqk": bqk, "bv": bv, "bout": bout,
        })
    return in_maps


def kernel(x, W_qkv, b_qkv, W_out, b_out):
    x = np.asarray(x, dtype=np.float32)
    W_qkv = np.asarray(W_qkv, dtype=np.float32)
    b_qkv = np.asarray(b_qkv, dtype=np.float32)
    W_out = np.asarray(W_out, dtype=np.float32)
    b_out = np.asarray(b_out, dtype=np.float32)

    run = _get_runner()
    in_maps = _prep_in_maps(x, W_qkv, b_qkv, W_out, b_out)
    results = run(in_maps)

    out = np.empty((B, T, E), np.float32)
    for c in range(N_CORES):
        y = results[c]["y"]          # [B, HL, 128, E]
        for hl in range(HL):
            hg = HL * c + hl
            out[:, hg * 128:(hg + 1) * 128, :] = y[:, hl]
    return out
